# revision 1
# baseline (speedup 1.0000x reference)
# Trainium2 Bass kernel for nn_CPPN (gnn_message_passing), 8-core SPMD.
#
# Sharding:
#   - Node-MLP weights (s2v_W1/W2, vn_W, sn_W, fus_W) sharded over the 2048
#     col/row dim (256 per core); stitched with one AllReduce (h2) and three
#     AllGathers (z_v, z_s, vu partials).
#   - Visual edge MLP (200x200 pairwise rows) sharded over i: 25 rows/core,
#     AllGather of the resulting edge rows.  Per-core column selection is via
#     a host-supplied one-hot matrix (SPMD program is identical on all cores;
#     only input data differs).
#   - img_feat batch sharded 256 rows/core for the final prob matmul; host
#     concatenates per-core outputs.
# Layout: activations are d-major [D on partitions (16x128), 200 on free].
# Heavy matmuls bf16; stats/broadcast matmuls f32r.  The reference's second
# edge_update (semantic) is dead code w.r.t. prob and is skipped.

import sys

sys.path.insert(0, "/opt/trn_rl_repo")

import numpy as np
import ml_dtypes

import concourse.bass as bass
import concourse.bacc as bacc
import concourse.tile as tile
from concourse import mybir
from concourse.bass_utils import run_bass_kernel_spmd
from concourse.masks import make_identity

F32 = mybir.dt.float32
F32R = mybir.dt.float32r
BF16 = mybir.dt.bfloat16
AF = mybir.ActivationFunctionType
OP = mybir.AluOpType
AX = mybir.AxisListType

NCORES = 8
N = 200
S = 312
D = 2048
H = 128
KEXP = 3
B = 2048
DSH = D // NCORES      # 256
ISH = N // NCORES      # 25
BSH = B // NCORES      # 256
EPS = 1e-5
NT = D // 128          # 16
S_KT = [128, 128, 56]
N_MT = ((0, 128), (128, 72))
G_I = 2                # i-group size for edge t-gen batching

_BUILT = None


def _rep(ap_src, dims):
    """Rebuild AP with explicit free dims [[step,count],...] after partition."""
    return bass.AP(tensor=ap_src.tensor, offset=ap_src.offset,
                   ap=[ap_src.ap[0]] + dims)


def build(debug=False):
    nc = bacc.Bacc("TRN2", target_bir_lowering=False, debug=False,
                   num_devices=NCORES)
    d = {}

    def din(name, shape, dt):
        d[name] = nc.dram_tensor(name, shape, dt, kind="ExternalInput")

    din("attrT", [S, N], F32)
    din("attrTb", [S, N], BF16)
    din("centT", [S, KEXP], F32)
    din("expW", [KEXP, S, D], F32R)
    din("expBT", [D, KEXP], F32)
    din("w1s", [D, DSH], F32R)
    din("bnG", [128, 2], F32)
    din("bnB", [128, 2], F32)
    din("w2s", [DSH, D], F32R)
    din("b2o8", [128, NT], F32)
    din("vnWs", [D, DSH], F32R)
    din("vnbs", [128, 2], F32)
    din("snWs", [D, DSH], F32R)
    din("snbs", [128, 2], F32)
    din("veW1", [D, H], BF16)
    din("veb1", [H, 1], F32)
    din("veW2", [H, 1], BF16)
    din("veb2", [ISH, 1], F32)
    din("fusWs", [D, DSH], F32R)
    din("fusUs", [128, 2], F32R)
    din("imgT", [D, BSH], BF16)
    din("selv", [N, ISH], BF16)
    prob_out = nc.dram_tensor("prob", [BSH, N], F32, kind="ExternalOutput")
    dbg = {}
    if debug:
        def dout(name, shape, dt):
            dbg[name] = nc.dram_tensor("dbg_" + name, shape, dt,
                                       kind="ExternalOutput")
        dout("SP", [D, N], F32R)
        dout("a1", [2 * 128, N], F32R)
        dout("h2", [D, N], F32R)
        dout("PVP", [D, N], F32R)
        dout("vedge", [N, N], F32R)
        dout("sedge", [N, N], F32R)
        dout("ybf", [D, N], F32R)
        dout("zv", [D, N], F32R)
        dout("VP2", [D, N], BF16)
        dout("AT", [128, N], F32)
        dout("Amy", [128, ISH], F32)
        dout("xmyn", [ISH, D], BF16)
        dout("cur", [ISH, N], F32)
        dout("ve2", [N, N], F32R)
        dout("SP2", [D, N], F32R)
        dout("vuf", [1, 512], F32)
        dout("alpha", [1, 512], F32)

    with tile.TileContext(nc) as tc:
        import contextlib
        with contextlib.ExitStack() as ctx, \
                nc.allow_low_precision(reason="bf16 PE transposes (no accum)"):
            _emit(ctx, nc, tc, d, prob_out, dbg)
    nc.compile()
    return nc


def _emit(ctx, nc, tc, d, prob_out, dbg=None):
    dbg = dbg or {}

    def dtap16(key, tiles):     # 16 d-major tiles -> [D, N] dram
        if key in dbg:
            for m in range(NT):
                nc.sync.dma_start(out=dbg[key].ap()[m * 128:(m + 1) * 128, :],
                                  in_=tiles[m][:, 0:N])

    def dtap(key, ap_in, row0=0, rows=None):
        if key in dbg:
            o = dbg[key].ap()
            nc.sync.dma_start(out=o[row0:row0 + (rows or o.shape[0]), :],
                              in_=ap_in)

    pw = ctx.enter_context(tc.tile_pool(name="wts", bufs=1))
    pa = ctx.enter_context(tc.tile_pool(name="acts", bufs=1))
    pt = ctx.enter_context(tc.tile_pool(name="tmp", bufs=2))
    pt1 = ctx.enter_context(tc.tile_pool(name="rows", bufs=1))
    psh = ctx.enter_context(tc.tile_pool(name="sh1k", bufs=1))
    pstr = ctx.enter_context(tc.tile_pool(name="stream", bufs=1))
    pdram = ctx.enter_context(tc.tile_pool(name="dram", bufs=1, space="DRAM"))
    pp = ctx.enter_context(tc.tile_pool(name="ps_mm", bufs=4, space="PSUM"))
    pph = ctx.enter_context(tc.tile_pool(name="ps_hold", bufs=2, space="PSUM"))
    pps = ctx.enter_context(tc.tile_pool(name="ps_st", bufs=2, space="PSUM"))

    dma = nc.sync.dma_start
    GRP = [list(range(NCORES))]

    def ppt(shape, name, dt=F32):
        return pp.tile(shape, dt, name=name, tag="mm")

    def ppt_h(shape, name, dt=F32):
        return pph.tile(shape, dt, name=name, tag="hold")

    def ppt_s(shape, name, dt=F32):
        return pps.tile(shape, dt, name=name, tag="stat")

    def sh1k(name, tagid, dt=F32R, shape=None):
        return psh.tile(shape or [128, 256], dt, name=name, tag=f"sh{tagid}")

    # ---------- constants ----------
    ident_b = pa.tile([128, 128], BF16, name="ident_b")
    make_identity(nc, ident_b)
    ident_f = pt1.tile([128, 128], F32, name="ident_f", tag="ident_f")
    make_identity(nc, ident_f)
    ident_r = pa.tile([128, 128], F32R, name="ident_r")
    nc.vector.tensor_copy(out=ident_r, in_=ident_f)
    ones_r = pa.tile([128, 1], F32R, name="ones_r")
    nc.vector.memset(ones_r[:].bitcast(F32), 1.0)
    ones1_r = pa.tile([1, 128], F32R, name="ones1_r")
    nc.vector.memset(ones1_r[:].bitcast(F32), 1.0)
    ones8_r = pa.tile([8, 1], F32R, name="ones8_r")
    nc.vector.memset(ones8_r[:].bitcast(F32), 1.0)
    epsc = pa.tile([128, 1], F32, name="epsc")
    nc.vector.memset(epsc, EPS)

    # ---------- inputs -> SBUF ----------
    at_f, at_b = [], []
    off = 0
    for kt, ksz in enumerate(S_KT):
        tf = pa.tile([128, N], F32, name=f"at_f{kt}", tag=f"atf{kt}")
        tb = pa.tile([128, N], BF16, name=f"at_b{kt}")
        dma(out=tf[0:ksz, :], in_=d["attrT"].ap()[off:off + ksz, :])
        dma(out=tb[0:ksz, :], in_=d["attrTb"].ap()[off:off + ksz, :])
        at_f.append(tf)
        at_b.append(tb)
        off += ksz
    centT_t = []
    off = 0
    for kt, ksz in enumerate(S_KT):
        t = pa.tile([128, KEXP], F32, name=f"centT{kt}")
        dma(out=t[0:ksz, :], in_=d["centT"].ap()[off:off + ksz, :])
        centT_t.append(t)
        off += ksz
    expbt = []
    for m in range(NT):
        t = pw.tile([128, KEXP], F32, name=f"expbt{m}")
        dma(out=t, in_=d["expBT"].ap()[m * 128:(m + 1) * 128, :])
        expbt.append(t)

    def load16(name, key, ncols, dt=F32R, tagbase=None):
        out = []
        for kt in range(NT):
            tg = f"{tagbase}{kt}" if tagbase else ""
            t = pw.tile([128, ncols], dt, name=f"{name}{kt}", tag=tg)
            dma(out=t, in_=d[key].ap()[kt * 128:(kt + 1) * 128, :])
            out.append(t)
        return out

    w1s_t = load16("w1s", "w1s", DSH)
    vnWs_t = load16("vnWs", "vnWs", DSH)
    snWs_t = load16("snWs", "snWs", DSH, tagbase="vnWs")
    fusWs_t = load16("fusWs", "fusWs", DSH, tagbase="w1s")
    # imgT packed: 2 tiles [128, 8*256] bf16; lhsT slice kt -> [:, (kt%8)*256+...]
    imgT_p = []
    for hh in range(2):
        t = pw.tile([128, 8 * BSH], BF16, name=f"imgTp{hh}")
        src_ap = d["imgT"].ap()[hh * 1024:(hh + 1) * 1024, :].rearrange(
            "(kt p) b -> p kt b", p=128)
        nc.sync.dma_start(out=t[:].rearrange("p (kt b) -> p kt b", kt=8),
                          in_=src_ap)
        imgT_p.append(t)

    def imgT_sl(kt, bt):
        return imgT_p[kt // 8][:, (kt % 8) * BSH + bt * 128:
                               (kt % 8) * BSH + (bt + 1) * 128]

    # veW1 packed single [128, 2048] bf16
    veW1_a = pw.tile([128, D], BF16, name="veW1_a")
    nc.sync.dma_start(out=veW1_a[:].rearrange("p (kt h) -> p kt h", kt=NT),
                      in_=d["veW1"].ap().rearrange("(kt p) h -> p kt h", p=128))

    def veW1_sl(kt):
        return veW1_a[:, kt * H:(kt + 1) * H]

    def loadmat(name, key, rows, cols, dt=F32):
        t = pw.tile([rows, cols], dt, name=name)
        dma(out=t, in_=d[key].ap()[:, :])
        return t

    bnG_t = loadmat("bnG_t", "bnG", 128, 2)
    bnB_t = loadmat("bnB_t", "bnB", 128, 2)
    vnbs_t = loadmat("vnbs_t", "vnbs", 128, 2)
    snbs_t = loadmat("snbs_t", "snbs", 128, 2)
    veb1_t = loadmat("veb1_t", "veb1", H, 1)
    veb2_t = loadmat("veb2_t", "veb2", ISH, 1)
    b2o8_t = loadmat("b2o8_t", "b2o8", 128, NT)
    veW2_t = loadmat("veW2_t", "veW2", H, 1, BF16)
    fusUs_t = loadmat("fusUs_t", "fusUs", 128, 2, F32R)
    selv_t = []
    for jb, (j0, jw) in enumerate(N_MT):
        t = pw.tile([128, ISH], BF16, name=f"selv{jb}")
        dma(out=t[0:jw, :], in_=d["selv"].ap()[j0:j0 + jw, :])
        selv_t.append(t)

    # collective bounce buffers
    ar_h2_in = pdram.tile([D, N], F32R, name="ar_h2_in")
    ar_h2_out = pdram.tile([D, N], F32R, addr_space="Shared", name="ar_h2_out")
    ag_zv_in = pdram.tile([DSH, N], F32R, name="ag_zv_in")
    ag_zv_out = pdram.tile([D, N], F32R, addr_space="Shared", name="ag_zv_out")
    ag_zs_in = pdram.tile([DSH, N], F32R, name="ag_zs_in")
    ag_zs_out = pdram.tile([D, N], F32R, addr_space="Shared", name="ag_zs_out")
    ag_ve_in = pdram.tile([ISH, N], F32R, name="ag_ve_in")
    ag_ve_out = pdram.tile([N, N], F32R, addr_space="Shared", name="ag_ve_out")
    ag_vu_in = pdram.tile([1, 512], F32R, name="ag_vu_in")
    ag_vu_out = pdram.tile([NCORES, 512], F32R, addr_space="Shared",
                           name="ag_vu_out")

    # =================================================================
    # P1: CooperationModule -> SP (f32r padded, d-major)
    # =================================================================
    offT = []
    for k in range(KEXP):
        row = []
        for kt, ksz in enumerate(S_KT):
            t = sh1k(f"offT{k}_{kt}", 3 * k + kt)
            nc.vector.memset(t[:, N:256].bitcast(F32), 0.0)
            nc.vector.tensor_scalar(
                out=t[0:ksz, 0:N], in0=at_f[kt][0:ksz, :],
                scalar1=centT_t[kt][0:ksz, k:k + 1], scalar2=None,
                op0=OP.subtract)
            row.append(t)
        offT.append(row)

    SP = [pa.tile([128, 256], F32R, name=f"SP{m}") for m in range(NT)]
    for m in range(NT):
        nc.vector.memset(SP[m][:, N:256].bitcast(F32), 0.0)
    for k in range(KEXP):
        for half in range(4):
            wst = []
            for kt, ksz in enumerate(S_KT):
                w = pstr.tile([128, D // 4], F32R, name=f"expw_st{kt}",
                              tag=f"str{kt}")
                dma(out=w[0:ksz, :],
                    in_=d["expW"].ap()[k, sum(S_KT[:kt]):sum(S_KT[:kt]) + ksz,
                                       half * 512:(half + 1) * 512])
                wst.append(w)
            for mh in range(4):
                m = half * 4 + mh
                ps = ppt([128, 256], "ps_exp")
                for kt, ksz in enumerate(S_KT):
                    nc.tensor.matmul(ps, wst[kt][0:ksz, mh * 128:(mh + 1) * 128],
                                     offT[k][kt][0:ksz, 0:256],
                                     start=(kt == 0),
                                     stop=(kt == len(S_KT) - 1))
                if k == 0:
                    nc.vector.tensor_scalar(out=SP[m][:, 0:N], in0=ps[:, 0:N],
                                            scalar1=expbt[m][:, 0:1],
                                            scalar2=0.0,
                                            op0=OP.add, op1=OP.max)
                else:
                    r1 = pt1.tile([128, N], F32, name="exr", tag="exr")
                    nc.scalar.activation(r1, ps[:, 0:N], AF.Relu,
                                         bias=expbt[m][:, k:k + 1], scale=1.0)
                    nc.vector.tensor_tensor(SP[m][:, 0:N], SP[m][:, 0:N], r1,
                                            OP.add)
    dtap16("SP", SP)

    # =================================================================
    # P2: semantic2visual -> AllReduce(h2)
    # =================================================================
    a1 = []
    for m2 in range(2):
        ps = ppt([128, 256], "ps_h1")
        for kt in range(NT):
            nc.tensor.matmul(ps, w1s_t[kt][:, m2 * 128:(m2 + 1) * 128],
                             SP[kt][:, 0:256], start=(kt == 0),
                             stop=(kt == NT - 1))
        st6 = pt.tile([128, 6], F32, name="bn_st", tag="bn_st")
        mv = pt.tile([128, 2], F32, name="bn_mv", tag="bn_mv")
        nc.vector.bn_stats(out=st6, in_=ps[:, 0:N])
        nc.vector.bn_aggr(out=mv, in_=st6)
        sd = pt.tile([128, 1], F32, name="bn_sd", tag="bn_sd")
        nc.scalar.activation(sd, mv[:, 1:2], AF.Sqrt, bias=epsc[:, 0:1],
                             scale=1.0)
        rs = pt.tile([128, 1], F32, name="bn_rs", tag="bn_rs")
        nc.vector.reciprocal(out=rs, in_=sd)
        Av = pt.tile([128, 1], F32, name="bn_A", tag="bn_A")
        nc.vector.tensor_tensor(Av, rs, bnG_t[:, m2:m2 + 1], OP.mult)
        Bt = pt.tile([128, 1], F32, name="bn_Bt", tag="bn_Bt")
        nc.vector.tensor_tensor(Bt, mv[:, 0:1], Av, OP.mult)
        Bv = pt.tile([128, 1], F32, name="bn_Bv", tag="bn_Bv")
        nc.vector.tensor_tensor(Bv, bnB_t[:, m2:m2 + 1], Bt, OP.subtract)
        t1 = pt.tile([128, N], F32, name="h1_t1", tag="zt1")
        nc.vector.tensor_scalar(out=t1, in0=ps[:, 0:N], scalar1=Av,
                                scalar2=Bv, op0=OP.mult, op1=OP.add)
        a1m = pa.tile([128, 256], F32R, name=f"a1_{m2}")
        nc.vector.memset(a1m[:, N:256].bitcast(F32), 0.0)
        nc.vector.scalar_tensor_tensor(out=a1m[:, 0:N], in0=t1, scalar=0.2,
                                       in1=t1, op0=OP.mult, op1=OP.max)
        dtap("a1", a1m[:, 0:N], row0=m2 * 128, rows=128)
        a1.append(a1m)

    for half in range(4):
        w2st = []
        for kt in range(2):
            w = pstr.tile([128, D // 4], F32R, name=f"w2_st{kt}",
                          tag=f"str{kt}")
            dma(out=w, in_=d["w2s"].ap()[kt * 128:(kt + 1) * 128,
                                         half * 512:(half + 1) * 512])
            w2st.append(w)
        for mh in range(4):
            m = half * 4 + mh
            ps = ppt([128, 256], "ps_h2")
            for kt2 in range(2):
                nc.tensor.matmul(ps, w2st[kt2][:, mh * 128:(mh + 1) * 128],
                                 a1[kt2][:, 0:256], start=(kt2 == 0),
                                 stop=(kt2 == 1))
            hp = pt.tile([128, N], F32R, name="h2p", tag="zouts")
            nc.vector.tensor_scalar(out=hp, in0=ps[:, 0:N],
                                    scalar1=b2o8_t[:, m:m + 1],
                                    scalar2=None, op0=OP.add)
            dma(out=ar_h2_in[m * 128:(m + 1) * 128, :], in_=hp)
    nc.gpsimd.collective_compute("AllReduce", OP.add, replica_groups=GRP,
                                 ins=[ar_h2_in[:].opt()],
                                 outs=[ar_h2_out[:].opt()])

    # ---- znorm: load z (f32r padded), inst-norm, fin(m, z, t1) writes ----
    def znorm(src_dram, fin, zn, tagset):
        z = []
        for m in range(NT):
            t = pa.tile([128, 256], F32R, name=f"z{zn}_{m}",
                        tag=f"{tagset}{m}")
            dma(out=t[:, 0:N], in_=src_dram[m * 128:(m + 1) * 128, :])
            nc.vector.memset(t[:, N:256].bitcast(F32), 0.0)
            z.append(t)
        pstat = ppt_s([1, 512], f"st_{zn}")
        for m in range(NT):
            nc.tensor.matmul(pstat[0:1, 0:256], ones_r, z[m][:, 0:256],
                             start=(m == 0), stop=(m == NT - 1))
        mu = pt1.tile([1, 256], F32R, name="zmu", tag="zmu")
        nc.vector.tensor_scalar_mul(mu, pstat[0:1, 0:256], 1.0 / D)
        pmu = ppt_h([128, 256], f"mub_{zn}")
        nc.tensor.matmul(pmu, ones1_r, mu, start=True, stop=True)
        for m in range(NT):
            nc.vector.tensor_tensor(z[m][:, 0:N], z[m][:, 0:N], pmu[:, 0:N],
                                    OP.subtract)
            zq = pt.tile([128, 256], F32R, name="zq", tag="zq")
            nc.vector.tensor_tensor(zq, z[m], z[m], OP.mult)
            nc.tensor.matmul(pstat[0:1, 256:512], ones_r, zq[:, 0:256],
                             start=(m == 0), stop=(m == NT - 1))
        va = pt1.tile([1, 256], F32, name="zva", tag="zva")
        nc.vector.tensor_scalar(out=va, in0=pstat[0:1, 256:512],
                                scalar1=1.0 / D, scalar2=EPS,
                                op0=OP.mult, op1=OP.add)
        ta = pt1.tile([1, 256], F32, name="zta", tag="zmu2")
        nc.scalar.activation(ta, va, AF.Sqrt)
        rsf = pt1.tile([1, 256], F32, name="zrsf", tag="znm")
        nc.vector.reciprocal(out=rsf, in_=ta)
        rs = pt1.tile([1, 256], F32R, name="zrs", tag="zrs")
        nc.vector.tensor_copy(out=rs, in_=rsf)
        prr = ppt_h([128, 256], f"rb_{zn}")
        nc.tensor.matmul(prr, ones1_r, rs, start=True, stop=True)
        for m in range(NT):
            t1 = pt.tile([128, N], F32, name="zt1", tag="zt1")
            nc.vector.tensor_tensor(t1, z[m][:, 0:N], prr[:, 0:N], OP.mult)
            fin(m, z, t1)
        return z

    # h2 -> PVP (leaky), in-place in z set "za"
    def fin_pvp(m, z, t1):
        nc.vector.scalar_tensor_tensor(out=z[m][:, 0:N], in0=t1, scalar=0.2,
                                       in1=t1, op0=OP.mult, op1=OP.max)
    if "h2" in dbg:
        nc.sync.dma_start(out=dbg["h2"].ap()[:, :], in_=ar_h2_out[:, :])
    PVP = znorm(ar_h2_out, fin_pvp, "h2", "za")
    dtap16("PVP", PVP)

    # PVP_n (f32r) on the shared n-major chain
    def transpose_nmajor(src_tiles, name, tagbase, dt):
        out = [pa.tile([128, D], dt, name=f"{name}0", tag=f"{tagbase}0"),
               pa.tile([128, D], dt, name=f"{name}1", tag=f"{tagbase}1")]
        for m in range(NT):
            for jb, (j0, jw) in enumerate(N_MT):
                if dt == F32R:
                    ps = ppt_h([128, 128], "ps_tr", F32)
                    nc.tensor.transpose(
                        ps[0:jw, 0:128],
                        src_tiles[m][:, j0:j0 + jw].bitcast(F32), ident_f)
                else:
                    ps = ppt_h([128, 128], "ps_tr", dt)
                    nc.tensor.transpose(ps[0:jw, 0:128],
                                        src_tiles[m][:, j0:j0 + jw], ident_b)
                nc.vector.tensor_copy(
                    out=out[jb][0:jw, m * 128:(m + 1) * 128],
                    in_=ps[0:jw, 0:128])
        return out

    PVP_n = transpose_nmajor(PVP, "PVP_n", "nmj", F32R)

    # =================================================================
    # P3: cos edges (visual f32r, semantic bf16 gram -> f32r edges)
    # =================================================================
    def cos_edge(x_tiles, ksizes, en, rdt, ones_g, rhs_w):
        nkt = len(ksizes)
        pn = ppt_s([1, 256] if rhs_w == 256 else [1, N], f"nrm_{en}")
        for kt, ksz in enumerate(ksizes):
            xq = pt.tile([128, rhs_w], rdt, name="xq", tag="xq")
            nc.vector.tensor_tensor(xq[0:ksz, :], x_tiles[kt][0:ksz, 0:rhs_w],
                                    x_tiles[kt][0:ksz, 0:rhs_w], OP.mult)
            nc.tensor.matmul(pn, ones_g[0:ksz, :], xq[0:ksz, :],
                             start=(kt == 0), stop=(kt == nkt - 1))
        sd = pt.tile([1, N], F32, name="esd", tag="zmu")
        nc.scalar.activation(sd, pn[0:1, 0:N], AF.Sqrt)
        rn_f = pt1.tile([1, 256], F32, name="ern_f", tag="zva")
        nc.vector.memset(rn_f[0:1, N:256], 0.0)
        nc.vector.reciprocal(out=rn_f[0:1, 0:N], in_=sd)
        rn = pt1.tile([1, 256], F32R, name="ern", tag="zAB")
        nc.vector.tensor_copy(out=rn, in_=rn_f)
        prn = ppt_h([128, 256], f"rnb_{en}")
        nc.tensor.matmul(prn, ones1_r, rn, start=True, stop=True)
        rcol = pt.tile([128, 2], F32, name="rc", tag=f"rc_{en}")
        for mt, (i0, iw) in enumerate(N_MT):
            pst = ppt_h([128, 128], "ps_tr3")
            nc.tensor.transpose(pst[0:iw, 0:1], rn_f[0:1, i0:i0 + iw],
                                ident_f[0:1, 0:1])
            nc.vector.tensor_copy(out=rcol[0:iw, mt:mt + 1],
                                  in_=pst[0:iw, 0:1])
        edge = []
        for mt, (i0, iw) in enumerate(N_MT):
            ps = ppt([128, rhs_w], f"ps_{en}")
            for kt, ksz in enumerate(ksizes):
                nc.tensor.matmul(ps[0:iw, :], x_tiles[kt][0:ksz, i0:i0 + iw],
                                 x_tiles[kt][0:ksz, 0:rhs_w],
                                 start=(kt == 0), stop=(kt == nkt - 1))
            s1 = pt.tile([128, N], F32, name="es1", tag="zt1")
            nc.vector.tensor_scalar(out=s1[0:iw, :], in0=ps[0:iw, 0:N],
                                    scalar1=rcol[0:iw, mt:mt + 1],
                                    scalar2=None, op0=OP.mult)
            nc.vector.tensor_tensor(s1[0:iw, :], s1[0:iw, :], prn[0:iw, 0:N],
                                    OP.mult)
            rmx = pt.tile([128, 1], F32, name="ermx", tag="ermx")
            nc.vector.reduce_max(rmx[0:iw, :], s1[0:iw, :], axis=AX.X)
            bia = pt.tile([128, 1], F32, name="ebia", tag="ebia")
            nc.vector.tensor_scalar_mul(bia[0:iw, :], rmx[0:iw, :], -100.0)
            nc.scalar.activation(s1[0:iw, :], s1[0:iw, :], AF.Exp,
                                 bias=bia[0:iw, 0:1], scale=100.0)
            sm = pt.tile([128, 1], F32, name="esm", tag="esm")
            nc.vector.reduce_sum(sm[0:iw, :], s1[0:iw, :], axis=AX.X)
            rr = pt.tile([128, 1], F32, name="err", tag="err")
            nc.vector.reciprocal(out=rr[0:iw, :], in_=sm[0:iw, :])
            ed = pa.tile([128, N], F32R, name=f"{en}_{mt}",
                         tag=f"edg_{en}_{mt}")
            nc.vector.tensor_scalar(out=ed[0:iw, :], in0=s1[0:iw, :],
                                    scalar1=rr[0:iw, 0:1], scalar2=None,
                                    op0=OP.mult)
            edge.append(ed)
        edgeT = [pa.tile([128, 256], F32R, name=f"{en}T0", tag=f"{en}T0"),
                 pa.tile([128, 256], F32R, name=f"{en}T1", tag=f"{en}T1")]
        for jb in range(2):
            nc.vector.memset(edgeT[jb][:].bitcast(F32), 0.0)
        for mt, (i0, iw) in enumerate(N_MT):
            for jb, (j0, jw) in enumerate(N_MT):
                pst = ppt_h([128, 128], "ps_tr4", F32)
                nc.tensor.transpose(pst[0:jw, 0:iw],
                                    edge[mt][0:iw, j0:j0 + jw].bitcast(F32),
                                    ident_f[0:iw, 0:iw])
                nc.vector.tensor_copy(out=edgeT[jb][0:jw, i0:i0 + iw],
                                      in_=pst[0:jw, 0:iw])
        return edge, edgeT

    ones_b = pa.tile([128, 1], BF16, name="ones_b")
    nc.vector.memset(ones_b, 1.0)
    vedge, vedgeT = cos_edge(PVP, [128] * NT, "ve", F32R, ones_r, 256)
    sedge, sedgeT = cos_edge(at_b, S_KT, "se", BF16, ones_b, N)
    if "vedge" in dbg:
        for mt, (i0, iw) in enumerate(N_MT):
            dtap("vedge", vedge[mt][0:iw, :], row0=i0, rows=iw)
            dtap("sedge", sedge[mt][0:iw, :], row0=i0, rows=iw)

    # =================================================================
    # P4: UpdateVisualNode -> AllGather(z_v) -> VP2 (in-place set "zb")
    # =================================================================
    vp_f, y_r = [], []
    for m in range(NT):
        pv = pp.tile([128, 256], F32, name="ps_vp", tag="mm")
        pe = pp.tile([128, 256], F32, name="ps_ev", tag="mm")
        for jb, (j0, jw) in enumerate(N_MT):
            nc.tensor.matmul(pv, PVP_n[jb][0:jw, m * 128:(m + 1) * 128],
                             vedgeT[jb][0:jw, :], start=(jb == 0),
                             stop=(jb == 1))
            nc.tensor.matmul(pe, PVP_n[jb][0:jw, m * 128:(m + 1) * 128],
                             sedgeT[jb][0:jw, :], start=(jb == 0),
                             stop=(jb == 1))
        vf = pa.tile([128, N], F32, name=f"vp_f{m}")
        nc.scalar.copy(out=vf, in_=pv[:, 0:N])
        vp_f.append(vf)
        yb = sh1k(f"y_{m}", m)
        nc.vector.memset(yb[:, N:256].bitcast(F32), 0.0)
        nc.vector.tensor_tensor(yb[:, 0:N], vf, pe[:, 0:N], OP.add)
        y_r.append(yb)
    dtap16("ybf", y_r)

    for m2 in range(2):
        ps = ppt([128, 256], "ps_zv")
        for kt in range(NT):
            nc.tensor.matmul(ps, vnWs_t[kt][:, m2 * 128:(m2 + 1) * 128],
                             y_r[kt][:, 0:256], start=(kt == 0),
                             stop=(kt == NT - 1))
        zc = pt.tile([128, N], F32R, name="zvc", tag="zouts")
        nc.vector.tensor_scalar(out=zc, in0=ps[:, 0:N],
                                scalar1=vnbs_t[:, m2:m2 + 1],
                                scalar2=None, op0=OP.add)
        dma(out=ag_zv_in[m2 * 128:(m2 + 1) * 128, :], in_=zc)
    nc.gpsimd.collective_compute("AllGather", OP.bypass, replica_groups=GRP,
                                 ins=[ag_zv_in[:].opt()],
                                 outs=[ag_zv_out[:].opt()])
    if "zv" in dbg:
        nc.sync.dma_start(out=dbg["zv"].ap()[:, :], in_=ag_zv_out[:, :])

    VP2_bf = [pa.tile([128, N], BF16, name=f"VP2b{m}", tag=f"VP2b{m}")
              for m in range(NT)]

    def fin_vp2(m, z, t1):
        nc.vector.scalar_tensor_tensor(out=z[m][:, 0:N], in0=t1, scalar=0.0,
                                       in1=vp_f[m], op0=OP.max, op1=OP.add)
        nc.vector.tensor_copy(out=VP2_bf[m], in_=z[m][:, 0:N])
    VP2 = znorm(ag_zv_out, fin_vp2, "zv", "zb")
    dtap16("VP2", VP2_bf)

    # =================================================================
    # P6: UpdateVisualEdge (i-sharded, bf16 path)
    # =================================================================
    VP2_n = transpose_nmajor(VP2_bf, "VP2_n", "nmj", BF16)
    xmyn = pw.tile([ISH, D], BF16, name="xmyn")
    for ch in range(4):
        ps = ppt([ISH, 512], "ps_xmy")
        for jb, (j0, jw) in enumerate(N_MT):
            nc.tensor.matmul(ps, selv_t[jb][0:jw, :],
                             VP2_n[jb][0:jw, ch * 512:(ch + 1) * 512],
                             start=(jb == 0), stop=(jb == 1))
        nc.vector.tensor_copy(out=xmyn[:, ch * 512:(ch + 1) * 512], in_=ps)
    dtap("xmyn", xmyn[:, :])
    xmyd, negx2my = [], []
    for kt in range(NT):
        pst = ppt_h([128, 128], "ps_tr5", BF16)
        nc.tensor.transpose(pst[0:128, 0:ISH],
                            xmyn[:, kt * 128:(kt + 1) * 128],
                            ident_b[0:ISH, 0:ISH])
        xd = pa.tile([128, ISH], BF16, name=f"xmyd{kt}")
        nc.vector.tensor_copy(out=xd, in_=pst[0:128, 0:ISH])
        xmyd.append(xd)
        ng = pa.tile([128, ISH], BF16, name=f"negx2my{kt}")
        nc.vector.tensor_scalar_mul(ng, pst[0:128, 0:ISH], -2.0)
        negx2my.append(ng)
    pA = ppt([128, N], "ps_A")
    pAm = ppt([128, ISH], "ps_Am")
    for m in range(NT):
        xq = pt.tile([128, N], BF16, name="vsq", tag="xq")
        nc.vector.tensor_tensor(xq, VP2_bf[m], VP2_bf[m], OP.mult)
        nc.tensor.matmul(pA, veW1_sl(m), xq[:, :], start=(m == 0),
                         stop=(m == NT - 1))
        xqm = pt.tile([128, ISH], BF16, name="vsqm", tag="vsqm")
        nc.vector.tensor_tensor(xqm, xmyd[m], xmyd[m], OP.mult)
        nc.tensor.matmul(pAm, veW1_sl(m), xqm[:, :], start=(m == 0),
                         stop=(m == NT - 1))
    A_T = pa.tile([128, N], F32, name="A_T")
    nc.vector.tensor_scalar(out=A_T[0:H, :], in0=pA[0:H, :],
                            scalar1=veb1_t[0:H, 0:1], scalar2=None, op0=OP.add)
    A_my = pa.tile([128, ISH], F32, name="A_my")
    nc.vector.tensor_scalar(out=A_my[0:H, :], in0=pAm[0:H, :],
                            scalar1=veb1_t[0:H, 0:1], scalar2=None, op0=OP.add)
    dtap("AT", A_T[0:128, 0:N])
    dtap("Amy", A_my[0:128, :])
    vedge_my = pa.tile([ISH, N], F32, name="vedge_my", tag="edg_se_0")
    psvm = ppt([ISH, N], "ps_vm")
    for mt, (i0, iw) in enumerate(N_MT):
        vb = pt.tile([128, N], BF16, name="vedgb", tag="xq")
        nc.vector.tensor_copy(out=vb[0:iw, :], in_=vedge[mt][0:iw, :])
        nc.tensor.matmul(psvm, selv_t[mt][0:iw, :], vb[0:iw, :],
                         start=(mt == 0), stop=(mt == 1))
    nc.vector.tensor_copy(out=vedge_my, in_=psvm)

    cur_sb = pa.tile([ISH, N], F32, name="cur_sb", tag="atf2")
    groups = [(g0, min(G_I, ISH - g0)) for g0 in range(0, ISH, G_I)]
    for i0, gsz in groups:
        tgt = []
        for kt in range(NT):
            tt = psh.tile([128, G_I * N], BF16, name=f"tg{kt}", tag=f"sh{kt}")
            in0 = _rep(VP2_bf[kt][:, :], [[0, gsz], [1, N]])
            in1 = _rep(negx2my[kt][:, i0:i0 + gsz], [[1, gsz], [0, N]])
            out3 = _rep(tt[:, :], [[N, gsz], [1, N]])
            eng = nc.vector if kt % 2 == 0 else nc.gpsimd
            eng.tensor_tensor(out3, in0, in1, OP.mult)
            tgt.append(tt)
        for il in range(gsz):
            ii = i0 + il
            ph = ppt([128, N], "ps_eh")
            for kt in range(NT):
                nc.tensor.matmul(ph, veW1_sl(kt),
                                 tgt[kt][:, il * N:(il + 1) * N],
                                 start=(kt == 0), stop=(kt == NT - 1))
            hsb = pt.tile([128, 256], F32R, name="hsb", tag="zq")
            nc.vector.scalar_tensor_tensor(
                out=hsb[0:H, 0:N], in0=ph[0:H, :],
                scalar=A_my[0:H, ii:ii + 1], in1=A_T[0:H, :],
                op0=OP.add, op1=OP.add)
            nc.vector.memset(hsb[0:H, N:256].bitcast(F32), 0.0)
            hsq = pt.tile([128, 256], F32R, name="hsq", tag="fth")
            nc.vector.tensor_tensor(hsq[0:H, :], hsb[0:H, :], hsb[0:H, :],
                                    OP.mult)
            pst = ppt_s([1, 512], "st_e")
            nc.tensor.matmul(pst[0:1, 0:256], ones_r[0:H, :], hsb[0:H, 0:256],
                             start=True, stop=True)
            nc.tensor.matmul(pst[0:1, 256:512], ones_r[0:H, :],
                             hsq[0:H, 0:256], start=True, stop=True)
            stt = pt1.tile([1, 512], F32, name="estt", tag="zstt")
            nc.vector.tensor_copy(out=stt, in_=pst)
            mu = pt1.tile([1, 256], F32, name="emu", tag="zmu")
            nc.vector.tensor_scalar_mul(mu, stt[0:1, 0:256], 1.0 / H)
            va = pt1.tile([1, 256], F32, name="eva", tag="zva")
            nc.vector.tensor_scalar_mul(va, stt[0:1, 256:512], 1.0 / H)
            ta = pt1.tile([1, 256], F32, name="eta", tag="zmu2")
            nc.vector.tensor_tensor(ta, mu, mu, OP.mult)
            nc.vector.tensor_tensor(va, va, ta, OP.subtract)
            nc.vector.tensor_scalar_add(va, va, EPS)
            nc.scalar.activation(ta, va, AF.Sqrt)
            rs = pt1.tile([1, 256], F32, name="ers2", tag="zrs")
            nc.vector.reciprocal(out=rs, in_=ta)
            AB = pt1.tile([1, 512], F32R, name="eAB", tag="zAB")
            nc.vector.tensor_copy(out=AB[0:1, 0:256], in_=rs)
            nc.vector.tensor_tensor(ta, mu, rs, OP.mult)
            nc.vector.tensor_scalar_mul(AB[0:1, 256:512], ta, -1.0)
            pab = ppt_h([128, 512], "ab_e")
            nc.tensor.matmul(pab, ones1_r, AB, start=True, stop=True)
            t1 = pt.tile([128, N], F32, name="et1", tag="zt1")
            nc.vector.tensor_tensor(t1[0:H, :], hsb[0:H, 0:N], pab[0:H, 0:N],
                                    OP.mult)
            nc.vector.tensor_tensor(t1[0:H, :], t1[0:H, :],
                                    pab[0:H, 256:256 + N], OP.add)
            h2b = pt.tile([128, N], BF16, name="eh2b", tag="xq")
            nc.vector.tensor_scalar_max(h2b[0:H, :], t1[0:H, :], 0.0)
            pcur = ppt_s([1, N], "ps_cur")
            nc.tensor.matmul(pcur, veW2_t[0:H, 0:1], h2b[0:H, :],
                             start=True, stop=True)
            cst = pt.tile([1, N], F32, name="cst", tag="cst")
            nc.vector.tensor_copy(out=cst, in_=pcur)
            dma(out=cur_sb[ii:ii + 1, :], in_=cst)
    dtap("cur", cur_sb[:, :])

    # tanh(cur + b2) * (vedge_my + 1e-8) -> softmax(/10) -> my edge rows
    curt = pa.tile([ISH, N], F32, name="curt", tag="atf0")
    nc.scalar.activation(curt, cur_sb, AF.Tanh, bias=veb2_t[0:ISH, 0:1],
                         scale=1.0)
    ne = pa.tile([ISH, N], F32, name="ne", tag="atf1")
    nc.vector.scalar_tensor_tensor(out=ne, in0=vedge_my, scalar=1e-8,
                                   in1=curt, op0=OP.add, op1=OP.mult)
    rmx = pt.tile([ISH, 1], F32, name="vermx", tag="vermx")
    nc.vector.reduce_max(rmx, ne, axis=AX.X)
    bia = pt.tile([ISH, 1], F32, name="vebia", tag="vebia")
    nc.vector.tensor_scalar_mul(bia, rmx, -0.1)
    ex = pt1.tile([ISH, N], F32, name="veex", tag="veex")
    nc.scalar.activation(ex, ne, AF.Exp, bias=bia[0:ISH, 0:1], scale=0.1)
    sm = pt.tile([ISH, 1], F32, name="vesm", tag="vesm")
    nc.vector.reduce_sum(sm, ex, axis=AX.X)
    rr = pt.tile([ISH, 1], F32, name="verr", tag="verr")
    nc.vector.reciprocal(out=rr, in_=sm)
    vemine = pt1.tile([ISH, N], F32R, name="vemine", tag="vemine")
    nc.vector.tensor_scalar(out=vemine, in0=ex, scalar1=rr[0:ISH, 0:1],
                            scalar2=None, op0=OP.mult)
    dma(out=ag_ve_in[:, :], in_=vemine)
    nc.gpsimd.collective_compute("AllGather", OP.bypass, replica_groups=GRP,
                                 ins=[ag_ve_in[:].opt()],
                                 outs=[ag_ve_out[:].opt()])
    if "ve2" in dbg:
        nc.sync.dma_start(out=dbg["ve2"].ap()[:, :], in_=ag_ve_out[:, :])
    ve2 = [pt1.tile([128, N], F32R, name="ve2_0", tag="ve2_0"),
           pt1.tile([128, N], F32R, name="ve2_1", tag="ve2_1")]
    for mt, (i0, iw) in enumerate(N_MT):
        dma(out=ve2[mt][0:iw, :], in_=ag_ve_out[i0:i0 + iw, :])
    ve2T = [pa.tile([128, 256], F32R, name="ve2T0", tag="veT0"),
            pa.tile([128, 256], F32R, name="ve2T1", tag="veT1")]
    for mt, (i0, iw) in enumerate(N_MT):
        for jb, (j0, jw) in enumerate(N_MT):
            pst = ppt_h([128, 128], "ps_tr6", F32)
            nc.tensor.transpose(pst[0:jw, 0:iw],
                                ve2[mt][0:iw, j0:j0 + jw].bitcast(F32),
                                ident_f[0:iw, 0:iw])
            nc.vector.tensor_copy(out=ve2T[jb][0:jw, i0:i0 + iw],
                                  in_=pst[0:jw, 0:iw])

    # =================================================================
    # P5: UpdateSemanticNode -> AllGather(z_s) -> SP2 (in-place set "za")
    # =================================================================
    SP_n = transpose_nmajor(SP, "SP_n", "nmj", F32R)
    sp_f, y2_r = [], []
    for m in range(NT):
        psp = pp.tile([128, 256], F32, name="ps_sp", tag="mm")
        pes = pp.tile([128, 256], F32, name="ps_es", tag="mm")
        for jb, (j0, jw) in enumerate(N_MT):
            nc.tensor.matmul(psp, SP_n[jb][0:jw, m * 128:(m + 1) * 128],
                             sedgeT[jb][0:jw, :], start=(jb == 0),
                             stop=(jb == 1))
            nc.tensor.matmul(pes, SP_n[jb][0:jw, m * 128:(m + 1) * 128],
                             ve2T[jb][0:jw, :], start=(jb == 0),
                             stop=(jb == 1))
        sf = pa.tile([128, N], F32, name=f"sp_f{m}", tag=f"vp_f{m}")
        nc.scalar.copy(out=sf, in_=psp[:, 0:N])
        sp_f.append(sf)
        yb = sh1k(f"y2_{m}", m)
        nc.vector.memset(yb[:, N:256].bitcast(F32), 0.0)
        nc.vector.tensor_tensor(yb[:, 0:N], sf, pes[:, 0:N], OP.add)
        y2_r.append(yb)

    for m2 in range(2):
        ps = ppt([128, 256], "ps_zs")
        for kt in range(NT):
            nc.tensor.matmul(ps, snWs_t[kt][:, m2 * 128:(m2 + 1) * 128],
                             y2_r[kt][:, 0:256], start=(kt == 0),
                             stop=(kt == NT - 1))
        zc = pt.tile([128, N], F32R, name="zsc", tag="zouts")
        nc.vector.tensor_scalar(out=zc, in0=ps[:, 0:N],
                                scalar1=snbs_t[:, m2:m2 + 1],
                                scalar2=None, op0=OP.add)
        dma(out=ag_zs_in[m2 * 128:(m2 + 1) * 128, :], in_=zc)
    nc.gpsimd.collective_compute("AllGather", OP.bypass, replica_groups=GRP,
                                 ins=[ag_zs_in[:].opt()],
                                 outs=[ag_zs_out[:].opt()])

    def fin_sp2(m, z, t1):
        nc.vector.scalar_tensor_tensor(out=z[m][:, 0:N], in0=t1, scalar=0.0,
                                       in1=sp_f[m], op0=OP.max, op1=OP.add)
    SP2 = znorm(ag_zs_out, fin_sp2, "zs", "za")
    if "SP2" in dbg:
        for m in range(NT):
            nc.sync.dma_start(out=dbg["SP2"].ap()[m * 128:(m + 1) * 128, :],
                              in_=SP2[m][:, 0:N])

    # =================================================================
    # P7: FusionLayer (f32r) -> alpha -> prob
    # =================================================================
    pvu = [ppt_s([1, 256], "ps_vu0"), ppt_s([1, 256], "ps_vu1")]
    for k, srct in enumerate((VP2, SP2)):
        for m2 in range(2):
            ps = ppt([128, 256], "ps_fus")
            for kt in range(NT):
                nc.tensor.matmul(ps, fusWs_t[kt][:, m2 * 128:(m2 + 1) * 128],
                                 srct[kt][:, 0:256], start=(kt == 0),
                                 stop=(kt == NT - 1))
            th = pt.tile([128, 256], F32R, name="fth", tag="fth")
            nc.scalar.activation(th, ps, AF.Tanh)
            nc.tensor.matmul(pvu[k], fusUs_t[:, m2:m2 + 1], th[:, :],
                             start=(m2 == 0), stop=(m2 == 1))
    vu_sb = pt1.tile([1, 512], F32R, name="vu_sb", tag="vu_sb")
    nc.vector.memset(vu_sb[:].bitcast(F32), 0.0)
    nc.vector.tensor_copy(out=vu_sb[0:1, 0:N], in_=pvu[0][0:1, 0:N])
    nc.vector.tensor_copy(out=vu_sb[0:1, 256:256 + N], in_=pvu[1][0:1, 0:N])
    dma(out=ag_vu_in[:, :], in_=vu_sb)
    nc.gpsimd.collective_compute("AllGather", OP.bypass, replica_groups=GRP,
                                 ins=[ag_vu_in[:].opt()],
                                 outs=[ag_vu_out[:].opt()])
    vus = pt1.tile([NCORES, 512], F32R, name="vus", tag="vu_sb")
    dma(out=vus, in_=ag_vu_out[:, :])
    pvk = ppt_s([1, 512], "ps_vuk")
    for k in range(2):
        nc.tensor.matmul(pvk[0:1, 256 * k:256 * k + 256],
                         ones8_r, vus[:, 256 * k:256 * k + 256],
                         start=True, stop=True)
    vuf = pt1.tile([1, 512], F32, name="vuf", tag="zstt")
    nc.vector.tensor_copy(out=vuf, in_=pvk)
    dtap("vuf", vuf[:, :])
    mx = pt.tile([1, N], F32, name="amx", tag="amx")
    nc.vector.tensor_tensor(mx, vuf[0:1, 0:N], vuf[0:1, 256:256 + N], OP.max)
    dv = pt1.tile([1, 512], F32R, name="adv", tag="adv")
    nc.vector.memset(dv[:].bitcast(F32), 0.0)
    for k in range(2):
        nc.vector.tensor_tensor(dv[0:1, 256 * k:256 * k + N],
                                vuf[0:1, 256 * k:256 * k + N], mx, OP.subtract)
    nc.scalar.activation(dv, dv, AF.Exp, scale=100.0)
    ssum = pt.tile([1, N], F32, name="assum", tag="assum")
    nc.vector.tensor_tensor(ssum, dv[0:1, 0:N], dv[0:1, 256:256 + N], OP.add)
    rsu = pt.tile([1, N], F32, name="arsu", tag="arsu")
    nc.vector.reciprocal(out=rsu, in_=ssum)
    for k in range(2):
        nc.vector.tensor_tensor(dv[0:1, 256 * k:256 * k + N],
                                dv[0:1, 256 * k:256 * k + N], rsu, OP.mult)
    alro = dv
    if "alpha" in dbg:
        al_f = pt1.tile([1, 512], F32, name="al_f", tag="zstt")
        nc.vector.tensor_copy(out=al_f, in_=alro)
        dtap("alpha", al_f[:, :])
    pal = ppt_h([128, 512], "ab_al")
    nc.tensor.matmul(pal, ones1_r, alro, start=True, stop=True)
    proto_bf = []
    for m in range(NT):
        t1 = pt.tile([128, N], F32, name="pr1", tag="zouts")
        nc.vector.tensor_tensor(t1, VP2[m][:, 0:N], pal[:, 0:N], OP.mult)
        t2 = pt.tile([128, N], F32, name="pr2", tag="zt1")
        nc.vector.tensor_tensor(t2, SP2[m][:, 0:N], pal[:, 256:256 + N],
                                OP.mult)
        pb = pa.tile([128, N], BF16, name=f"proto{m}", tag=f"VP2b{m}")
        nc.vector.tensor_tensor(pb, t1, t2, OP.add)
        proto_bf.append(pb)
    for bt in range(2):
        ps = ppt([128, N], "ps_prob")
        for kt in range(NT):
            nc.tensor.matmul(ps, imgT_sl(kt, bt), proto_bf[kt][:, :],
                             start=(kt == 0), stop=(kt == NT - 1))
        t1 = pt.tile([128, N], F32, name="probf", tag="zouts")
        nc.vector.tensor_copy(out=t1, in_=ps)
        dma(out=prob_out.ap()[bt * 128:(bt + 1) * 128, :], in_=t1)


# =====================================================================
# Host side
# =====================================================================
def _prep_inputs(inputs):
    bf = ml_dtypes.bfloat16
    f32 = np.float32
    att = np.asarray(inputs["attribute"], f32)
    cen = np.asarray(inputs["centers"], f32)
    expW = np.asarray(inputs["expert_W"], f32)
    expB = np.asarray(inputs["expert_b"], f32)
    w1 = np.asarray(inputs["s2v_W1"], f32)
    w2 = np.asarray(inputs["s2v_W2"], f32)
    in_maps = []
    for c in range(NCORES):
        cs = slice(c * DSH, (c + 1) * DSH)
        isl = slice(c * ISH, (c + 1) * ISH)
        bs = slice(c * BSH, (c + 1) * BSH)
        selv = np.zeros((N, ISH), f32)
        selv[np.arange(c * ISH, (c + 1) * ISH), np.arange(ISH)] = 1.0
        m = {
            "attrT": np.ascontiguousarray(att.T),
            "attrTb": np.ascontiguousarray(att.T).astype(bf),
            "centT": np.ascontiguousarray(cen.T),
            "expW": expW,
            "expBT": np.ascontiguousarray(expB.T),
            "w1s": np.ascontiguousarray(w1[:, cs]),
            "bnG": np.ascontiguousarray(np.asarray(inputs["bn_g"], f32)[cs].reshape(2, 128).T),
            "bnB": np.ascontiguousarray(np.asarray(inputs["bn_b"], f32)[cs].reshape(2, 128).T),
            "w2s": np.ascontiguousarray(w2[cs, :]),
            "b2o8": np.ascontiguousarray((np.asarray(inputs["s2v_b2"], f32) / NCORES).reshape(NT, 128).T),
            "vnWs": np.ascontiguousarray(np.asarray(inputs["vn_W"], f32)[:, cs]),
            "vnbs": np.ascontiguousarray(np.asarray(inputs["vn_b"], f32)[cs].reshape(2, 128).T),
            "snWs": np.ascontiguousarray(np.asarray(inputs["sn_W"], f32)[:, cs]),
            "snbs": np.ascontiguousarray(np.asarray(inputs["sn_b"], f32)[cs].reshape(2, 128).T),
            "veW1": np.asarray(inputs["ve_W1"], f32).astype(bf),
            "veb1": np.asarray(inputs["ve_b1"], f32)[:, None],
            "veW2": np.asarray(inputs["ve_W2"], f32).astype(bf),
            "veb2": np.full((ISH, 1), float(np.asarray(inputs["ve_b2"])[0]),
                            f32),
            "fusWs": np.ascontiguousarray(np.asarray(inputs["fus_W"], f32)[:, cs]),
            "fusUs": np.ascontiguousarray(np.asarray(inputs["fus_u"], f32)[cs, 0].reshape(2, 128).T),
            "imgT": np.ascontiguousarray(
                np.asarray(inputs["img_feat"], f32)[bs, :].T).astype(bf),
            "selv": selv.astype(bf),
        }
        in_maps.append(m)
    return in_maps


def kernel(**inputs):
    global _BUILT
    if _BUILT is None:
        _BUILT = build()
    nc = _BUILT
    in_maps = _prep_inputs(inputs)
    res = run_bass_kernel_spmd(nc, in_maps, core_ids=list(range(NCORES)))
    out = np.concatenate([res.results[c]["prob"] for c in range(NCORES)],
                         axis=0)
    return out.astype(np.float32)


def kernel_debug(**inputs):
    nc = build(debug=True)
    in_maps = _prep_inputs(inputs)
    res = run_bass_kernel_spmd(nc, in_maps, core_ids=list(range(NCORES)))
    out = np.concatenate([res.results[c]["prob"] for c in range(NCORES)],
                         axis=0)
    return out.astype(np.float32), res.results


if __name__ == "__main__":
    import reference
    inp = {k: np.asarray(v) for k, v in reference.setup_inputs().items()}
    got = kernel(**inp)
    exp = np.asarray(reference.reference(**reference.setup_inputs()))
    err = np.abs(got - exp).max() / (np.abs(exp).max() + 1e-9)
    print("Relative error:", err)



# revision 10
# speedup vs baseline: 1.0995x; 1.0995x over previous
# Trainium2 Bass kernel for nn_CPPN (gnn_message_passing), 8-core SPMD.
#
# Sharding:
#   - Node-MLP weights (s2v_W1/W2, vn_W, sn_W, fus_W) sharded over the 2048
#     col/row dim (256 per core); stitched with one AllReduce (h2) and three
#     AllGathers (z_v, z_s, vu partials).
#   - Visual edge MLP (200x200 pairwise rows) sharded over i: 25 rows/core,
#     AllGather of the resulting edge rows.  Per-core column selection is via
#     a host-supplied one-hot matrix (SPMD program is identical on all cores;
#     only input data differs).
#   - img_feat batch sharded 256 rows/core for the final prob matmul; host
#     concatenates per-core outputs.
# Layout: activations are d-major [D on partitions (16x128), 200 on free].
# Heavy matmuls bf16; stats/broadcast matmuls f32r.  The reference's second
# edge_update (semantic) is dead code w.r.t. prob and is skipped.

import sys

sys.path.insert(0, "/opt/trn_rl_repo")

import numpy as np
import ml_dtypes

import concourse.bass as bass
import concourse.bacc as bacc
import concourse.tile as tile
from concourse import mybir
from concourse.bass_utils import run_bass_kernel_spmd
from concourse.masks import make_identity

F32 = mybir.dt.float32
F32R = mybir.dt.float32r
BF16 = mybir.dt.bfloat16
F16 = mybir.dt.float16
AF = mybir.ActivationFunctionType
OP = mybir.AluOpType
AX = mybir.AxisListType

NCORES = 8
N = 200
S = 312
D = 2048
H = 128
KEXP = 3
B = 2048
DSH = D // NCORES      # 256
ISH = N // NCORES      # 25
BSH = B // NCORES      # 256
EPS = 1e-5
NT = D // 128          # 16
S_KT = [128, 128, 56]
N_MT = ((0, 128), (128, 72))
G_I = 2                # i-group size for edge t-gen batching

_BUILT = None


def _rep(ap_src, dims):
    """Rebuild AP with explicit free dims [[step,count],...] after partition."""
    return bass.AP(tensor=ap_src.tensor, offset=ap_src.offset,
                   ap=[ap_src.ap[0]] + dims)


def build(debug=False):
    nc = bacc.Bacc("TRN2", target_bir_lowering=False, debug=False,
                   num_devices=NCORES)
    d = {}

    def din(name, shape, dt):
        d[name] = nc.dram_tensor(name, shape, dt, kind="ExternalInput")

    din("attrT", [S, N], F32)
    din("attrTb", [S, N], BF16)
    din("centT", [S, KEXP], F32)
    din("expW", [KEXP, S, D], F32R)
    din("expBT", [D, KEXP], F32)
    din("w1s", [D, DSH], F32R)
    din("bnG", [128, 2], F32)
    din("bnB", [128, 2], F32)
    din("w2s", [DSH, D], F32R)
    din("b2o8", [128, NT], F32)
    din("vnWs", [D, DSH], F32R)
    din("vnbs", [128, 2], F32)
    din("snWs", [D, DSH], F32R)
    din("snbs", [128, 2], F32)
    din("veW1", [D, H], BF16)
    din("veb1", [H, 1], F32)
    din("veW2", [H, 1], BF16)
    din("veb2", [ISH, 1], F32)
    din("fusWs", [D, DSH], F32R)
    din("fusUs", [128, 2], F32R)
    din("imgT", [D, BSH], BF16)
    din("selv", [N, ISH], BF16)
    prob_out = nc.dram_tensor("prob", [BSH, N], F32, kind="ExternalOutput")
    dbg = {}
    if debug:
        def dout(name, shape, dt):
            dbg[name] = nc.dram_tensor("dbg_" + name, shape, dt,
                                       kind="ExternalOutput")
        dout("SP", [D, N], F32R)
        dout("a1", [2 * 128, N], F32R)
        dout("h2", [D, N], F16)
        dout("PVP", [D, N], F32R)
        dout("vedge", [N, N], F32R)
        dout("sedge", [N, N], F32R)
        dout("ybf", [D, N], F32R)
        dout("zv", [D, N], F16)
        dout("VP2", [D, N], BF16)
        dout("AT", [128, N], F32)
        dout("Amy", [128, ISH], F32)
        dout("xmyn", [ISH, D], BF16)
        dout("cur", [ISH, N], F32)
        dout("ve2", [N, N], F32R)
        dout("SP2", [D, N], F32R)
        dout("vuf", [1, 512], F32)
        dout("alpha", [1, 512], F32)

    with tile.TileContext(nc) as tc:
        import contextlib
        with contextlib.ExitStack() as ctx, \
                nc.allow_low_precision(reason="bf16 PE transposes (no accum)"):
            _emit(ctx, nc, tc, d, prob_out, dbg)
    nc.compile()
    return nc


def _emit(ctx, nc, tc, d, prob_out, dbg=None):
    dbg = dbg or {}

    def dtap16(key, tiles):     # 16 d-major tiles -> [D, N] dram
        if key in dbg:
            for m in range(NT):
                nc.sync.dma_start(out=dbg[key].ap()[m * 128:(m + 1) * 128, :],
                                  in_=tiles[m][:, 0:N])

    def dtap(key, ap_in, row0=0, rows=None):
        if key in dbg:
            o = dbg[key].ap()
            nc.sync.dma_start(out=o[row0:row0 + (rows or o.shape[0]), :],
                              in_=ap_in)

    pw = ctx.enter_context(tc.tile_pool(name="wts", bufs=1))
    pa = ctx.enter_context(tc.tile_pool(name="acts", bufs=1))
    pt = ctx.enter_context(tc.tile_pool(name="tmp", bufs=2))
    pt1 = ctx.enter_context(tc.tile_pool(name="rows", bufs=1))
    psh = ctx.enter_context(tc.tile_pool(name="sh1k", bufs=1))
    pstr = ctx.enter_context(tc.tile_pool(name="stream", bufs=1))
    pdram = ctx.enter_context(tc.tile_pool(name="dram", bufs=1, space="DRAM"))
    pp = ctx.enter_context(tc.tile_pool(name="ps_mm", bufs=4, space="PSUM"))
    pph = ctx.enter_context(tc.tile_pool(name="ps_hold", bufs=2, space="PSUM"))
    pps = ctx.enter_context(tc.tile_pool(name="ps_st", bufs=2, space="PSUM"))

    dma = nc.sync.dma_start
    GRP = [list(range(NCORES))]

    def ppt(shape, name, dt=F32):
        return pp.tile(shape, dt, name=name, tag="mm")

    def ppt_h(shape, name, dt=F32):
        return pph.tile(shape, dt, name=name, tag="hold")

    def ppt_s(shape, name, dt=F32):
        return pps.tile(shape, dt, name=name, tag="stat")

    def sh1k(name, tagid, dt=F32R, shape=None):
        return psh.tile(shape or [128, 256], dt, name=name, tag=f"sh{tagid}")

    # ---------- constants ----------
    ident_b = pa.tile([128, 128], BF16, name="ident_b")
    make_identity(nc, ident_b)
    ident_f = pt1.tile([128, 128], F32, name="ident_f", tag="ident_f")
    make_identity(nc, ident_f)
    ident_r = pa.tile([128, 128], F32R, name="ident_r")
    nc.vector.tensor_copy(out=ident_r, in_=ident_f)
    ones_r = pa.tile([128, 1], F32R, name="ones_r")
    nc.vector.memset(ones_r[:].bitcast(F32), 1.0)
    ones_h = pa.tile([128, 1], F16, name="ones_h")
    nc.vector.memset(ones_h, 1.0)
    ones1_r = pa.tile([1, 128], F32R, name="ones1_r")
    nc.vector.memset(ones1_r[:].bitcast(F32), 1.0)
    ones8_r = pa.tile([8, 1], F32R, name="ones8_r")
    nc.vector.memset(ones8_r[:].bitcast(F32), 1.0)
    epsc = pa.tile([128, 1], F32, name="epsc")
    nc.vector.memset(epsc, EPS)

    # ---------- inputs -> SBUF ----------
    at_f, at_b = [], []
    off = 0
    for kt, ksz in enumerate(S_KT):
        tf = pa.tile([128, N], F32, name=f"at_f{kt}", tag=f"atf{kt}")
        tb = pa.tile([128, N], BF16, name=f"at_b{kt}")
        dma(out=tf[0:ksz, :], in_=d["attrT"].ap()[off:off + ksz, :])
        dma(out=tb[0:ksz, :], in_=d["attrTb"].ap()[off:off + ksz, :])
        at_f.append(tf)
        at_b.append(tb)
        off += ksz
    centT_t = []
    off = 0
    for kt, ksz in enumerate(S_KT):
        t = pa.tile([128, KEXP], F32, name=f"centT{kt}")
        dma(out=t[0:ksz, :], in_=d["centT"].ap()[off:off + ksz, :])
        centT_t.append(t)
        off += ksz
    expbt = []
    for m in range(NT):
        t = pw.tile([128, KEXP], F32, name=f"expbt{m}")
        dma(out=t, in_=d["expBT"].ap()[m * 128:(m + 1) * 128, :])
        expbt.append(t)

    def load16(name, key, ncols, dt=F32R, tagbase=None):
        out = []
        for kt in range(NT):
            tg = f"{tagbase}{kt}" if tagbase else ""
            t = pw.tile([128, ncols], dt, name=f"{name}{kt}", tag=tg)
            dma(out=t, in_=d[key].ap()[kt * 128:(kt + 1) * 128, :])
            out.append(t)
        return out

    w1s_t = load16("w1s", "w1s", DSH)
    vnWs_t = load16("vnWs", "vnWs", DSH)
    snWs_t = load16("snWs", "snWs", DSH, tagbase="vnWs")
    fusWs_t = load16("fusWs", "fusWs", DSH, tagbase="w1s")
    # imgT packed: 2 tiles [128, 8*256] bf16; lhsT slice kt -> [:, (kt%8)*256+...]
    imgT_p = []
    for hh in range(2):
        t = pw.tile([128, 8 * BSH], BF16, name=f"imgTp{hh}")
        src_ap = d["imgT"].ap()[hh * 1024:(hh + 1) * 1024, :].rearrange(
            "(kt p) b -> p kt b", p=128)
        nc.sync.dma_start(out=t[:].rearrange("p (kt b) -> p kt b", kt=8),
                          in_=src_ap)
        imgT_p.append(t)

    def imgT_sl(kt, bt):
        return imgT_p[kt // 8][:, (kt % 8) * BSH + bt * 128:
                               (kt % 8) * BSH + (bt + 1) * 128]

    # veW1 packed single [128, 2048] bf16
    veW1_a = pw.tile([128, D], BF16, name="veW1_a")
    nc.sync.dma_start(out=veW1_a[:].rearrange("p (kt h) -> p kt h", kt=NT),
                      in_=d["veW1"].ap().rearrange("(kt p) h -> p kt h", p=128))

    def veW1_sl(kt):
        return veW1_a[:, kt * H:(kt + 1) * H]

    def loadmat(name, key, rows, cols, dt=F32):
        t = pw.tile([rows, cols], dt, name=name)
        dma(out=t, in_=d[key].ap()[:, :])
        return t

    bnG_t = loadmat("bnG_t", "bnG", 128, 2)
    bnB_t = loadmat("bnB_t", "bnB", 128, 2)
    vnbs_t = loadmat("vnbs_t", "vnbs", 128, 2)
    snbs_t = loadmat("snbs_t", "snbs", 128, 2)
    veb1_t = loadmat("veb1_t", "veb1", H, 1)
    veb2_t = loadmat("veb2_t", "veb2", ISH, 1)
    b2o8_t = loadmat("b2o8_t", "b2o8", 128, NT)
    veW2_t = loadmat("veW2_t", "veW2", H, 1, BF16)
    fusUs_t = loadmat("fusUs_t", "fusUs", 128, 2, F32R)
    selv_t = []
    for jb, (j0, jw) in enumerate(N_MT):
        t = pw.tile([128, ISH], BF16, name=f"selv{jb}")
        dma(out=t[0:jw, :], in_=d["selv"].ap()[j0:j0 + jw, :])
        selv_t.append(t)

    # collective bounce buffers (fp16 payloads: 10-bit mantissa is enough —
    # verified against reference; halves on-wire bytes)
    ar_h2_in = pdram.tile([D, N], F16, name="ar_h2_in")
    ar_h2_out = pdram.tile([D, N], F16, addr_space="Shared", name="ar_h2_out")
    ag_zv_in = pdram.tile([DSH, N], F16, name="ag_zv_in")
    ag_zv_out = pdram.tile([D, N], F16, addr_space="Shared", name="ag_zv_out")
    ag_zs_in = pdram.tile([DSH, N], F16, name="ag_zs_in")
    ag_zs_out = pdram.tile([D, N], F16, addr_space="Shared", name="ag_zs_out")
    ag_ve_in = pdram.tile([ISH, N], F32R, name="ag_ve_in")
    ag_ve_out = pdram.tile([N, N], F32R, addr_space="Shared", name="ag_ve_out")
    ag_vu_in = pdram.tile([1, 512], F32R, name="ag_vu_in")
    ag_vu_out = pdram.tile([NCORES, 512], F32R, addr_space="Shared",
                           name="ag_vu_out")

    # =================================================================
    # P1: CooperationModule -> SP (f32r padded, d-major)
    # =================================================================
    offT = []
    for k in range(KEXP):
        row = []
        for kt, ksz in enumerate(S_KT):
            t = sh1k(f"offT{k}_{kt}", 3 * k + kt)
            nc.vector.memset(t[:, N:256].bitcast(F32), 0.0)
            nc.vector.tensor_scalar(
                out=t[0:ksz, 0:N], in0=at_f[kt][0:ksz, :],
                scalar1=centT_t[kt][0:ksz, k:k + 1], scalar2=None,
                op0=OP.subtract)
            row.append(t)
        offT.append(row)

    SP = [pa.tile([128, 256], F32R, name=f"SP{m}") for m in range(NT)]
    for m in range(NT):
        nc.vector.memset(SP[m][:, N:256].bitcast(F32), 0.0)
    for k in range(KEXP):
        for half in range(4):
            wst = []
            for kt, ksz in enumerate(S_KT):
                w = pstr.tile([128, D // 4], F32R, name=f"expw_st{kt}",
                              tag=f"str{kt}")
                dma(out=w[0:ksz, :],
                    in_=d["expW"].ap()[k, sum(S_KT[:kt]):sum(S_KT[:kt]) + ksz,
                                       half * 512:(half + 1) * 512])
                wst.append(w)
            for mh in range(4):
                m = half * 4 + mh
                ps = ppt([128, 256], "ps_exp")
                for kt, ksz in enumerate(S_KT):
                    nc.tensor.matmul(ps, wst[kt][0:ksz, mh * 128:(mh + 1) * 128],
                                     offT[k][kt][0:ksz, 0:256],
                                     start=(kt == 0),
                                     stop=(kt == len(S_KT) - 1))
                if k == 0:
                    nc.vector.tensor_scalar(out=SP[m][:, 0:N], in0=ps[:, 0:N],
                                            scalar1=expbt[m][:, 0:1],
                                            scalar2=0.0,
                                            op0=OP.add, op1=OP.max)
                else:
                    r1 = pt1.tile([128, N], F32, name="exr", tag="exr")
                    nc.scalar.activation(r1, ps[:, 0:N], AF.Relu,
                                         bias=expbt[m][:, k:k + 1], scale=1.0)
                    nc.vector.tensor_tensor(SP[m][:, 0:N], SP[m][:, 0:N], r1,
                                            OP.add)
    dtap16("SP", SP)

    # =================================================================
    # P2: semantic2visual -> AllReduce(h2)
    # =================================================================
    a1 = []
    for m2 in range(2):
        ps = ppt([128, 256], "ps_h1")
        for kt in range(NT):
            nc.tensor.matmul(ps, w1s_t[kt][:, m2 * 128:(m2 + 1) * 128],
                             SP[kt][:, 0:256], start=(kt == 0),
                             stop=(kt == NT - 1))
        st6 = pt.tile([128, 6], F32, name="bn_st", tag="bn_st")
        mv = pt.tile([128, 2], F32, name="bn_mv", tag="bn_mv")
        nc.vector.bn_stats(out=st6, in_=ps[:, 0:N])
        nc.vector.bn_aggr(out=mv, in_=st6)
        sd = pt.tile([128, 1], F32, name="bn_sd", tag="bn_sd")
        nc.scalar.activation(sd, mv[:, 1:2], AF.Sqrt, bias=epsc[:, 0:1],
                             scale=1.0)
        rs = pt.tile([128, 1], F32, name="bn_rs", tag="bn_rs")
        nc.vector.reciprocal(out=rs, in_=sd)
        Av = pt.tile([128, 1], F32, name="bn_A", tag="bn_A")
        nc.vector.tensor_tensor(Av, rs, bnG_t[:, m2:m2 + 1], OP.mult)
        Bt = pt.tile([128, 1], F32, name="bn_Bt", tag="bn_Bt")
        nc.vector.tensor_tensor(Bt, mv[:, 0:1], Av, OP.mult)
        Bv = pt.tile([128, 1], F32, name="bn_Bv", tag="bn_Bv")
        nc.vector.tensor_tensor(Bv, bnB_t[:, m2:m2 + 1], Bt, OP.subtract)
        t1 = pt.tile([128, N], F32, name="h1_t1", tag="zt1")
        nc.vector.tensor_scalar(out=t1, in0=ps[:, 0:N], scalar1=Av,
                                scalar2=Bv, op0=OP.mult, op1=OP.add)
        a1m = pa.tile([128, 256], F32R, name=f"a1_{m2}")
        nc.vector.memset(a1m[:, N:256].bitcast(F32), 0.0)
        nc.vector.scalar_tensor_tensor(out=a1m[:, 0:N], in0=t1, scalar=0.2,
                                       in1=t1, op0=OP.mult, op1=OP.max)
        dtap("a1", a1m[:, 0:N], row0=m2 * 128, rows=128)
        a1.append(a1m)

    for half in range(4):
        w2st = []
        for kt in range(2):
            w = pstr.tile([128, D // 4], F32R, name=f"w2_st{kt}",
                          tag=f"str{kt}")
            dma(out=w, in_=d["w2s"].ap()[kt * 128:(kt + 1) * 128,
                                         half * 512:(half + 1) * 512])
            w2st.append(w)
        for mh in range(4):
            m = half * 4 + mh
            ps = ppt([128, 256], "ps_h2")
            for kt2 in range(2):
                nc.tensor.matmul(ps, w2st[kt2][:, mh * 128:(mh + 1) * 128],
                                 a1[kt2][:, 0:256], start=(kt2 == 0),
                                 stop=(kt2 == 1))
            hp = pt.tile([128, N], F16, name="h2p", tag="zouts")
            nc.vector.tensor_scalar(out=hp, in0=ps[:, 0:N],
                                    scalar1=b2o8_t[:, m:m + 1],
                                    scalar2=None, op0=OP.add)
            dma(out=ar_h2_in[m * 128:(m + 1) * 128, :], in_=hp)
    nc.gpsimd.collective_compute("AllReduce", OP.add, replica_groups=GRP,
                                 ins=[ar_h2_in[:].opt()],
                                 outs=[ar_h2_out[:].opt()])

    # ---- znorm: load z (f16 payload), inst-norm, fin(m, z, t1) writes ----
    def znorm(src_dram, fin, zn, tagset):
        zh = []
        for m in range(NT):
            t = psh.tile([128, 256], F16, name=f"zh{zn}_{m}", tag=f"sh{m}")
            dma(out=t[:, 0:N], in_=src_dram[m * 128:(m + 1) * 128, :])
            nc.vector.memset(t[:, N:256].bitcast(F32), 0.0)
            zh.append(t)
        pstat = ppt_s([1, 512], f"st_{zn}")
        for m in range(NT):
            nc.tensor.matmul(pstat[0:1, 0:256], ones_h, zh[m][:, 0:256],
                             start=(m == 0), stop=(m == NT - 1))
        mu = pt1.tile([1, 256], F32R, name="zmu", tag="zmu")
        nc.vector.tensor_scalar_mul(mu, pstat[0:1, 0:256], 1.0 / D)
        pmu = ppt_h([128, 256], f"mub_{zn}")
        nc.tensor.matmul(pmu, ones1_r, mu, start=True, stop=True)
        z = []
        for m in range(NT):
            # centered z in f32r (f16->f32r conversion fused into subtract)
            t = pa.tile([128, 256], F32R, name=f"z{zn}_{m}",
                        tag=f"{tagset}{m}")
            nc.vector.memset(t[:, N:256].bitcast(F32), 0.0)
            nc.vector.tensor_tensor(t[:, 0:N], zh[m][:, 0:N], pmu[:, 0:N],
                                    OP.subtract)
            z.append(t)
            zq = pt.tile([128, 256], F32R, name="zq", tag="zq")
            nc.vector.tensor_tensor(zq, t, t, OP.mult)
            nc.tensor.matmul(pstat[0:1, 256:512], ones_r, zq[:, 0:256],
                             start=(m == 0), stop=(m == NT - 1))
        va = pt1.tile([1, 256], F32, name="zva", tag="zva")
        nc.vector.tensor_scalar(out=va, in0=pstat[0:1, 256:512],
                                scalar1=1.0 / D, scalar2=EPS,
                                op0=OP.mult, op1=OP.add)
        ta = pt1.tile([1, 256], F32, name="zta", tag="zmu2")
        nc.scalar.activation(ta, va, AF.Sqrt)
        rsf = pt1.tile([1, 256], F32, name="zrsf", tag="znm")
        nc.vector.reciprocal(out=rsf, in_=ta)
        rs = pt1.tile([1, 256], F32R, name="zrs", tag="zrs")
        nc.vector.tensor_copy(out=rs, in_=rsf)
        prr = ppt_h([128, 256], f"rb_{zn}")
        nc.tensor.matmul(prr, ones1_r, rs, start=True, stop=True)
        for m in range(NT):
            t1 = pt.tile([128, N], F32, name="zt1", tag="zt1")
            nc.vector.tensor_tensor(t1, z[m][:, 0:N], prr[:, 0:N], OP.mult)
            fin(m, z, t1)
        return z

    # h2 -> PVP (leaky), in-place in z set "za"
    def fin_pvp(m, z, t1):
        nc.vector.scalar_tensor_tensor(out=z[m][:, 0:N], in0=t1, scalar=0.2,
                                       in1=t1, op0=OP.mult, op1=OP.max)
    if "h2" in dbg:
        nc.sync.dma_start(out=dbg["h2"].ap()[:, :], in_=ar_h2_out[:, :])
    PVP = znorm(ar_h2_out, fin_pvp, "h2", "za")
    dtap16("PVP", PVP)

    # PVP_n (f32r) on the shared n-major chain
    def transpose_nmajor(src_tiles, name, tagbase, dt):
        out = [pa.tile([128, D], dt, name=f"{name}0", tag=f"{tagbase}0"),
               pa.tile([128, D], dt, name=f"{name}1", tag=f"{tagbase}1")]
        for m in range(NT):
            for jb, (j0, jw) in enumerate(N_MT):
                if dt == F32R:
                    ps = ppt_h([128, 128], "ps_tr", F32)
                    nc.tensor.transpose(
                        ps[0:jw, 0:128],
                        src_tiles[m][:, j0:j0 + jw].bitcast(F32), ident_f)
                else:
                    ps = ppt_h([128, 128], "ps_tr", dt)
                    nc.tensor.transpose(ps[0:jw, 0:128],
                                        src_tiles[m][:, j0:j0 + jw], ident_b)
                nc.vector.tensor_copy(
                    out=out[jb][0:jw, m * 128:(m + 1) * 128],
                    in_=ps[0:jw, 0:128])
        return out

    PVP_n = transpose_nmajor(PVP, "PVP_n", "nmj", F32R)

    # =================================================================
    # P3: cos edges (visual f32r, semantic bf16 gram -> f32r edges)
    # =================================================================
    def cos_edge(x_tiles, ksizes, en, rdt, ones_g, rhs_w):
        nkt = len(ksizes)
        pn = ppt_s([1, 256] if rhs_w == 256 else [1, N], f"nrm_{en}")
        for kt, ksz in enumerate(ksizes):
            xq = pt.tile([128, rhs_w], rdt, name="xq", tag="xq")
            nc.vector.tensor_tensor(xq[0:ksz, :], x_tiles[kt][0:ksz, 0:rhs_w],
                                    x_tiles[kt][0:ksz, 0:rhs_w], OP.mult)
            nc.tensor.matmul(pn, ones_g[0:ksz, :], xq[0:ksz, :],
                             start=(kt == 0), stop=(kt == nkt - 1))
        sd = pt.tile([1, N], F32, name="esd", tag="zmu")
        nc.scalar.activation(sd, pn[0:1, 0:N], AF.Sqrt)
        rn_f = pt1.tile([1, 256], F32, name="ern_f", tag="zva")
        nc.vector.memset(rn_f[0:1, N:256], 0.0)
        nc.vector.reciprocal(out=rn_f[0:1, 0:N], in_=sd)
        rn = pt1.tile([1, 256], F32R, name="ern", tag="zAB")
        nc.vector.tensor_copy(out=rn, in_=rn_f)
        prn = ppt_h([128, 256], f"rnb_{en}")
        nc.tensor.matmul(prn, ones1_r, rn, start=True, stop=True)
        rcol = pt.tile([128, 2], F32, name="rc", tag=f"rc_{en}")
        for mt, (i0, iw) in enumerate(N_MT):
            pst = ppt_h([128, 128], "ps_tr3")
            nc.tensor.transpose(pst[0:iw, 0:1], rn_f[0:1, i0:i0 + iw],
                                ident_f[0:1, 0:1])
            nc.vector.tensor_copy(out=rcol[0:iw, mt:mt + 1],
                                  in_=pst[0:iw, 0:1])
        edge = []
        for mt, (i0, iw) in enumerate(N_MT):
            ps = ppt([128, rhs_w], f"ps_{en}")
            for kt, ksz in enumerate(ksizes):
                nc.tensor.matmul(ps[0:iw, :], x_tiles[kt][0:ksz, i0:i0 + iw],
                                 x_tiles[kt][0:ksz, 0:rhs_w],
                                 start=(kt == 0), stop=(kt == nkt - 1))
            s1 = pt.tile([128, N], F32, name="es1", tag="zt1")
            nc.vector.tensor_scalar(out=s1[0:iw, :], in0=ps[0:iw, 0:N],
                                    scalar1=rcol[0:iw, mt:mt + 1],
                                    scalar2=None, op0=OP.mult)
            nc.vector.tensor_tensor(s1[0:iw, :], s1[0:iw, :], prn[0:iw, 0:N],
                                    OP.mult)
            rmx = pt.tile([128, 1], F32, name="ermx", tag="ermx")
            nc.vector.reduce_max(rmx[0:iw, :], s1[0:iw, :], axis=AX.X)
            bia = pt.tile([128, 1], F32, name="ebia", tag="ebia")
            nc.vector.tensor_scalar_mul(bia[0:iw, :], rmx[0:iw, :], -100.0)
            nc.scalar.activation(s1[0:iw, :], s1[0:iw, :], AF.Exp,
                                 bias=bia[0:iw, 0:1], scale=100.0)
            sm = pt.tile([128, 1], F32, name="esm", tag="esm")
            nc.vector.reduce_sum(sm[0:iw, :], s1[0:iw, :], axis=AX.X)
            rr = pt.tile([128, 1], F32, name="err", tag="err")
            nc.vector.reciprocal(out=rr[0:iw, :], in_=sm[0:iw, :])
            ed = pa.tile([128, N], F32R, name=f"{en}_{mt}",
                         tag=f"edg_{en}_{mt}")
            nc.vector.tensor_scalar(out=ed[0:iw, :], in0=s1[0:iw, :],
                                    scalar1=rr[0:iw, 0:1], scalar2=None,
                                    op0=OP.mult)
            edge.append(ed)
        edgeT = [pa.tile([128, 256], F32R, name=f"{en}T0", tag=f"{en}T0"),
                 pa.tile([128, 256], F32R, name=f"{en}T1", tag=f"{en}T1")]
        for jb in range(2):
            nc.vector.memset(edgeT[jb][:].bitcast(F32), 0.0)
        for mt, (i0, iw) in enumerate(N_MT):
            for jb, (j0, jw) in enumerate(N_MT):
                pst = ppt_h([128, 128], "ps_tr4", F32)
                nc.tensor.transpose(pst[0:jw, 0:iw],
                                    edge[mt][0:iw, j0:j0 + jw].bitcast(F32),
                                    ident_f[0:iw, 0:iw])
                nc.vector.tensor_copy(out=edgeT[jb][0:jw, i0:i0 + iw],
                                      in_=pst[0:jw, 0:iw])
        return edge, edgeT

    ones_b = pa.tile([128, 1], BF16, name="ones_b")
    nc.vector.memset(ones_b, 1.0)
    vedge, vedgeT = cos_edge(PVP, [128] * NT, "ve", F32R, ones_r, 256)
    sedge, sedgeT = cos_edge(at_b, S_KT, "se", BF16, ones_b, N)
    if "vedge" in dbg:
        for mt, (i0, iw) in enumerate(N_MT):
            dtap("vedge", vedge[mt][0:iw, :], row0=i0, rows=iw)
            dtap("sedge", sedge[mt][0:iw, :], row0=i0, rows=iw)

    # =================================================================
    # P4: UpdateVisualNode -> AllGather(z_v) -> VP2 (in-place set "zb")
    # =================================================================
    vp_f, y_r = [], []
    for m in range(NT):
        pv = pp.tile([128, 256], F32, name="ps_vp", tag="mm")
        pe = pp.tile([128, 256], F32, name="ps_ev", tag="mm")
        for jb, (j0, jw) in enumerate(N_MT):
            nc.tensor.matmul(pv, PVP_n[jb][0:jw, m * 128:(m + 1) * 128],
                             vedgeT[jb][0:jw, :], start=(jb == 0),
                             stop=(jb == 1))
            nc.tensor.matmul(pe, PVP_n[jb][0:jw, m * 128:(m + 1) * 128],
                             sedgeT[jb][0:jw, :], start=(jb == 0),
                             stop=(jb == 1))
        vf = pa.tile([128, N], F32, name=f"vp_f{m}")
        nc.scalar.copy(out=vf, in_=pv[:, 0:N])
        vp_f.append(vf)
        yb = sh1k(f"y_{m}", m)
        nc.vector.memset(yb[:, N:256].bitcast(F32), 0.0)
        nc.vector.tensor_tensor(yb[:, 0:N], vf, pe[:, 0:N], OP.add)
        y_r.append(yb)
    dtap16("ybf", y_r)

    for m2 in range(2):
        ps = ppt([128, 256], "ps_zv")
        for kt in range(NT):
            nc.tensor.matmul(ps, vnWs_t[kt][:, m2 * 128:(m2 + 1) * 128],
                             y_r[kt][:, 0:256], start=(kt == 0),
                             stop=(kt == NT - 1))
        zc = pt.tile([128, N], F16, name="zvc", tag="zouts")
        nc.vector.tensor_scalar(out=zc, in0=ps[:, 0:N],
                                scalar1=vnbs_t[:, m2:m2 + 1],
                                scalar2=None, op0=OP.add)
        dma(out=ag_zv_in[m2 * 128:(m2 + 1) * 128, :], in_=zc)
    nc.gpsimd.collective_compute("AllGather", OP.bypass, replica_groups=GRP,
                                 ins=[ag_zv_in[:].opt()],
                                 outs=[ag_zv_out[:].opt()])
    if "zv" in dbg:
        nc.sync.dma_start(out=dbg["zv"].ap()[:, :], in_=ag_zv_out[:, :])

    VP2_bf = [pa.tile([128, N], BF16, name=f"VP2b{m}", tag=f"VP2b{m}")
              for m in range(NT)]

    def fin_vp2(m, z, t1):
        nc.vector.scalar_tensor_tensor(out=z[m][:, 0:N], in0=t1, scalar=0.0,
                                       in1=vp_f[m], op0=OP.max, op1=OP.add)
        nc.vector.tensor_copy(out=VP2_bf[m], in_=z[m][:, 0:N])
    VP2 = znorm(ag_zv_out, fin_vp2, "zv", "zb")
    dtap16("VP2", VP2_bf)

    # =================================================================
    # P6: UpdateVisualEdge (i-sharded, bf16 path)
    # =================================================================
    VP2_n = transpose_nmajor(VP2_bf, "VP2_n", "nmj", BF16)
    xmyn = pw.tile([ISH, D], BF16, name="xmyn")
    for ch in range(4):
        ps = ppt([ISH, 512], "ps_xmy")
        for jb, (j0, jw) in enumerate(N_MT):
            nc.tensor.matmul(ps, selv_t[jb][0:jw, :],
                             VP2_n[jb][0:jw, ch * 512:(ch + 1) * 512],
                             start=(jb == 0), stop=(jb == 1))
        nc.vector.tensor_copy(out=xmyn[:, ch * 512:(ch + 1) * 512], in_=ps)
    dtap("xmyn", xmyn[:, :])
    xmyd, negx2my = [], []
    for kt in range(NT):
        pst = ppt_h([128, 128], "ps_tr5", BF16)
        nc.tensor.transpose(pst[0:128, 0:ISH],
                            xmyn[:, kt * 128:(kt + 1) * 128],
                            ident_b[0:ISH, 0:ISH])
        xd = pa.tile([128, ISH], BF16, name=f"xmyd{kt}")
        nc.vector.tensor_copy(out=xd, in_=pst[0:128, 0:ISH])
        xmyd.append(xd)
        ng = pa.tile([128, ISH], BF16, name=f"negx2my{kt}")
        nc.vector.tensor_scalar_mul(ng, pst[0:128, 0:ISH], -2.0)
        negx2my.append(ng)
    pA = ppt([128, N], "ps_A")
    pAm = ppt([128, ISH], "ps_Am")
    for m in range(NT):
        xq = pt.tile([128, N], BF16, name="vsq", tag="xq")
        nc.vector.tensor_tensor(xq, VP2_bf[m], VP2_bf[m], OP.mult)
        nc.tensor.matmul(pA, veW1_sl(m), xq[:, :], start=(m == 0),
                         stop=(m == NT - 1))
        xqm = pt.tile([128, ISH], BF16, name="vsqm", tag="vsqm")
        nc.vector.tensor_tensor(xqm, xmyd[m], xmyd[m], OP.mult)
        nc.tensor.matmul(pAm, veW1_sl(m), xqm[:, :], start=(m == 0),
                         stop=(m == NT - 1))
    A_T = pa.tile([128, N], F32, name="A_T")
    nc.vector.tensor_scalar(out=A_T[0:H, :], in0=pA[0:H, :],
                            scalar1=veb1_t[0:H, 0:1], scalar2=None, op0=OP.add)
    A_my = pa.tile([128, ISH], F32, name="A_my")
    nc.vector.tensor_scalar(out=A_my[0:H, :], in0=pAm[0:H, :],
                            scalar1=veb1_t[0:H, 0:1], scalar2=None, op0=OP.add)
    dtap("AT", A_T[0:128, 0:N])
    dtap("Amy", A_my[0:128, :])
    vedge_my = pa.tile([ISH, N], F32, name="vedge_my", tag="edg_se_0")
    psvm = ppt([ISH, N], "ps_vm")
    for mt, (i0, iw) in enumerate(N_MT):
        vb = pt.tile([128, N], BF16, name="vedgb", tag="xq")
        nc.vector.tensor_copy(out=vb[0:iw, :], in_=vedge[mt][0:iw, :])
        nc.tensor.matmul(psvm, selv_t[mt][0:iw, :], vb[0:iw, :],
                         start=(mt == 0), stop=(mt == 1))
    nc.vector.tensor_copy(out=vedge_my, in_=psvm)

    cur_sb = pa.tile([ISH, N], F32, name="cur_sb", tag="atf2")
    groups = [(g0, min(G_I, ISH - g0)) for g0 in range(0, ISH, G_I)]
    for i0, gsz in groups:
        tgt = []
        for kt in range(NT):
            tt = psh.tile([128, G_I * N], BF16, name=f"tg{kt}", tag=f"sh{kt}")
            in0 = _rep(VP2_bf[kt][:, :], [[0, gsz], [1, N]])
            in1 = _rep(negx2my[kt][:, i0:i0 + gsz], [[1, gsz], [0, N]])
            out3 = _rep(tt[:, :], [[N, gsz], [1, N]])
            eng = nc.vector if kt % 2 == 0 else nc.gpsimd
            eng.tensor_tensor(out3, in0, in1, OP.mult)
            tgt.append(tt)
        for il in range(gsz):
            ii = i0 + il
            ph = ppt([128, N], "ps_eh")
            for kt in range(NT):
                nc.tensor.matmul(ph, veW1_sl(kt),
                                 tgt[kt][:, il * N:(il + 1) * N],
                                 start=(kt == 0), stop=(kt == NT - 1))
            hsb = pt.tile([128, 256], F32R, name="hsb", tag="zq")
            nc.vector.scalar_tensor_tensor(
                out=hsb[0:H, 0:N], in0=ph[0:H, :],
                scalar=A_my[0:H, ii:ii + 1], in1=A_T[0:H, :],
                op0=OP.add, op1=OP.add)
            nc.vector.memset(hsb[0:H, N:256].bitcast(F32), 0.0)
            hsq = pt.tile([128, 256], F32R, name="hsq", tag="fth")
            nc.vector.tensor_tensor(hsq[0:H, :], hsb[0:H, :], hsb[0:H, :],
                                    OP.mult)
            pst = ppt_s([1, 512], "st_e")
            nc.tensor.matmul(pst[0:1, 0:256], ones_r[0:H, :], hsb[0:H, 0:256],
                             start=True, stop=True)
            nc.tensor.matmul(pst[0:1, 256:512], ones_r[0:H, :],
                             hsq[0:H, 0:256], start=True, stop=True)
            stt = pt1.tile([1, 512], F32, name="estt", tag="zstt")
            nc.vector.tensor_copy(out=stt, in_=pst)
            mu = pt1.tile([1, 256], F32, name="emu", tag="zmu")
            nc.vector.tensor_scalar_mul(mu, stt[0:1, 0:256], 1.0 / H)
            va = pt1.tile([1, 256], F32, name="eva", tag="zva")
            nc.vector.tensor_scalar_mul(va, stt[0:1, 256:512], 1.0 / H)
            ta = pt1.tile([1, 256], F32, name="eta", tag="zmu2")
            nc.vector.tensor_tensor(ta, mu, mu, OP.mult)
            nc.vector.tensor_tensor(va, va, ta, OP.subtract)
            nc.vector.tensor_scalar_add(va, va, EPS)
            nc.scalar.activation(ta, va, AF.Sqrt)
            rs = pt1.tile([1, 256], F32, name="ers2", tag="zrs")
            nc.vector.reciprocal(out=rs, in_=ta)
            AB = pt1.tile([1, 512], F32R, name="eAB", tag="zAB")
            nc.vector.tensor_copy(out=AB[0:1, 0:256], in_=rs)
            nc.vector.tensor_tensor(ta, mu, rs, OP.mult)
            nc.vector.tensor_scalar_mul(AB[0:1, 256:512], ta, -1.0)
            pab = ppt_h([128, 512], "ab_e")
            nc.tensor.matmul(pab, ones1_r, AB, start=True, stop=True)
            t1 = pt.tile([128, N], F32, name="et1", tag="zt1")
            nc.vector.tensor_tensor(t1[0:H, :], hsb[0:H, 0:N], pab[0:H, 0:N],
                                    OP.mult)
            nc.vector.tensor_tensor(t1[0:H, :], t1[0:H, :],
                                    pab[0:H, 256:256 + N], OP.add)
            h2b = pt.tile([128, N], BF16, name="eh2b", tag="xq")
            nc.vector.tensor_scalar_max(h2b[0:H, :], t1[0:H, :], 0.0)
            pcur = ppt_s([1, N], "ps_cur")
            nc.tensor.matmul(pcur, veW2_t[0:H, 0:1], h2b[0:H, :],
                             start=True, stop=True)
            cst = pt.tile([1, N], F32, name="cst", tag="cst")
            nc.vector.tensor_copy(out=cst, in_=pcur)
            dma(out=cur_sb[ii:ii + 1, :], in_=cst)
    dtap("cur", cur_sb[:, :])

    # tanh(cur + b2) * (vedge_my + 1e-8) -> softmax(/10) -> my edge rows
    curt = pa.tile([ISH, N], F32, name="curt", tag="atf0")
    nc.scalar.activation(curt, cur_sb, AF.Tanh, bias=veb2_t[0:ISH, 0:1],
                         scale=1.0)
    ne = pa.tile([ISH, N], F32, name="ne", tag="atf1")
    nc.vector.scalar_tensor_tensor(out=ne, in0=vedge_my, scalar=1e-8,
                                   in1=curt, op0=OP.add, op1=OP.mult)
    rmx = pt.tile([ISH, 1], F32, name="vermx", tag="vermx")
    nc.vector.reduce_max(rmx, ne, axis=AX.X)
    bia = pt.tile([ISH, 1], F32, name="vebia", tag="vebia")
    nc.vector.tensor_scalar_mul(bia, rmx, -0.1)
    ex = pt1.tile([ISH, N], F32, name="veex", tag="veex")
    nc.scalar.activation(ex, ne, AF.Exp, bias=bia[0:ISH, 0:1], scale=0.1)
    sm = pt.tile([ISH, 1], F32, name="vesm", tag="vesm")
    nc.vector.reduce_sum(sm, ex, axis=AX.X)
    rr = pt.tile([ISH, 1], F32, name="verr", tag="verr")
    nc.vector.reciprocal(out=rr, in_=sm)
    vemine = pt1.tile([ISH, N], F32R, name="vemine", tag="vemine")
    nc.vector.tensor_scalar(out=vemine, in0=ex, scalar1=rr[0:ISH, 0:1],
                            scalar2=None, op0=OP.mult)
    dma(out=ag_ve_in[:, :], in_=vemine)
    nc.gpsimd.collective_compute("AllGather", OP.bypass, replica_groups=GRP,
                                 ins=[ag_ve_in[:].opt()],
                                 outs=[ag_ve_out[:].opt()])
    if "ve2" in dbg:
        nc.sync.dma_start(out=dbg["ve2"].ap()[:, :], in_=ag_ve_out[:, :])
    ve2 = [pt1.tile([128, N], F32R, name="ve2_0", tag="ve2_0"),
           pt1.tile([128, N], F32R, name="ve2_1", tag="ve2_1")]
    for mt, (i0, iw) in enumerate(N_MT):
        dma(out=ve2[mt][0:iw, :], in_=ag_ve_out[i0:i0 + iw, :])
    ve2T = [pa.tile([128, 256], F32R, name="ve2T0", tag="veT0"),
            pa.tile([128, 256], F32R, name="ve2T1", tag="veT1")]
    for mt, (i0, iw) in enumerate(N_MT):
        for jb, (j0, jw) in enumerate(N_MT):
            pst = ppt_h([128, 128], "ps_tr6", F32)
            nc.tensor.transpose(pst[0:jw, 0:iw],
                                ve2[mt][0:iw, j0:j0 + jw].bitcast(F32),
                                ident_f[0:iw, 0:iw])
            nc.vector.tensor_copy(out=ve2T[jb][0:jw, i0:i0 + iw],
                                  in_=pst[0:jw, 0:iw])

    # =================================================================
    # P5: UpdateSemanticNode -> AllGather(z_s) -> SP2 (in-place set "za")
    # =================================================================
    SP_n = transpose_nmajor(SP, "SP_n", "nmj", F32R)
    sp_f, y2_r = [], []
    for m in range(NT):
        psp = pp.tile([128, 256], F32, name="ps_sp", tag="mm")
        pes = pp.tile([128, 256], F32, name="ps_es", tag="mm")
        for jb, (j0, jw) in enumerate(N_MT):
            nc.tensor.matmul(psp, SP_n[jb][0:jw, m * 128:(m + 1) * 128],
                             sedgeT[jb][0:jw, :], start=(jb == 0),
                             stop=(jb == 1))
            nc.tensor.matmul(pes, SP_n[jb][0:jw, m * 128:(m + 1) * 128],
                             ve2T[jb][0:jw, :], start=(jb == 0),
                             stop=(jb == 1))
        sf = pa.tile([128, N], F32, name=f"sp_f{m}", tag=f"vp_f{m}")
        nc.scalar.copy(out=sf, in_=psp[:, 0:N])
        sp_f.append(sf)
        yb = sh1k(f"y2_{m}", m)
        nc.vector.memset(yb[:, N:256].bitcast(F32), 0.0)
        nc.vector.tensor_tensor(yb[:, 0:N], sf, pes[:, 0:N], OP.add)
        y2_r.append(yb)

    for m2 in range(2):
        ps = ppt([128, 256], "ps_zs")
        for kt in range(NT):
            nc.tensor.matmul(ps, snWs_t[kt][:, m2 * 128:(m2 + 1) * 128],
                             y2_r[kt][:, 0:256], start=(kt == 0),
                             stop=(kt == NT - 1))
        zc = pt.tile([128, N], F16, name="zsc", tag="zouts")
        nc.vector.tensor_scalar(out=zc, in0=ps[:, 0:N],
                                scalar1=snbs_t[:, m2:m2 + 1],
                                scalar2=None, op0=OP.add)
        dma(out=ag_zs_in[m2 * 128:(m2 + 1) * 128, :], in_=zc)
    nc.gpsimd.collective_compute("AllGather", OP.bypass, replica_groups=GRP,
                                 ins=[ag_zs_in[:].opt()],
                                 outs=[ag_zs_out[:].opt()])

    def fin_sp2(m, z, t1):
        nc.vector.scalar_tensor_tensor(out=z[m][:, 0:N], in0=t1, scalar=0.0,
                                       in1=sp_f[m], op0=OP.max, op1=OP.add)
    SP2 = znorm(ag_zs_out, fin_sp2, "zs", "za")
    if "SP2" in dbg:
        for m in range(NT):
            nc.sync.dma_start(out=dbg["SP2"].ap()[m * 128:(m + 1) * 128, :],
                              in_=SP2[m][:, 0:N])

    # =================================================================
    # P7: FusionLayer (f32r) -> alpha -> prob
    # =================================================================
    pvu = [ppt_s([1, 256], "ps_vu0"), ppt_s([1, 256], "ps_vu1")]
    for k, srct in enumerate((VP2, SP2)):
        for m2 in range(2):
            ps = ppt([128, 256], "ps_fus")
            for kt in range(NT):
                nc.tensor.matmul(ps, fusWs_t[kt][:, m2 * 128:(m2 + 1) * 128],
                                 srct[kt][:, 0:256], start=(kt == 0),
                                 stop=(kt == NT - 1))
            th = pt.tile([128, 256], F32R, name="fth", tag="fth")
            nc.scalar.activation(th, ps, AF.Tanh)
            nc.tensor.matmul(pvu[k], fusUs_t[:, m2:m2 + 1], th[:, :],
                             start=(m2 == 0), stop=(m2 == 1))
    vu_sb = pt1.tile([1, 512], F32R, name="vu_sb", tag="vu_sb")
    nc.vector.memset(vu_sb[:].bitcast(F32), 0.0)
    nc.vector.tensor_copy(out=vu_sb[0:1, 0:N], in_=pvu[0][0:1, 0:N])
    nc.vector.tensor_copy(out=vu_sb[0:1, 256:256 + N], in_=pvu[1][0:1, 0:N])
    dma(out=ag_vu_in[:, :], in_=vu_sb)
    nc.gpsimd.collective_compute("AllGather", OP.bypass, replica_groups=GRP,
                                 ins=[ag_vu_in[:].opt()],
                                 outs=[ag_vu_out[:].opt()])
    vus = pt1.tile([NCORES, 512], F32R, name="vus", tag="vu_sb")
    dma(out=vus, in_=ag_vu_out[:, :])
    pvk = ppt_s([1, 512], "ps_vuk")
    for k in range(2):
        nc.tensor.matmul(pvk[0:1, 256 * k:256 * k + 256],
                         ones8_r, vus[:, 256 * k:256 * k + 256],
                         start=True, stop=True)
    vuf = pt1.tile([1, 512], F32, name="vuf", tag="zstt")
    nc.vector.tensor_copy(out=vuf, in_=pvk)
    dtap("vuf", vuf[:, :])
    mx = pt.tile([1, N], F32, name="amx", tag="amx")
    nc.vector.tensor_tensor(mx, vuf[0:1, 0:N], vuf[0:1, 256:256 + N], OP.max)
    dv = pt1.tile([1, 512], F32R, name="adv", tag="adv")
    nc.vector.memset(dv[:].bitcast(F32), 0.0)
    for k in range(2):
        nc.vector.tensor_tensor(dv[0:1, 256 * k:256 * k + N],
                                vuf[0:1, 256 * k:256 * k + N], mx, OP.subtract)
    nc.scalar.activation(dv, dv, AF.Exp, scale=100.0)
    ssum = pt.tile([1, N], F32, name="assum", tag="assum")
    nc.vector.tensor_tensor(ssum, dv[0:1, 0:N], dv[0:1, 256:256 + N], OP.add)
    rsu = pt.tile([1, N], F32, name="arsu", tag="arsu")
    nc.vector.reciprocal(out=rsu, in_=ssum)
    for k in range(2):
        nc.vector.tensor_tensor(dv[0:1, 256 * k:256 * k + N],
                                dv[0:1, 256 * k:256 * k + N], rsu, OP.mult)
    alro = dv
    if "alpha" in dbg:
        al_f = pt1.tile([1, 512], F32, name="al_f", tag="zstt")
        nc.vector.tensor_copy(out=al_f, in_=alro)
        dtap("alpha", al_f[:, :])
    pal = ppt_h([128, 512], "ab_al")
    nc.tensor.matmul(pal, ones1_r, alro, start=True, stop=True)
    proto_bf = []
    for m in range(NT):
        t1 = pt.tile([128, N], F32, name="pr1", tag="zouts")
        nc.vector.tensor_tensor(t1, VP2[m][:, 0:N], pal[:, 0:N], OP.mult)
        t2 = pt.tile([128, N], F32, name="pr2", tag="zt1")
        nc.vector.tensor_tensor(t2, SP2[m][:, 0:N], pal[:, 256:256 + N],
                                OP.mult)
        pb = pa.tile([128, N], BF16, name=f"proto{m}", tag=f"VP2b{m}")
        nc.vector.tensor_tensor(pb, t1, t2, OP.add)
        proto_bf.append(pb)
    for bt in range(2):
        ps = ppt([128, N], "ps_prob")
        for kt in range(NT):
            nc.tensor.matmul(ps, imgT_sl(kt, bt), proto_bf[kt][:, :],
                             start=(kt == 0), stop=(kt == NT - 1))
        t1 = pt.tile([128, N], F32, name="probf", tag="zouts")
        nc.vector.tensor_copy(out=t1, in_=ps)
        dma(out=prob_out.ap()[bt * 128:(bt + 1) * 128, :], in_=t1)


# =====================================================================
# Host side
# =====================================================================
def _prep_inputs(inputs):
    bf = ml_dtypes.bfloat16
    f32 = np.float32
    att = np.asarray(inputs["attribute"], f32)
    cen = np.asarray(inputs["centers"], f32)
    expW = np.asarray(inputs["expert_W"], f32)
    expB = np.asarray(inputs["expert_b"], f32)
    w1 = np.asarray(inputs["s2v_W1"], f32)
    w2 = np.asarray(inputs["s2v_W2"], f32)
    in_maps = []
    for c in range(NCORES):
        cs = slice(c * DSH, (c + 1) * DSH)
        isl = slice(c * ISH, (c + 1) * ISH)
        bs = slice(c * BSH, (c + 1) * BSH)
        selv = np.zeros((N, ISH), f32)
        selv[np.arange(c * ISH, (c + 1) * ISH), np.arange(ISH)] = 1.0
        m = {
            "attrT": np.ascontiguousarray(att.T),
            "attrTb": np.ascontiguousarray(att.T).astype(bf),
            "centT": np.ascontiguousarray(cen.T),
            "expW": expW,
            "expBT": np.ascontiguousarray(expB.T),
            "w1s": np.ascontiguousarray(w1[:, cs]),
            "bnG": np.ascontiguousarray(np.asarray(inputs["bn_g"], f32)[cs].reshape(2, 128).T),
            "bnB": np.ascontiguousarray(np.asarray(inputs["bn_b"], f32)[cs].reshape(2, 128).T),
            "w2s": np.ascontiguousarray(w2[cs, :]),
            "b2o8": np.ascontiguousarray((np.asarray(inputs["s2v_b2"], f32) / NCORES).reshape(NT, 128).T),
            "vnWs": np.ascontiguousarray(np.asarray(inputs["vn_W"], f32)[:, cs]),
            "vnbs": np.ascontiguousarray(np.asarray(inputs["vn_b"], f32)[cs].reshape(2, 128).T),
            "snWs": np.ascontiguousarray(np.asarray(inputs["sn_W"], f32)[:, cs]),
            "snbs": np.ascontiguousarray(np.asarray(inputs["sn_b"], f32)[cs].reshape(2, 128).T),
            "veW1": np.asarray(inputs["ve_W1"], f32).astype(bf),
            "veb1": np.asarray(inputs["ve_b1"], f32)[:, None],
            "veW2": np.asarray(inputs["ve_W2"], f32).astype(bf),
            "veb2": np.full((ISH, 1), float(np.asarray(inputs["ve_b2"])[0]),
                            f32),
            "fusWs": np.ascontiguousarray(np.asarray(inputs["fus_W"], f32)[:, cs]),
            "fusUs": np.ascontiguousarray(np.asarray(inputs["fus_u"], f32)[cs, 0].reshape(2, 128).T),
            "imgT": np.ascontiguousarray(
                np.asarray(inputs["img_feat"], f32)[bs, :].T).astype(bf),
            "selv": selv.astype(bf),
        }
        in_maps.append(m)
    return in_maps


def kernel(**inputs):
    global _BUILT
    if _BUILT is None:
        _BUILT = build()
    nc = _BUILT
    in_maps = _prep_inputs(inputs)
    res = run_bass_kernel_spmd(nc, in_maps, core_ids=list(range(NCORES)))
    out = np.concatenate([res.results[c]["prob"] for c in range(NCORES)],
                         axis=0)
    return out.astype(np.float32)


def kernel_debug(**inputs):
    nc = build(debug=True)
    in_maps = _prep_inputs(inputs)
    res = run_bass_kernel_spmd(nc, in_maps, core_ids=list(range(NCORES)))
    out = np.concatenate([res.results[c]["prob"] for c in range(NCORES)],
                         axis=0)
    return out.astype(np.float32), res.results


if __name__ == "__main__":
    import reference
    inp = {k: np.asarray(v) for k, v in reference.setup_inputs().items()}
    got = kernel(**inp)
    exp = np.asarray(reference.reference(**reference.setup_inputs()))
    err = np.abs(got - exp).max() / (np.abs(exp).max() + 1e-9)
    print("Relative error:", err)



# revision 35
# speedup vs baseline: 1.2755x; 1.1601x over previous
# Trainium2 Bass kernel for nn_CPPN (gnn_message_passing), 8-core SPMD.
#
# Sharding:
#   - Node-MLP weights (s2v_W1/W2, vn_W, sn_W, fus_W) sharded over the 2048
#     col/row dim (256 per core); stitched with one AllReduce (h2) and three
#     AllGathers (z_v, z_s, vu partials).
#   - Visual edge MLP (200x200 pairwise rows) sharded over i: 25 rows/core,
#     AllGather of the resulting edge rows.  Per-core column selection is via
#     a host-supplied one-hot matrix (SPMD program is identical on all cores;
#     only input data differs).
#   - img_feat batch sharded 256 rows/core for the final prob matmul; host
#     concatenates per-core outputs.
# Layout: activations are d-major [D on partitions (16x128), 200 on free].
# Heavy matmuls bf16; stats/broadcast matmuls f32r.  The reference's second
# edge_update (semantic) is dead code w.r.t. prob and is skipped.

import sys

sys.path.insert(0, "/opt/trn_rl_repo")

import numpy as np
import ml_dtypes

import concourse.bass as bass
import concourse.bacc as bacc
import concourse.tile as tile
from concourse import mybir
from concourse.bass_utils import run_bass_kernel_spmd
from concourse.masks import make_identity

F32 = mybir.dt.float32
F32R = mybir.dt.float32r
BF16 = mybir.dt.bfloat16
F16 = mybir.dt.float16
AF = mybir.ActivationFunctionType
OP = mybir.AluOpType
AX = mybir.AxisListType

NCORES = 8
N = 200
S = 312
D = 2048
H = 128
KEXP = 3
B = 2048
DSH = D // NCORES      # 256
ISH = N // NCORES      # 25
BSH = B // NCORES      # 256
EPS = 1e-5
NT = D // 128          # 16
S_KT = [128, 128, 56]
N_MT = ((0, 128), (128, 72))
G_I = 2                # i-group size for edge t-gen batching

_BUILT = None


def _rep(ap_src, dims):
    """Rebuild AP with explicit free dims [[step,count],...] after partition."""
    return bass.AP(tensor=ap_src.tensor, offset=ap_src.offset,
                   ap=[ap_src.ap[0]] + dims)


def build(debug=False):
    nc = bacc.Bacc("TRN2", target_bir_lowering=False, debug=False,
                   num_devices=NCORES)
    d = {}

    def din(name, shape, dt):
        d[name] = nc.dram_tensor(name, shape, dt, kind="ExternalInput")

    din("attrT", [S, N], F32)
    din("attrTb", [S, N], BF16)
    din("centT", [S, KEXP], F32)
    din("expW", [KEXP, S, D], F32R)
    din("expBT", [D, KEXP], F32)
    din("w1s", [D, DSH], F32R)
    din("bnG", [128, 2], F32)
    din("bnB", [128, 2], F32)
    din("w2s", [DSH, D], F32R)
    din("b2o8", [128, NT], F32)
    din("vnWs", [D, DSH], F32R)
    din("vnbs", [128, 2], F32)
    din("snWs", [D, DSH], F32R)
    din("snbs", [128, 2], F32)
    din("veW1", [D, H], BF16)
    din("veb1", [H, 1], F32)
    din("veW2", [H, 1], BF16)
    din("veb2", [ISH, 1], F32)
    din("fusWs", [D, DSH], F32R)
    din("fusUs", [128, 2], F32R)
    din("imgT", [D, BSH], BF16)
    din("selv", [N, ISH], BF16)
    prob_out = nc.dram_tensor("prob", [BSH, N], F32, kind="ExternalOutput")
    dbg = {}
    if debug:
        def dout(name, shape, dt):
            dbg[name] = nc.dram_tensor("dbg_" + name, shape, dt,
                                       kind="ExternalOutput")
        dout("SP", [D, N], F32R)
        dout("a1", [2 * 128, N], F32R)
        dout("h2", [D, N], F16)
        dout("PVP", [D, N], F32R)
        dout("vedge", [N, N], F32R)
        dout("sedge", [N, N], F32R)
        dout("ybf", [D, N], F32R)
        dout("zv", [D, N], F16)
        dout("VP2", [D, N], BF16)
        dout("AT", [128, N], F32)
        dout("Amy", [128, ISH], F32)
        dout("xmyn", [ISH, D], BF16)
        dout("cur", [ISH, N], F32)
        dout("ve2", [N, N], F32R)
        dout("SP2", [D, N], F32R)
        dout("vuf", [1, 512], F32)
        dout("alpha", [1, 512], F32)

    with tile.TileContext(nc) as tc:
        import contextlib
        with contextlib.ExitStack() as ctx, \
                nc.allow_low_precision(reason="bf16 PE transposes (no accum)"):
            _emit(ctx, nc, tc, d, prob_out, dbg)
    nc.compile()
    return nc


def _emit(ctx, nc, tc, d, prob_out, dbg=None):
    dbg = dbg or {}

    def dtap16(key, tiles):     # 16 d-major tiles -> [D, N] dram
        if key in dbg:
            for m in range(NT):
                nc.sync.dma_start(out=dbg[key].ap()[m * 128:(m + 1) * 128, :],
                                  in_=tiles[m][:, 0:N])

    def dtap(key, ap_in, row0=0, rows=None):
        if key in dbg:
            o = dbg[key].ap()
            nc.sync.dma_start(out=o[row0:row0 + (rows or o.shape[0]), :],
                              in_=ap_in)

    pw = ctx.enter_context(tc.tile_pool(name="wts", bufs=1))
    pa = ctx.enter_context(tc.tile_pool(name="acts", bufs=1))
    pt = ctx.enter_context(tc.tile_pool(name="tmp", bufs=2))
    pt1 = ctx.enter_context(tc.tile_pool(name="rows", bufs=1))
    psh = ctx.enter_context(tc.tile_pool(name="sh1k", bufs=1))
    pstr = ctx.enter_context(tc.tile_pool(name="stream", bufs=1))
    pdram = ctx.enter_context(tc.tile_pool(name="dram", bufs=1, space="DRAM"))
    pp = ctx.enter_context(tc.tile_pool(name="ps_mm", bufs=3, space="PSUM"))
    pph = ctx.enter_context(tc.tile_pool(name="ps_hold", bufs=2, space="PSUM"))
    pps = ctx.enter_context(tc.tile_pool(name="ps_st", bufs=1, space="PSUM"))

    dma = nc.sync.dma_start
    GRP = [list(range(NCORES))]

    def ppt(shape, name, dt=F32):
        return pp.tile(shape, dt, name=name, tag="mm")

    def ppt_h(shape, name, dt=F32):
        return pph.tile(shape, dt, name=name, tag="hold")

    def ppt_s(shape, name, dt=F32):
        return pps.tile(shape, dt, name=name, tag="stat")

    def sh1k(name, tagid, dt=F32R, shape=None):
        return psh.tile(shape or [128, 256], dt, name=name, tag=f"sh{tagid}")

    # ---------- constants ----------
    ident_b = pa.tile([128, 128], BF16, name="ident_b")
    make_identity(nc, ident_b)
    ident_f = pt1.tile([128, 128], F32, name="ident_f", tag="ident_f")
    make_identity(nc, ident_f)

    ones_r = pa.tile([128, 1], F32R, name="ones_r")
    nc.vector.memset(ones_r[:].bitcast(F32), 1.0)
    ones_h = pa.tile([128, 1], F16, name="ones_h")
    nc.vector.memset(ones_h, 1.0)
    ones1_r = pa.tile([1, 128], F32R, name="ones1_r")
    nc.vector.memset(ones1_r[:].bitcast(F32), 1.0)

    ones8_r = pa.tile([8, 1], F32R, name="ones8_r")
    nc.vector.memset(ones8_r[:].bitcast(F32), 1.0)
    epsc = pa.tile([128, 1], F32, name="epsc")
    nc.vector.memset(epsc, EPS)

    # ---------- inputs -> SBUF ----------
    at_f, at_b = [], []
    off = 0
    for kt, ksz in enumerate(S_KT):
        tf = psh.tile([128, N], F32, name=f"at_f{kt}", tag=f"sh{9 + kt}")
        tb = pa.tile([128, N], BF16, name=f"at_b{kt}")
        dma(out=tf[0:ksz, :], in_=d["attrT"].ap()[off:off + ksz, :])
        dma(out=tb[0:ksz, :], in_=d["attrTb"].ap()[off:off + ksz, :])
        at_f.append(tf)
        at_b.append(tb)
        off += ksz
    centT_t = []
    off = 0
    for kt, ksz in enumerate(S_KT):
        t = pa.tile([128, KEXP], F32, name=f"centT{kt}")
        dma(out=t[0:ksz, :], in_=d["centT"].ap()[off:off + ksz, :])
        centT_t.append(t)
        off += ksz
    expbt = []
    for m in range(NT):
        t = pw.tile([128, KEXP], F32, name=f"expbt{m}")
        dma(out=t, in_=d["expBT"].ap()[m * 128:(m + 1) * 128, :])
        expbt.append(t)

    def load16(name, key, ncols, dt=F32R, tagbase=None):
        out = []
        for kt in range(NT):
            tg = f"{tagbase}{kt}" if tagbase else ""
            t = pw.tile([128, ncols], dt, name=f"{name}{kt}", tag=tg)
            dma(out=t, in_=d[key].ap()[kt * 128:(kt + 1) * 128, :])
            out.append(t)
        return out

    w1s_t = load16("w1s", "w1s", DSH)
    vnWs_t = load16("vnWs", "vnWs", DSH)
    snWs_t = load16("snWs", "snWs", DSH, tagbase="vnWs")
    fusWs_t = load16("fusWs", "fusWs", DSH, tagbase="w1s")
    # imgT packed: 2 tiles [128, 8*256] bf16; lhsT slice kt -> [:, (kt%8)*256+...]
    imgT_p = []
    for hh in range(2):
        t = pw.tile([128, 8 * BSH], BF16, name=f"imgTp{hh}")
        src_ap = d["imgT"].ap()[hh * 1024:(hh + 1) * 1024, :].rearrange(
            "(kt p) b -> p kt b", p=128)
        nc.sync.dma_start(out=t[:].rearrange("p (kt b) -> p kt b", kt=8),
                          in_=src_ap)
        imgT_p.append(t)

    def imgT_sl(kt, bt):
        return imgT_p[kt // 8][:, (kt % 8) * BSH + bt * 128:
                               (kt % 8) * BSH + (bt + 1) * 128]

    # veW1 packed single [128, 2048] bf16
    veW1_a = pw.tile([128, D], BF16, name="veW1_a")
    nc.sync.dma_start(out=veW1_a[:].rearrange("p (kt h) -> p kt h", kt=NT),
                      in_=d["veW1"].ap().rearrange("(kt p) h -> p kt h", p=128))

    def veW1_sl(kt):
        return veW1_a[:, kt * H:(kt + 1) * H]

    def loadmat(name, key, rows, cols, dt=F32):
        t = pw.tile([rows, cols], dt, name=name)
        dma(out=t, in_=d[key].ap()[:, :])
        return t

    bnG_t = loadmat("bnG_t", "bnG", 128, 2)
    bnB_t = loadmat("bnB_t", "bnB", 128, 2)
    vnbs_t = loadmat("vnbs_t", "vnbs", 128, 2)
    snbs_t = loadmat("snbs_t", "snbs", 128, 2)
    veb1_t = loadmat("veb1_t", "veb1", H, 1)
    veb2_t = loadmat("veb2_t", "veb2", ISH, 1)
    b2o8_t = loadmat("b2o8_t", "b2o8", 128, NT)
    veW2_t = loadmat("veW2_t", "veW2", H, 1, BF16)
    fusUs_t = loadmat("fusUs_t", "fusUs", 128, 2, F32R)
    selv_t = []
    for jb, (j0, jw) in enumerate(N_MT):
        t = pw.tile([128, ISH], BF16, name=f"selv{jb}")
        dma(out=t[0:jw, :], in_=d["selv"].ap()[j0:j0 + jw, :])
        selv_t.append(t)

    # collective bounce buffers (fp16 payloads: 10-bit mantissa is enough —
    # verified against reference; halves on-wire bytes)
    ar_h2_in = pdram.tile([D, N], F16, name="ar_h2_in")
    ar_h2_out = pdram.tile([D, N], F16, addr_space="Shared", name="ar_h2_out")
    ag_zv_in = pdram.tile([DSH, N], F16, name="ag_zv_in")
    ag_zv_out = pdram.tile([D, N], F16, addr_space="Shared", name="ag_zv_out")
    ag_zs_in = pdram.tile([DSH, N], F16, name="ag_zs_in")
    ag_zs_out = pdram.tile([D, N], F16, addr_space="Shared", name="ag_zs_out")
    ag_ve_in = pdram.tile([ISH, N], F32R, name="ag_ve_in")
    ag_ve_out = pdram.tile([N, N], F32R, addr_space="Shared", name="ag_ve_out")
    ag_vu_in = pdram.tile([1, 512], F32R, name="ag_vu_in")
    ag_vu_out = pdram.tile([NCORES, 512], F32R, addr_space="Shared",
                           name="ag_vu_out")

    # =================================================================
    # P1: CooperationModule -> SP (f32r padded, d-major)
    # =================================================================
    offT = []
    for k in range(KEXP):
        row = []
        for kt, ksz in enumerate(S_KT):
            t = sh1k(f"offT{k}_{kt}", 3 * k + kt)
            nc.vector.memset(t[:, N:256].bitcast(F32), 0.0)
            nc.vector.tensor_scalar(
                out=t[0:ksz, 0:N], in0=at_f[kt][0:ksz, :],
                scalar1=centT_t[kt][0:ksz, k:k + 1], scalar2=None,
                op0=OP.subtract)
            row.append(t)
        offT.append(row)

    SP = [pa.tile([128, 256], F32R, name=f"SP{m}") for m in range(NT)]
    for m in range(NT):
        nc.vector.memset(SP[m][:, N:256].bitcast(F32), 0.0)
    for k in range(KEXP):
        for half in range(4):
            wst = []
            for kt, ksz in enumerate(S_KT):
                w = pstr.tile([128, D // 4], F32R, name=f"expw_st{kt}",
                              tag=f"str{kt}")
                dma(out=w[0:ksz, :],
                    in_=d["expW"].ap()[k, sum(S_KT[:kt]):sum(S_KT[:kt]) + ksz,
                                       half * 512:(half + 1) * 512])
                wst.append(w)
            for mh in range(4):
                m = half * 4 + mh
                ps = ppt([128, 256], "ps_exp")
                for kt, ksz in enumerate(S_KT):
                    nc.tensor.matmul(ps, wst[kt][0:ksz, mh * 128:(mh + 1) * 128],
                                     offT[k][kt][0:ksz, 0:256],
                                     start=(kt == 0),
                                     stop=(kt == len(S_KT) - 1))
                if k == 0:
                    nc.vector.tensor_scalar(out=SP[m][:, 0:N], in0=ps[:, 0:N],
                                            scalar1=expbt[m][:, 0:1],
                                            scalar2=0.0,
                                            op0=OP.add, op1=OP.max)
                else:
                    r1 = pt1.tile([128, N], F32, name="exr", tag="exr")
                    nc.scalar.activation(r1, ps[:, 0:N], AF.Relu,
                                         bias=expbt[m][:, k:k + 1], scale=1.0)
                    nc.vector.tensor_tensor(SP[m][:, 0:N], SP[m][:, 0:N], r1,
                                            OP.add)
    dtap16("SP", SP)

    # =================================================================
    # P2: semantic2visual -> AllReduce(h2)
    # =================================================================
    a1 = []
    for m2 in range(2):
        ps = ppt([128, 256], "ps_h1")
        for kt in range(NT):
            nc.tensor.matmul(ps, w1s_t[kt][:, m2 * 128:(m2 + 1) * 128],
                             SP[kt][:, 0:256], start=(kt == 0),
                             stop=(kt == NT - 1))
        st6 = pt.tile([128, 6], F32, name="bn_st", tag="bn_st")
        mv = pt.tile([128, 2], F32, name="bn_mv", tag="bn_mv")
        nc.vector.bn_stats(out=st6, in_=ps[:, 0:N])
        nc.vector.bn_aggr(out=mv, in_=st6)
        sd = pt.tile([128, 1], F32, name="bn_sd", tag="bn_sd")
        nc.scalar.activation(sd, mv[:, 1:2], AF.Sqrt, bias=epsc[:, 0:1],
                             scale=1.0)
        rs = pt.tile([128, 1], F32, name="bn_rs", tag="bn_rs")
        nc.vector.reciprocal(out=rs, in_=sd)
        Av = pt.tile([128, 1], F32, name="bn_A", tag="bn_A")
        nc.vector.tensor_tensor(Av, rs, bnG_t[:, m2:m2 + 1], OP.mult)
        Bt = pt.tile([128, 1], F32, name="bn_Bt", tag="bn_Bt")
        nc.vector.tensor_tensor(Bt, mv[:, 0:1], Av, OP.mult)
        Bv = pt.tile([128, 1], F32, name="bn_Bv", tag="bn_Bv")
        nc.vector.tensor_tensor(Bv, bnB_t[:, m2:m2 + 1], Bt, OP.subtract)
        t1 = pt.tile([128, N], F32, name="h1_t1", tag="zt1")
        nc.vector.tensor_scalar(out=t1, in0=ps[:, 0:N], scalar1=Av,
                                scalar2=Bv, op0=OP.mult, op1=OP.add)
        a1m = pa.tile([128, 256], F32R, name=f"a1_{m2}")
        nc.vector.memset(a1m[:, N:256].bitcast(F32), 0.0)
        nc.vector.scalar_tensor_tensor(out=a1m[:, 0:N], in0=t1, scalar=0.2,
                                       in1=t1, op0=OP.mult, op1=OP.max)
        dtap("a1", a1m[:, 0:N], row0=m2 * 128, rows=128)
        a1.append(a1m)

    for half in range(4):
        w2st = []
        for kt in range(2):
            w = pstr.tile([128, D // 4], F32R, name=f"w2_st{kt}",
                          tag=f"str{kt}")
            dma(out=w, in_=d["w2s"].ap()[kt * 128:(kt + 1) * 128,
                                         half * 512:(half + 1) * 512])
            w2st.append(w)
        for mh in range(4):
            m = half * 4 + mh
            ps = ppt([128, 256], "ps_h2")
            for kt2 in range(2):
                nc.tensor.matmul(ps, w2st[kt2][:, mh * 128:(mh + 1) * 128],
                                 a1[kt2][:, 0:256], start=(kt2 == 0),
                                 stop=(kt2 == 1))
            hp = pt.tile([128, N], F16, name="h2p", tag="zouts")
            nc.vector.tensor_scalar(out=hp, in0=ps[:, 0:N],
                                    scalar1=b2o8_t[:, m:m + 1],
                                    scalar2=None, op0=OP.add)
            dma(out=ar_h2_in[m * 128:(m + 1) * 128, :], in_=hp)
    nc.gpsimd.collective_compute("AllReduce", OP.add, replica_groups=GRP,
                                 ins=[ar_h2_in[:].opt()],
                                 outs=[ar_h2_out[:].opt()])

    # ---- znorm: load z (f16 payload), inst-norm, fin(m, z, t1) writes ----
    def znorm(src_dram, fin, zn, tagset):
        zh = []
        for m in range(NT):
            t = psh.tile([128, 256], F16, name=f"zh{zn}_{m}", tag=f"sh{m}")
            dma(out=t[:, 0:N], in_=src_dram[m * 128:(m + 1) * 128, :])
            nc.vector.memset(t[:, N:256].bitcast(F32), 0.0)
            zh.append(t)
        pstat = ppt_s([1, 512], f"st_{zn}")
        for m in range(NT):
            nc.tensor.matmul(pstat[0:1, 0:256], ones_h, zh[m][:, 0:256],
                             start=(m == 0), stop=(m == NT - 1))
        mu = pt1.tile([1, 256], F32R, name="zmu", tag="zmu")
        nc.vector.tensor_scalar_mul(mu, pstat[0:1, 0:256], 1.0 / D)
        pmu = ppt_h([128, 256], f"mub_{zn}")
        nc.tensor.matmul(pmu, ones1_r, mu, start=True, stop=True)
        z = []
        for m in range(NT):
            # centered z in f32r (f16->f32r conversion fused into subtract)
            t = pa.tile([128, 256], F32R, name=f"z{zn}_{m}",
                        tag=f"{tagset}{m}")
            nc.vector.memset(t[:, N:256].bitcast(F32), 0.0)
            nc.vector.tensor_tensor(t[:, 0:N], zh[m][:, 0:N], pmu[:, 0:N],
                                    OP.subtract)
            z.append(t)
            zq = pt.tile([128, 256], F32R, name="zq", tag="zq")
            nc.vector.tensor_tensor(zq, t, t, OP.mult)
            nc.tensor.matmul(pstat[0:1, 256:512], ones_r, zq[:, 0:256],
                             start=(m == 0), stop=(m == NT - 1))
        va = pt1.tile([1, 256], F32, name="zva", tag="zva")
        nc.vector.tensor_scalar(out=va, in0=pstat[0:1, 256:512],
                                scalar1=1.0 / D, scalar2=EPS,
                                op0=OP.mult, op1=OP.add)
        ta = pt1.tile([1, 256], F32, name="zta", tag="zmu2")
        nc.scalar.activation(ta, va, AF.Sqrt)
        rsf = pt1.tile([1, 256], F32, name="zrsf", tag="znm")
        nc.vector.reciprocal(out=rsf, in_=ta)
        rs = pt1.tile([1, 256], F32R, name="zrs", tag="zrs")
        nc.vector.tensor_copy(out=rs, in_=rsf)
        prr = ppt_h([128, 256], f"rb_{zn}")
        nc.tensor.matmul(prr, ones1_r, rs, start=True, stop=True)
        for m in range(NT):
            t1 = pt.tile([128, N], F32, name="zt1", tag="zt1")
            nc.vector.tensor_tensor(t1, z[m][:, 0:N], prr[:, 0:N], OP.mult)
            fin(m, z, t1)
        return z

    # h2 -> PVP (leaky), in-place in z set "za"
    def fin_pvp(m, z, t1):
        nc.vector.scalar_tensor_tensor(out=z[m][:, 0:N], in0=t1, scalar=0.2,
                                       in1=t1, op0=OP.mult, op1=OP.max)
    if "h2" in dbg:
        nc.sync.dma_start(out=dbg["h2"].ap()[:, :], in_=ar_h2_out[:, :])
    PVP = znorm(ar_h2_out, fin_pvp, "h2", "za")
    dtap16("PVP", PVP)

    # PVP_n (f32r) on the shared n-major chain
    def transpose_nmajor(src_tiles, name, tagbase, dt):
        out = [pa.tile([128, D], dt, name=f"{name}0", tag=f"{tagbase}0"),
               pa.tile([128, D], dt, name=f"{name}1", tag=f"{tagbase}1")]
        for m in range(NT):
            for jb, (j0, jw) in enumerate(N_MT):
                if dt == F32R:
                    ps = ppt_h([128, 128], "ps_tr", F32)
                    nc.tensor.transpose(
                        ps[0:jw, 0:128],
                        src_tiles[m][:, j0:j0 + jw].bitcast(F32), ident_f)
                else:
                    ps = ppt_h([128, 128], "ps_tr", dt)
                    nc.tensor.transpose(ps[0:jw, 0:128],
                                        src_tiles[m][:, j0:j0 + jw], ident_b)
                nc.vector.tensor_copy(
                    out=out[jb][0:jw, m * 128:(m + 1) * 128],
                    in_=ps[0:jw, 0:128])
        return out

    PVP_n = transpose_nmajor(PVP, "PVP_n", "nmj", F32R)

    # =================================================================
    # P3: cos edges (visual f32r, semantic bf16 gram -> f32r edges)
    # =================================================================
    def cos_edge(x_tiles, ksizes, en, rdt, ones_g, rhs_w):
        nkt = len(ksizes)
        pn = ppt_s([1, 256] if rhs_w == 256 else [1, N], f"nrm_{en}")
        for kt, ksz in enumerate(ksizes):
            xq = pt.tile([128, rhs_w], rdt, name="xq", tag="xq")
            nc.vector.tensor_tensor(xq[0:ksz, :], x_tiles[kt][0:ksz, 0:rhs_w],
                                    x_tiles[kt][0:ksz, 0:rhs_w], OP.mult)
            nc.tensor.matmul(pn, ones_g[0:ksz, :], xq[0:ksz, :],
                             start=(kt == 0), stop=(kt == nkt - 1))
        sd = pt.tile([1, N], F32, name="esd", tag="zmu")
        nc.scalar.activation(sd, pn[0:1, 0:N], AF.Sqrt)
        rn_f = pt1.tile([1, 256], F32, name="ern_f", tag="zva")
        nc.vector.memset(rn_f[0:1, N:256], 0.0)
        nc.vector.reciprocal(out=rn_f[0:1, 0:N], in_=sd)
        rn = pt1.tile([1, 256], F32R, name="ern", tag="zAB")
        nc.vector.tensor_copy(out=rn, in_=rn_f)
        prn = ppt_h([128, 256], f"rnb_{en}")
        nc.tensor.matmul(prn, ones1_r, rn, start=True, stop=True)
        rcol = pt.tile([128, 2], F32, name="rc", tag=f"rc_{en}")
        for mt, (i0, iw) in enumerate(N_MT):
            pst = ppt_h([128, 128], "ps_tr3")
            nc.tensor.transpose(pst[0:iw, 0:1], rn_f[0:1, i0:i0 + iw],
                                ident_f[0:1, 0:1])
            nc.vector.tensor_copy(out=rcol[0:iw, mt:mt + 1],
                                  in_=pst[0:iw, 0:1])
        edge = []
        for mt, (i0, iw) in enumerate(N_MT):
            ps = ppt([128, rhs_w], f"ps_{en}")
            for kt, ksz in enumerate(ksizes):
                nc.tensor.matmul(ps[0:iw, :], x_tiles[kt][0:ksz, i0:i0 + iw],
                                 x_tiles[kt][0:ksz, 0:rhs_w],
                                 start=(kt == 0), stop=(kt == nkt - 1))
            s1 = pt.tile([128, N], F32, name="es1", tag="zt1")
            nc.vector.tensor_scalar(out=s1[0:iw, :], in0=ps[0:iw, 0:N],
                                    scalar1=rcol[0:iw, mt:mt + 1],
                                    scalar2=None, op0=OP.mult)
            nc.vector.tensor_tensor(s1[0:iw, :], s1[0:iw, :], prn[0:iw, 0:N],
                                    OP.mult)
            rmx = pt.tile([128, 1], F32, name="ermx", tag="ermx")
            nc.vector.reduce_max(rmx[0:iw, :], s1[0:iw, :], axis=AX.X)
            bia = pt.tile([128, 1], F32, name="ebia", tag="ebia")
            nc.vector.tensor_scalar_mul(bia[0:iw, :], rmx[0:iw, :], -100.0)
            nc.scalar.activation(s1[0:iw, :], s1[0:iw, :], AF.Exp,
                                 bias=bia[0:iw, 0:1], scale=100.0)
            sm = pt.tile([128, 1], F32, name="esm", tag="esm")
            nc.vector.reduce_sum(sm[0:iw, :], s1[0:iw, :], axis=AX.X)
            rr = pt.tile([128, 1], F32, name="err", tag="err")
            nc.vector.reciprocal(out=rr[0:iw, :], in_=sm[0:iw, :])
            ed = pa.tile([128, N], F32R, name=f"{en}_{mt}",
                         tag=f"edg_{en}_{mt}")
            nc.vector.tensor_scalar(out=ed[0:iw, :], in0=s1[0:iw, :],
                                    scalar1=rr[0:iw, 0:1], scalar2=None,
                                    op0=OP.mult)
            edge.append(ed)
        edgeT = [pa.tile([128, 256], F32R, name=f"{en}T0", tag=f"{en}T0"),
                 pa.tile([128, 256], F32R, name=f"{en}T1", tag=f"{en}T1")]
        for jb in range(2):
            nc.vector.memset(edgeT[jb][:].bitcast(F32), 0.0)
        for mt, (i0, iw) in enumerate(N_MT):
            for jb, (j0, jw) in enumerate(N_MT):
                pst = ppt_h([128, 128], "ps_tr4", F32)
                nc.tensor.transpose(pst[0:jw, 0:iw],
                                    edge[mt][0:iw, j0:j0 + jw].bitcast(F32),
                                    ident_f[0:iw, 0:iw])
                nc.vector.tensor_copy(out=edgeT[jb][0:jw, i0:i0 + iw],
                                      in_=pst[0:jw, 0:iw])
        return edge, edgeT

    ones_b = pa.tile([128, 1], BF16, name="ones_b")
    nc.vector.memset(ones_b, 1.0)
    vedge, vedgeT = cos_edge(PVP, [128] * NT, "ve", F32R, ones_r, 256)
    sedge, sedgeT = cos_edge(at_b, S_KT, "se", BF16, ones_b, N)
    if "vedge" in dbg:
        for mt, (i0, iw) in enumerate(N_MT):
            dtap("vedge", vedge[mt][0:iw, :], row0=i0, rows=iw)
            dtap("sedge", sedge[mt][0:iw, :], row0=i0, rows=iw)

    # =================================================================
    # P4: UpdateVisualNode -> AllGather(z_v) -> VP2 (in-place set "zb")
    # =================================================================
    vp_f, y_r = [], []
    for m in range(NT):
        pv = pp.tile([128, 256], F32, name="ps_vp", tag="mm")
        pe = pp.tile([128, 256], F32, name="ps_ev", tag="mm")
        for jb, (j0, jw) in enumerate(N_MT):
            nc.tensor.matmul(pv, PVP_n[jb][0:jw, m * 128:(m + 1) * 128],
                             vedgeT[jb][0:jw, :], start=(jb == 0),
                             stop=(jb == 1))
            nc.tensor.matmul(pe, PVP_n[jb][0:jw, m * 128:(m + 1) * 128],
                             sedgeT[jb][0:jw, :], start=(jb == 0),
                             stop=(jb == 1))
        vf = pa.tile([128, N], F32, name=f"vp_f{m}")
        nc.scalar.copy(out=vf, in_=pv[:, 0:N])
        vp_f.append(vf)
        yb = sh1k(f"y_{m}", m)
        nc.vector.memset(yb[:, N:256].bitcast(F32), 0.0)
        nc.vector.tensor_tensor(yb[:, 0:N], vf, pe[:, 0:N], OP.add)
        y_r.append(yb)
    dtap16("ybf", y_r)

    for m2 in range(2):
        ps = ppt([128, 256], "ps_zv")
        for kt in range(NT):
            nc.tensor.matmul(ps, vnWs_t[kt][:, m2 * 128:(m2 + 1) * 128],
                             y_r[kt][:, 0:256], start=(kt == 0),
                             stop=(kt == NT - 1))
        zc = pt.tile([128, N], F16, name="zvc", tag="zouts")
        nc.vector.tensor_scalar(out=zc, in0=ps[:, 0:N],
                                scalar1=vnbs_t[:, m2:m2 + 1],
                                scalar2=None, op0=OP.add)
        dma(out=ag_zv_in[m2 * 128:(m2 + 1) * 128, :], in_=zc)
    nc.gpsimd.collective_compute("AllGather", OP.bypass, replica_groups=GRP,
                                 ins=[ag_zv_in[:].opt()],
                                 outs=[ag_zv_out[:].opt()])
    if "zv" in dbg:
        nc.sync.dma_start(out=dbg["zv"].ap()[:, :], in_=ag_zv_out[:, :])

    VP2_bf = [pa.tile([128, N], BF16, name=f"VP2b{m}", tag=f"VP2b{m}")
              for m in range(NT)]

    def fin_vp2(m, z, t1):
        nc.vector.scalar_tensor_tensor(out=z[m][:, 0:N], in0=t1, scalar=0.0,
                                       in1=vp_f[m], op0=OP.max, op1=OP.add)
        nc.vector.tensor_copy(out=VP2_bf[m], in_=z[m][:, 0:N])
    VP2 = znorm(ag_zv_out, fin_vp2, "zv", "zb")
    dtap16("VP2", VP2_bf)

    # =================================================================
    # P6: UpdateVisualEdge (i-sharded, batched row stats, bf16)
    #   h1e(i,:) per hidden h: A_j + A_i - 2*C_ij with C via W1-tile scaling
    #   instnorm-over-H stats land as psum ROWS (one-hot partition matmuls)
    # =================================================================
    VP2_n = transpose_nmajor(VP2_bf, "VP2_n", "nmj", BF16)
    negx2my = [None] * NT
    for hh in range(2):
        xmyn = pt1.tile([ISH, D // 2], BF16, name=f"xmyn{hh}", tag="xmyn")
        for ch in range(4):
            ps = ppt([ISH, 256], "ps_xmy")
            for jb, (j0, jw) in enumerate(N_MT):
                nc.tensor.matmul(
                    ps, selv_t[jb][0:jw, :],
                    VP2_n[jb][0:jw, hh * 1024 + ch * 256:
                               hh * 1024 + (ch + 1) * 256],
                    start=(jb == 0), stop=(jb == 1))
            nc.vector.tensor_copy(out=xmyn[:, ch * 256:(ch + 1) * 256],
                                  in_=ps)
        if "xmyn" in dbg:
            nc.sync.dma_start(out=dbg["xmyn"].ap()[:, hh * 1024:
                                                   (hh + 1) * 1024],
                              in_=xmyn[:, :])
        for ktl in range(8):
            kt = hh * 8 + ktl
            pst = ppt_h([128, 128], "ps_tr5", BF16)
            nc.tensor.transpose(pst[0:128, 0:ISH],
                                xmyn[:, ktl * 128:(ktl + 1) * 128],
                                ident_b[0:ISH, 0:ISH])
            ng = pa.tile([128, ISH], F32, name=f"negx2my{kt}")
            nc.vector.tensor_scalar_mul(ng, pst[0:128, 0:ISH], -2.0)
            negx2my[kt] = ng
    pA = ppt([128, N], "ps_A")
    pAm = ppt([128, ISH], "ps_Am")
    for m in range(NT):
        xq = pt.tile([128, N], BF16, name="vsq", tag="xq")
        nc.vector.tensor_tensor(xq, VP2_bf[m], VP2_bf[m], OP.mult)
        nc.tensor.matmul(pA, veW1_sl(m), xq[:, :], start=(m == 0),
                         stop=(m == NT - 1))
        xqm = pt.tile([128, ISH], BF16, name="vsqm", tag="vsqm")
        nc.vector.tensor_tensor(xqm, negx2my[m], negx2my[m], OP.mult)
        nc.tensor.matmul(pAm, veW1_sl(m), xqm[:, :], start=(m == 0),
                         stop=(m == NT - 1))
    # A_T_bf = A_j + b1 (bf16);  A_my = A_i (f32, (-2x)^2/4)
    A_T_bf = pa.tile([128, N], BF16, name="A_T_bf", tag="A_T")
    nc.vector.tensor_scalar(out=A_T_bf, in0=pA[:, 0:N],
                            scalar1=veb1_t[:, 0:1], scalar2=None, op0=OP.add)
    A_my = pa.tile([128, ISH], F32, name="A_my")
    nc.vector.tensor_scalar_mul(A_my, pAm, 0.25)
    if "AT" in dbg:
        atf = pt.tile([128, N], F32, name="atf", tag="zt1")
        nc.vector.tensor_copy(out=atf, in_=A_T_bf)
        dtap("AT", atf[0:128, 0:N])
        amf = pt.tile([128, ISH], F32, name="amf", tag="vsqm")
        nc.vector.tensor_scalar(out=amf, in0=A_my, scalar1=veb1_t[:, 0:1],
                                scalar2=None, op0=OP.add)
        dtap("Amy", amf[0:128, :])
    vedge_my = pa.tile([ISH, N], F32, name="vedge_my", tag="edg_se_0")
    psvm = ppt([ISH, N], "ps_vm")
    for mt, (i0, iw) in enumerate(N_MT):
        vb = pt.tile([128, N], BF16, name="vedgb", tag="xq")
        nc.vector.tensor_copy(out=vb[0:iw, :], in_=vedge[mt][0:iw, :])
        nc.tensor.matmul(psvm, selv_t[mt][0:iw, :], vb[0:iw, :],
                         start=(mt == 0), stop=(mt == 1))
    nc.vector.tensor_copy(out=vedge_my, in_=psvm)

    # lhsT row-placement patterns: slice [ISH-1-k : 2*ISH-1-k] has ones (or
    # veW2) in column k, zeros elsewhere -> matmul accumulates into psum row k
    PAT = pa.tile([128, 2 * ISH - 1], BF16, name="PAT")
    nc.vector.memset(PAT, 0.0)
    nc.vector.memset(PAT[:, ISH - 1:ISH], 1.0)
    W2PAT = pa.tile([128, 2 * ISH - 1], BF16, name="W2PAT")
    nc.vector.memset(W2PAT, 0.0)
    nc.vector.tensor_copy(out=W2PAT[:, ISH - 1:ISH], in_=veW2_t[:, 0:1])
    # mean-over-H broadcast as a constant matmul: mu_b = (1/H) ones @ hsb
    ONESM = pa.tile([128, 128], BF16, name="ONESM")
    nc.vector.memset(ONESM, 1.0 / H)

    cur_ps = pps.tile([ISH, 256], F32, name="cur_ps", tag="cur")
    S_ps = ppt_s([ISH, 512], "st_e")
    for ii in range(ISH):
        psC = ppt([128, 256], "ps_C")
        for kt in range(NT):
            w1i = pstr.tile([128, H], BF16, name=f"w1i{kt}",
                            tag=f"w1i{kt % 4}")
            sc = negx2my[kt][:, ii:ii + 1]
            if kt % 3 == 0:
                nc.vector.tensor_scalar(out=w1i, in0=veW1_sl(kt),
                                        scalar1=sc, scalar2=None,
                                        op0=OP.mult)
            elif kt % 3 == 1:
                nc.gpsimd.tensor_scalar(out=w1i, in0=veW1_sl(kt),
                                        scalar1=sc, scalar2=None,
                                        op0=OP.mult)
            else:
                nc.scalar.activation(w1i, veW1_sl(kt), AF.Copy, scale=sc)
            nc.tensor.matmul(psC[:, 0:N], w1i, VP2_bf[kt],
                             start=(kt == 0), stop=(kt == NT - 1))
        # hsb = (-2C + A_i) + (A_j + b1):  Act (psum in) then DVE bf16 2x
        hp_ = pt.tile([128, N], BF16, name="ehp", tag="zt1")
        nc.scalar.activation(hp_, psC[:, 0:N], AF.Identity,
                             bias=A_my[:, ii:ii + 1], scale=1.0)
        hsb = pt.tile([128, N], BF16, name="ehsb", tag=f"hsb{ii % 2}")
        nc.vector.tensor_tensor(hsb, hp_, A_T_bf, OP.add)
        hsq = pt.tile([128, N], BF16, name="ehsq", tag="xq")
        nc.vector.tensor_tensor(hsq, hsb, hsb, OP.mult)
        psel = PAT[:, ISH - 1 - ii:2 * ISH - 1 - ii]
        nc.tensor.matmul(S_ps[0:ISH, 0:N], psel, hsb,
                         start=(ii == 0), stop=(ii == ISH - 1))
        nc.tensor.matmul(S_ps[0:ISH, 256:256 + N], psel, hsq,
                         start=(ii == 0), stop=(ii == ISH - 1))
        # centered h, relu; rs scaling deferred to cur rows (rs>0)
        pm = ppt_h([128, 256], "mu_b")
        nc.tensor.matmul(pm[:, 0:N], ONESM, hsb, start=True, stop=True)
        t1b = pt.tile([128, N], BF16, name="et1b", tag="zt1")
        nc.vector.tensor_tensor(t1b, hsb, pm[:, 0:N], OP.subtract)
        h2b = pt.tile([128, N], BF16, name="eh2b", tag="h2b")
        nc.vector.tensor_scalar_max(h2b, t1b, 0.0)
        nc.tensor.matmul(cur_ps[0:ISH, 0:N],
                         W2PAT[:, ISH - 1 - ii:2 * ISH - 1 - ii], h2b,
                         start=(ii == 0), stop=(ii == ISH - 1))
    # ---- batched row stats -> rs rows; cur = cur_raw * rs ----
    mu = pt1.tile([ISH, N], F32, name="emu", tag="emu")
    nc.vector.tensor_scalar_mul(mu, S_ps[0:ISH, 0:N], 1.0 / H)
    va = pt1.tile([ISH, N], F32, name="eva", tag="eva")
    nc.vector.tensor_scalar_mul(va, S_ps[0:ISH, 256:256 + N], 1.0 / H)
    musq = pt1.tile([ISH, N], F32, name="emusq", tag="emusq")
    nc.vector.tensor_tensor(musq, mu, mu, OP.mult)
    nc.vector.tensor_tensor(va, va, musq, OP.subtract)
    sdv = pt1.tile([ISH, N], F32, name="esdv", tag="emusq")
    nc.scalar.activation(sdv, va, AF.Sqrt, bias=epsc[0:ISH, 0:1], scale=1.0)
    rsr = pt1.tile([ISH, N], F32, name="ersr", tag="eva")
    nc.vector.reciprocal(out=rsr, in_=sdv)
    curm = pa.tile([ISH, N], F32, name="curm", tag="atf2")
    nc.vector.tensor_tensor(curm, cur_ps[0:ISH, 0:N], rsr, OP.mult)
    dtap("cur", curm[:, :])

    # tanh(cur + b2) * (vedge_my + 1e-8) -> softmax(/10) -> my edge rows
    curt = pa.tile([ISH, N], F32, name="curt", tag="atf0")
    nc.scalar.activation(curt, curm, AF.Tanh,
                         bias=veb2_t[0:ISH, 0:1], scale=1.0)
    ne = pa.tile([ISH, N], F32, name="ne", tag="atf1")
    nc.vector.scalar_tensor_tensor(out=ne, in0=vedge_my, scalar=1e-8,
                                   in1=curt, op0=OP.add, op1=OP.mult)
    rmx = pt.tile([ISH, 1], F32, name="vermx", tag="vermx")
    nc.vector.reduce_max(rmx, ne, axis=AX.X)
    bia = pt.tile([ISH, 1], F32, name="vebia", tag="vebia")
    nc.vector.tensor_scalar_mul(bia, rmx, -0.1)
    ex = pt1.tile([ISH, N], F32, name="veex", tag="veex")
    nc.scalar.activation(ex, ne, AF.Exp, bias=bia[0:ISH, 0:1], scale=0.1)
    sm = pt.tile([ISH, 1], F32, name="vesm", tag="vesm")
    nc.vector.reduce_sum(sm, ex, axis=AX.X)
    rr = pt.tile([ISH, 1], F32, name="verr", tag="verr")
    nc.vector.reciprocal(out=rr, in_=sm)
    vemine = pt1.tile([ISH, N], F32R, name="vemine", tag="vemine")
    nc.vector.tensor_scalar(out=vemine, in0=ex, scalar1=rr[0:ISH, 0:1],
                            scalar2=None, op0=OP.mult)
    dma(out=ag_ve_in[:, :], in_=vemine)
    nc.gpsimd.collective_compute("AllGather", OP.bypass, replica_groups=GRP,
                                 ins=[ag_ve_in[:].opt()],
                                 outs=[ag_ve_out[:].opt()])
    if "ve2" in dbg:
        nc.sync.dma_start(out=dbg["ve2"].ap()[:, :], in_=ag_ve_out[:, :])
    ve2 = [pt1.tile([128, N], F32R, name="ve2_0", tag="ve2_0"),
           pt1.tile([128, N], F32R, name="ve2_1", tag="ve2_1")]
    for mt, (i0, iw) in enumerate(N_MT):
        dma(out=ve2[mt][0:iw, :], in_=ag_ve_out[i0:i0 + iw, :])
    ve2T = [pa.tile([128, 256], F32R, name="ve2T0", tag="veT0"),
            pa.tile([128, 256], F32R, name="ve2T1", tag="veT1")]
    for mt, (i0, iw) in enumerate(N_MT):
        for jb, (j0, jw) in enumerate(N_MT):
            pst = ppt_h([128, 128], "ps_tr6", F32)
            nc.tensor.transpose(pst[0:jw, 0:iw],
                                ve2[mt][0:iw, j0:j0 + jw].bitcast(F32),
                                ident_f[0:iw, 0:iw])
            nc.vector.tensor_copy(out=ve2T[jb][0:jw, i0:i0 + iw],
                                  in_=pst[0:jw, 0:iw])

    # =================================================================
    # P5: UpdateSemanticNode -> AllGather(z_s) -> SP2 (in-place set "za")
    # =================================================================
    SP_n = transpose_nmajor(SP, "SP_n", "nmj", F32R)
    sp_f, y2_r = [], []
    for m in range(NT):
        psp = pp.tile([128, 256], F32, name="ps_sp", tag="mm")
        pes = pp.tile([128, 256], F32, name="ps_es", tag="mm")
        for jb, (j0, jw) in enumerate(N_MT):
            nc.tensor.matmul(psp, SP_n[jb][0:jw, m * 128:(m + 1) * 128],
                             sedgeT[jb][0:jw, :], start=(jb == 0),
                             stop=(jb == 1))
            nc.tensor.matmul(pes, SP_n[jb][0:jw, m * 128:(m + 1) * 128],
                             ve2T[jb][0:jw, :], start=(jb == 0),
                             stop=(jb == 1))
        sf = pa.tile([128, N], F32, name=f"sp_f{m}", tag=f"vp_f{m}")
        nc.scalar.copy(out=sf, in_=psp[:, 0:N])
        sp_f.append(sf)
        yb = sh1k(f"y2_{m}", m)
        nc.vector.memset(yb[:, N:256].bitcast(F32), 0.0)
        nc.vector.tensor_tensor(yb[:, 0:N], sf, pes[:, 0:N], OP.add)
        y2_r.append(yb)

    for m2 in range(2):
        ps = ppt([128, 256], "ps_zs")
        for kt in range(NT):
            nc.tensor.matmul(ps, snWs_t[kt][:, m2 * 128:(m2 + 1) * 128],
                             y2_r[kt][:, 0:256], start=(kt == 0),
                             stop=(kt == NT - 1))
        zc = pt.tile([128, N], F16, name="zsc", tag="zouts")
        nc.vector.tensor_scalar(out=zc, in0=ps[:, 0:N],
                                scalar1=snbs_t[:, m2:m2 + 1],
                                scalar2=None, op0=OP.add)
        dma(out=ag_zs_in[m2 * 128:(m2 + 1) * 128, :], in_=zc)
    nc.gpsimd.collective_compute("AllGather", OP.bypass, replica_groups=GRP,
                                 ins=[ag_zs_in[:].opt()],
                                 outs=[ag_zs_out[:].opt()])

    def fin_sp2(m, z, t1):
        nc.vector.scalar_tensor_tensor(out=z[m][:, 0:N], in0=t1, scalar=0.0,
                                       in1=sp_f[m], op0=OP.max, op1=OP.add)
    SP2 = znorm(ag_zs_out, fin_sp2, "zs", "za")
    if "SP2" in dbg:
        for m in range(NT):
            nc.sync.dma_start(out=dbg["SP2"].ap()[m * 128:(m + 1) * 128, :],
                              in_=SP2[m][:, 0:N])

    # =================================================================
    # P7: FusionLayer (f32r) -> alpha -> prob
    # =================================================================
    pvu = [ppt_s([1, 256], "ps_vu0"), ppt_s([1, 256], "ps_vu1")]
    for k, srct in enumerate((VP2, SP2)):
        for m2 in range(2):
            ps = ppt([128, 256], "ps_fus")
            for kt in range(NT):
                nc.tensor.matmul(ps, fusWs_t[kt][:, m2 * 128:(m2 + 1) * 128],
                                 srct[kt][:, 0:256], start=(kt == 0),
                                 stop=(kt == NT - 1))
            th = pt.tile([128, 256], F32R, name="fth", tag="fth")
            nc.scalar.activation(th, ps, AF.Tanh)
            nc.tensor.matmul(pvu[k], fusUs_t[:, m2:m2 + 1], th[:, :],
                             start=(m2 == 0), stop=(m2 == 1))
    vu_sb = pt1.tile([1, 512], F32R, name="vu_sb", tag="vu_sb")
    nc.vector.memset(vu_sb[:].bitcast(F32), 0.0)
    nc.vector.tensor_copy(out=vu_sb[0:1, 0:N], in_=pvu[0][0:1, 0:N])
    nc.vector.tensor_copy(out=vu_sb[0:1, 256:256 + N], in_=pvu[1][0:1, 0:N])
    dma(out=ag_vu_in[:, :], in_=vu_sb)
    nc.gpsimd.collective_compute("AllGather", OP.bypass, replica_groups=GRP,
                                 ins=[ag_vu_in[:].opt()],
                                 outs=[ag_vu_out[:].opt()])
    vus = pt1.tile([NCORES, 512], F32R, name="vus", tag="vu_sb")
    dma(out=vus, in_=ag_vu_out[:, :])
    pvk = ppt_s([1, 512], "ps_vuk")
    for k in range(2):
        nc.tensor.matmul(pvk[0:1, 256 * k:256 * k + 256],
                         ones8_r, vus[:, 256 * k:256 * k + 256],
                         start=True, stop=True)
    vuf = pt1.tile([1, 512], F32, name="vuf", tag="zstt")
    nc.vector.tensor_copy(out=vuf, in_=pvk)
    dtap("vuf", vuf[:, :])
    mx = pt.tile([1, N], F32, name="amx", tag="amx")
    nc.vector.tensor_tensor(mx, vuf[0:1, 0:N], vuf[0:1, 256:256 + N], OP.max)
    dv = pt1.tile([1, 512], F32R, name="adv", tag="adv")
    nc.vector.memset(dv[:].bitcast(F32), 0.0)
    for k in range(2):
        nc.vector.tensor_tensor(dv[0:1, 256 * k:256 * k + N],
                                vuf[0:1, 256 * k:256 * k + N], mx, OP.subtract)
    nc.scalar.activation(dv, dv, AF.Exp, scale=100.0)
    ssum = pt.tile([1, N], F32, name="assum", tag="assum")
    nc.vector.tensor_tensor(ssum, dv[0:1, 0:N], dv[0:1, 256:256 + N], OP.add)
    rsu = pt.tile([1, N], F32, name="arsu", tag="arsu")
    nc.vector.reciprocal(out=rsu, in_=ssum)
    for k in range(2):
        nc.vector.tensor_tensor(dv[0:1, 256 * k:256 * k + N],
                                dv[0:1, 256 * k:256 * k + N], rsu, OP.mult)
    alro = dv
    if "alpha" in dbg:
        al_f = pt1.tile([1, 512], F32, name="al_f", tag="zstt")
        nc.vector.tensor_copy(out=al_f, in_=alro)
        dtap("alpha", al_f[:, :])
    pal = ppt_h([128, 512], "ab_al")
    nc.tensor.matmul(pal, ones1_r, alro, start=True, stop=True)
    proto_bf = []
    for m in range(NT):
        t1 = pt.tile([128, N], F32, name="pr1", tag="zouts")
        nc.vector.tensor_tensor(t1, VP2[m][:, 0:N], pal[:, 0:N], OP.mult)
        t2 = pt.tile([128, N], F32, name="pr2", tag="zt1")
        nc.vector.tensor_tensor(t2, SP2[m][:, 0:N], pal[:, 256:256 + N],
                                OP.mult)
        pb = pa.tile([128, N], BF16, name=f"proto{m}", tag=f"VP2b{m}")
        nc.vector.tensor_tensor(pb, t1, t2, OP.add)
        proto_bf.append(pb)
    for bt in range(2):
        ps = ppt([128, N], "ps_prob")
        for kt in range(NT):
            nc.tensor.matmul(ps, imgT_sl(kt, bt), proto_bf[kt][:, :],
                             start=(kt == 0), stop=(kt == NT - 1))
        t1 = pt.tile([128, N], F32, name="probf", tag="zouts")
        nc.vector.tensor_copy(out=t1, in_=ps)
        dma(out=prob_out.ap()[bt * 128:(bt + 1) * 128, :], in_=t1)


# =====================================================================
# Host side
# =====================================================================
def _prep_inputs(inputs):
    bf = ml_dtypes.bfloat16
    f32 = np.float32
    att = np.asarray(inputs["attribute"], f32)
    cen = np.asarray(inputs["centers"], f32)
    expW = np.asarray(inputs["expert_W"], f32)
    expB = np.asarray(inputs["expert_b"], f32)
    w1 = np.asarray(inputs["s2v_W1"], f32)
    w2 = np.asarray(inputs["s2v_W2"], f32)
    in_maps = []
    for c in range(NCORES):
        cs = slice(c * DSH, (c + 1) * DSH)
        isl = slice(c * ISH, (c + 1) * ISH)
        bs = slice(c * BSH, (c + 1) * BSH)
        selv = np.zeros((N, ISH), f32)
        selv[np.arange(c * ISH, (c + 1) * ISH), np.arange(ISH)] = 1.0
        m = {
            "attrT": np.ascontiguousarray(att.T),
            "attrTb": np.ascontiguousarray(att.T).astype(bf),
            "centT": np.ascontiguousarray(cen.T),
            "expW": expW,
            "expBT": np.ascontiguousarray(expB.T),
            "w1s": np.ascontiguousarray(w1[:, cs]),
            "bnG": np.ascontiguousarray(np.asarray(inputs["bn_g"], f32)[cs].reshape(2, 128).T),
            "bnB": np.ascontiguousarray(np.asarray(inputs["bn_b"], f32)[cs].reshape(2, 128).T),
            "w2s": np.ascontiguousarray(w2[cs, :]),
            "b2o8": np.ascontiguousarray((np.asarray(inputs["s2v_b2"], f32) / NCORES).reshape(NT, 128).T),
            "vnWs": np.ascontiguousarray(np.asarray(inputs["vn_W"], f32)[:, cs]),
            "vnbs": np.ascontiguousarray(np.asarray(inputs["vn_b"], f32)[cs].reshape(2, 128).T),
            "snWs": np.ascontiguousarray(np.asarray(inputs["sn_W"], f32)[:, cs]),
            "snbs": np.ascontiguousarray(np.asarray(inputs["sn_b"], f32)[cs].reshape(2, 128).T),
            "veW1": np.asarray(inputs["ve_W1"], f32).astype(bf),
            "veb1": np.asarray(inputs["ve_b1"], f32)[:, None],
            "veW2": np.asarray(inputs["ve_W2"], f32).astype(bf),
            "veb2": np.full((ISH, 1), float(np.asarray(inputs["ve_b2"])[0]),
                            f32),
            "fusWs": np.ascontiguousarray(np.asarray(inputs["fus_W"], f32)[:, cs]),
            "fusUs": np.ascontiguousarray(np.asarray(inputs["fus_u"], f32)[cs, 0].reshape(2, 128).T),
            "imgT": np.ascontiguousarray(
                np.asarray(inputs["img_feat"], f32)[bs, :].T).astype(bf),
            "selv": selv.astype(bf),
        }
        in_maps.append(m)
    return in_maps


def kernel(**inputs):
    global _BUILT
    if _BUILT is None:
        _BUILT = build()
    nc = _BUILT
    in_maps = _prep_inputs(inputs)
    res = run_bass_kernel_spmd(nc, in_maps, core_ids=list(range(NCORES)))
    out = np.concatenate([res.results[c]["prob"] for c in range(NCORES)],
                         axis=0)
    return out.astype(np.float32)


def kernel_debug(**inputs):
    nc = build(debug=True)
    in_maps = _prep_inputs(inputs)
    res = run_bass_kernel_spmd(nc, in_maps, core_ids=list(range(NCORES)))
    out = np.concatenate([res.results[c]["prob"] for c in range(NCORES)],
                         axis=0)
    return out.astype(np.float32), res.results


if __name__ == "__main__":
    import reference
    inp = {k: np.asarray(v) for k, v in reference.setup_inputs().items()}
    got = kernel(**inp)
    exp = np.asarray(reference.reference(**reference.setup_inputs()))
    err = np.abs(got - exp).max() / (np.abs(exp).max() + 1e-9)
    print("Relative error:", err)



# revision 40
# speedup vs baseline: 1.3102x; 1.0272x over previous
# Trainium2 Bass kernel for nn_CPPN (gnn_message_passing), 8-core SPMD.
#
# Sharding:
#   - Node-MLP weights (s2v_W1/W2, vn_W, sn_W, fus_W) sharded over the 2048
#     col/row dim (256 per core); stitched with one AllReduce (h2) and three
#     AllGathers (z_v, z_s, vu partials).
#   - Visual edge MLP (200x200 pairwise rows) sharded over i: 25 rows/core,
#     AllGather of the resulting edge rows.  Per-core column selection is via
#     a host-supplied one-hot matrix (SPMD program is identical on all cores;
#     only input data differs).
#   - img_feat batch sharded 256 rows/core for the final prob matmul; host
#     concatenates per-core outputs.
# Layout: activations are d-major [D on partitions (16x128), 200 on free].
# Heavy matmuls bf16; stats/broadcast matmuls f32r.  The reference's second
# edge_update (semantic) is dead code w.r.t. prob and is skipped.

import sys

sys.path.insert(0, "/opt/trn_rl_repo")

import numpy as np
import ml_dtypes

import concourse.bass as bass
import concourse.bacc as bacc
import concourse.tile as tile
from concourse import mybir
from concourse.bass_utils import run_bass_kernel_spmd
from concourse.masks import make_identity

F32 = mybir.dt.float32
F32R = mybir.dt.float32r
BF16 = mybir.dt.bfloat16
F16 = mybir.dt.float16
AF = mybir.ActivationFunctionType
OP = mybir.AluOpType
AX = mybir.AxisListType

NCORES = 8
N = 200
S = 312
D = 2048
H = 128
KEXP = 3
B = 2048
DSH = D // NCORES      # 256
ISH = N // NCORES      # 25
BSH = B // NCORES      # 256
EPS = 1e-5
NT = D // 128          # 16
S_KT = [128, 128, 56]
N_MT = ((0, 128), (128, 72))
G_I = 2                # i-group size for edge t-gen batching

_BUILT = None


def _rep(ap_src, dims):
    """Rebuild AP with explicit free dims [[step,count],...] after partition."""
    return bass.AP(tensor=ap_src.tensor, offset=ap_src.offset,
                   ap=[ap_src.ap[0]] + dims)


def build(debug=False):
    nc = bacc.Bacc("TRN2", target_bir_lowering=False, debug=False,
                   num_devices=NCORES)
    d = {}

    def din(name, shape, dt):
        d[name] = nc.dram_tensor(name, shape, dt, kind="ExternalInput")

    din("attrT", [S, N], F16)
    din("attrTb", [S, N], BF16)
    din("centT", [S, KEXP], F32)
    din("expW", [KEXP, S, D], F16)
    din("expBT", [D, KEXP], F32)
    din("w1s", [D, DSH], F16)
    din("bnG", [128, 2], F32)
    din("bnB", [128, 2], F32)
    din("w2s", [DSH, D], F16)
    din("b2o8", [128, NT], F32)
    din("vnWs", [D, DSH], F16)
    din("vnbs", [128, 2], F32)
    din("snWs", [D, DSH], F16)
    din("snbs", [128, 2], F32)
    din("veW1", [D, H], BF16)
    din("veb1", [H, 1], F32)
    din("veW2", [H, 1], BF16)
    din("veb2", [ISH, 1], F32)
    din("fusWs", [D, DSH], F16)
    din("fusUs", [128, 2], F16)
    din("imgT", [D, BSH], F16)
    din("selv", [N, ISH], BF16)
    prob_out = nc.dram_tensor("prob", [BSH, N], F32, kind="ExternalOutput")
    dbg = {}
    if debug:
        def dout(name, shape, dt):
            dbg[name] = nc.dram_tensor("dbg_" + name, shape, dt,
                                       kind="ExternalOutput")
        dout("SP", [D, N], F16)
        dout("a1", [2 * 128, N], F16)
        dout("h2", [D, N], F16)
        dout("PVP", [D, N], F16)
        dout("vedge", [N, N], F16)
        dout("sedge", [N, N], F16)
        dout("ybf", [D, N], F16)
        dout("zv", [D, N], F16)
        dout("VP2", [D, N], BF16)
        dout("AT", [128, N], F32)
        dout("Amy", [128, ISH], F32)
        dout("xmyn", [ISH, D], BF16)
        dout("cur", [ISH, N], F32)
        dout("ve2", [N, N], F16)
        dout("SP2", [D, N], F16)
        dout("vuf", [1, 512], F32)
        dout("alpha", [1, 512], F32)

    with tile.TileContext(nc) as tc:
        import contextlib
        with contextlib.ExitStack() as ctx, \
                nc.allow_low_precision(reason="bf16 PE transposes (no accum)"):
            _emit(ctx, nc, tc, d, prob_out, dbg)
    nc.compile()
    return nc


def _emit(ctx, nc, tc, d, prob_out, dbg=None):
    dbg = dbg or {}

    def dtap16(key, tiles):     # 16 d-major tiles -> [D, N] dram
        if key in dbg:
            for m in range(NT):
                nc.sync.dma_start(out=dbg[key].ap()[m * 128:(m + 1) * 128, :],
                                  in_=tiles[m][:, 0:N])

    def dtap(key, ap_in, row0=0, rows=None):
        if key in dbg:
            o = dbg[key].ap()
            nc.sync.dma_start(out=o[row0:row0 + (rows or o.shape[0]), :],
                              in_=ap_in)

    pw = ctx.enter_context(tc.tile_pool(name="wts", bufs=1))
    pa = ctx.enter_context(tc.tile_pool(name="acts", bufs=1))
    pt = ctx.enter_context(tc.tile_pool(name="tmp", bufs=2))
    pt1 = ctx.enter_context(tc.tile_pool(name="rows", bufs=1))
    psh = ctx.enter_context(tc.tile_pool(name="sh1k", bufs=1))
    pstr = ctx.enter_context(tc.tile_pool(name="stream", bufs=1))
    pdram = ctx.enter_context(tc.tile_pool(name="dram", bufs=1, space="DRAM"))
    pp = ctx.enter_context(tc.tile_pool(name="ps_mm", bufs=3, space="PSUM"))
    pph = ctx.enter_context(tc.tile_pool(name="ps_hold", bufs=2, space="PSUM"))
    pps = ctx.enter_context(tc.tile_pool(name="ps_st", bufs=1, space="PSUM"))

    dma = nc.sync.dma_start
    GRP = [list(range(NCORES))]

    def ppt(shape, name, dt=F32):
        return pp.tile(shape, dt, name=name, tag="mm")

    def ppt_h(shape, name, dt=F32):
        return pph.tile(shape, dt, name=name, tag="hold")

    def ppt_s(shape, name, dt=F32):
        return pps.tile(shape, dt, name=name, tag="stat")

    def sh1k(name, tagid, dt=F16, shape=None):
        return psh.tile(shape or [128, 256], dt, name=name, tag=f"sh{tagid}")

    # ---------- constants ----------
    ident_b = pa.tile([128, 128], BF16, name="ident_b")
    make_identity(nc, ident_b)
    ident_f = pt1.tile([128, 128], F32, name="ident_f", tag="ident_f")
    make_identity(nc, ident_f)
    ident_h = pa.tile([128, 128], F16, name="ident_h")
    nc.vector.tensor_copy(out=ident_h, in_=ident_f)

    ones_r = pa.tile([128, 1], F32R, name="ones_r")
    nc.vector.memset(ones_r[:].bitcast(F32), 1.0)
    ones_h = pa.tile([128, 1], F16, name="ones_h")
    nc.vector.memset(ones_h, 1.0)
    ones1_r = pa.tile([1, 128], F32R, name="ones1_r")
    nc.vector.memset(ones1_r[:].bitcast(F32), 1.0)

    ones8_r = pa.tile([8, 1], F32R, name="ones8_r")
    nc.vector.memset(ones8_r[:].bitcast(F32), 1.0)
    epsc = pa.tile([128, 1], F32, name="epsc")
    nc.vector.memset(epsc, EPS)

    # ---------- inputs -> SBUF ----------
    at_f, at_b = [], []
    off = 0
    for kt, ksz in enumerate(S_KT):
        tf = psh.tile([128, N], F16, name=f"at_f{kt}", tag=f"sh{9 + kt}")
        tb = pa.tile([128, N], BF16, name=f"at_b{kt}")
        dma(out=tf[0:ksz, :], in_=d["attrT"].ap()[off:off + ksz, :])
        dma(out=tb[0:ksz, :], in_=d["attrTb"].ap()[off:off + ksz, :])
        at_f.append(tf)
        at_b.append(tb)
        off += ksz
    centT_t = []
    off = 0
    for kt, ksz in enumerate(S_KT):
        t = pa.tile([128, KEXP], F32, name=f"centT{kt}")
        dma(out=t[0:ksz, :], in_=d["centT"].ap()[off:off + ksz, :])
        centT_t.append(t)
        off += ksz
    expbt = []
    for m in range(NT):
        t = pw.tile([128, KEXP], F32, name=f"expbt{m}")
        dma(out=t, in_=d["expBT"].ap()[m * 128:(m + 1) * 128, :])
        expbt.append(t)

    def load16(name, key, ncols, dt=F16, tagbase=None):
        out = []
        for kt in range(NT):
            tg = f"{tagbase}{kt}" if tagbase else ""
            t = pw.tile([128, ncols], dt, name=f"{name}{kt}", tag=tg)
            dma(out=t, in_=d[key].ap()[kt * 128:(kt + 1) * 128, :])
            out.append(t)
        return out

    w1s_t = load16("w1s", "w1s", DSH)
    vnWs_t = load16("vnWs", "vnWs", DSH)
    snWs_t = load16("snWs", "snWs", DSH, tagbase="vnWs")
    fusWs_t = load16("fusWs", "fusWs", DSH, tagbase="w1s")
    # imgT packed: 2 tiles [128, 8*256] bf16; lhsT slice kt -> [:, (kt%8)*256+...]
    imgT_p = []
    for hh in range(2):
        t = pw.tile([128, 8 * BSH], F16, name=f"imgTp{hh}")
        src_ap = d["imgT"].ap()[hh * 1024:(hh + 1) * 1024, :].rearrange(
            "(kt p) b -> p kt b", p=128)
        nc.sync.dma_start(out=t[:].rearrange("p (kt b) -> p kt b", kt=8),
                          in_=src_ap)
        imgT_p.append(t)

    def imgT_sl(kt, bt):
        return imgT_p[kt // 8][:, (kt % 8) * BSH + bt * 128:
                               (kt % 8) * BSH + (bt + 1) * 128]

    # veW1 packed single [128, 2048] bf16
    veW1_a = pw.tile([128, D], BF16, name="veW1_a")
    nc.sync.dma_start(out=veW1_a[:].rearrange("p (kt h) -> p kt h", kt=NT),
                      in_=d["veW1"].ap().rearrange("(kt p) h -> p kt h", p=128))

    def veW1_sl(kt):
        return veW1_a[:, kt * H:(kt + 1) * H]

    def loadmat(name, key, rows, cols, dt=F32):
        t = pw.tile([rows, cols], dt, name=name)
        dma(out=t, in_=d[key].ap()[:, :])
        return t

    bnG_t = loadmat("bnG_t", "bnG", 128, 2)
    bnB_t = loadmat("bnB_t", "bnB", 128, 2)
    vnbs_t = loadmat("vnbs_t", "vnbs", 128, 2)
    snbs_t = loadmat("snbs_t", "snbs", 128, 2)
    veb1_t = loadmat("veb1_t", "veb1", H, 1)
    veb2_t = loadmat("veb2_t", "veb2", ISH, 1)
    b2o8_t = loadmat("b2o8_t", "b2o8", 128, NT)
    veW2_t = loadmat("veW2_t", "veW2", H, 1, BF16)
    fusUs_t = loadmat("fusUs_t", "fusUs", 128, 2, F16)
    selv_t = []
    for jb, (j0, jw) in enumerate(N_MT):
        t = pw.tile([128, ISH], BF16, name=f"selv{jb}")
        dma(out=t[0:jw, :], in_=d["selv"].ap()[j0:j0 + jw, :])
        selv_t.append(t)

    # collective bounce buffers (fp16 payloads: 10-bit mantissa is enough —
    # verified against reference; halves on-wire bytes)
    ar_h2_in = pdram.tile([D, N], F16, name="ar_h2_in")
    ar_h2_out = pdram.tile([D, N], F16, addr_space="Shared", name="ar_h2_out")
    ag_zv_in = pdram.tile([DSH, N], F16, name="ag_zv_in")
    ag_zv_out = pdram.tile([D, N], F16, addr_space="Shared", name="ag_zv_out")
    ag_zs_in = pdram.tile([DSH, N], F16, name="ag_zs_in")
    ag_zs_out = pdram.tile([D, N], F16, addr_space="Shared", name="ag_zs_out")
    ag_ve_in = pdram.tile([ISH, N], F16, name="ag_ve_in")
    ag_ve_out = pdram.tile([N, N], F16, addr_space="Shared", name="ag_ve_out")
    ag_vu_in = pdram.tile([1, 512], F32R, name="ag_vu_in")
    ag_vu_out = pdram.tile([NCORES, 512], F32R, addr_space="Shared",
                           name="ag_vu_out")

    # =================================================================
    # P1: CooperationModule -> SP (f32r padded, d-major)
    # =================================================================
    offT = []
    for k in range(KEXP):
        row = []
        for kt, ksz in enumerate(S_KT):
            t = sh1k(f"offT{k}_{kt}", 3 * k + kt)
            nc.vector.memset(t[:, N:256].bitcast(F32), 0.0)
            nc.vector.tensor_scalar(
                out=t[0:ksz, 0:N], in0=at_f[kt][0:ksz, :],
                scalar1=centT_t[kt][0:ksz, k:k + 1], scalar2=None,
                op0=OP.subtract)
            row.append(t)
        offT.append(row)

    SP = [pa.tile([128, 256], F16, name=f"SP{m}") for m in range(NT)]
    for m in range(NT):
        nc.vector.memset(SP[m][:, N:256].bitcast(F32), 0.0)
    for k in range(KEXP):
        for half in range(4):
            wst = []
            for kt, ksz in enumerate(S_KT):
                w = pstr.tile([128, D // 4], F16, name=f"expw_st{kt}",
                              tag=f"str{kt}")
                dma(out=w[0:ksz, :],
                    in_=d["expW"].ap()[k, sum(S_KT[:kt]):sum(S_KT[:kt]) + ksz,
                                       half * 512:(half + 1) * 512])
                wst.append(w)
            for mh in range(4):
                m = half * 4 + mh
                ps = ppt([128, 256], "ps_exp")
                for kt, ksz in enumerate(S_KT):
                    nc.tensor.matmul(ps, wst[kt][0:ksz, mh * 128:(mh + 1) * 128],
                                     offT[k][kt][0:ksz, 0:256],
                                     start=(kt == 0),
                                     stop=(kt == len(S_KT) - 1))
                if k == 0:
                    nc.vector.tensor_scalar(out=SP[m][:, 0:N], in0=ps[:, 0:N],
                                            scalar1=expbt[m][:, 0:1],
                                            scalar2=0.0,
                                            op0=OP.add, op1=OP.max)
                else:
                    r1 = pt1.tile([128, N], F16, name="exr", tag="exr")
                    nc.scalar.activation(r1, ps[:, 0:N], AF.Relu,
                                         bias=expbt[m][:, k:k + 1], scale=1.0)
                    nc.vector.tensor_tensor(SP[m][:, 0:N], SP[m][:, 0:N], r1,
                                            OP.add)
    dtap16("SP", SP)

    # =================================================================
    # P2: semantic2visual -> AllReduce(h2)
    # =================================================================
    a1 = []
    for m2 in range(2):
        ps = ppt([128, 256], "ps_h1")
        for kt in range(NT):
            nc.tensor.matmul(ps, w1s_t[kt][:, m2 * 128:(m2 + 1) * 128],
                             SP[kt][:, 0:256], start=(kt == 0),
                             stop=(kt == NT - 1))
        st6 = pt.tile([128, 6], F32, name="bn_st", tag="bn_st")
        mv = pt.tile([128, 2], F32, name="bn_mv", tag="bn_mv")
        nc.vector.bn_stats(out=st6, in_=ps[:, 0:N])
        nc.vector.bn_aggr(out=mv, in_=st6)
        sd = pt.tile([128, 1], F32, name="bn_sd", tag="bn_sd")
        nc.scalar.activation(sd, mv[:, 1:2], AF.Sqrt, bias=epsc[:, 0:1],
                             scale=1.0)
        rs = pt.tile([128, 1], F32, name="bn_rs", tag="bn_rs")
        nc.vector.reciprocal(out=rs, in_=sd)
        Av = pt.tile([128, 1], F32, name="bn_A", tag="bn_A")
        nc.vector.tensor_tensor(Av, rs, bnG_t[:, m2:m2 + 1], OP.mult)
        Bt = pt.tile([128, 1], F32, name="bn_Bt", tag="bn_Bt")
        nc.vector.tensor_tensor(Bt, mv[:, 0:1], Av, OP.mult)
        Bv = pt.tile([128, 1], F32, name="bn_Bv", tag="bn_Bv")
        nc.vector.tensor_tensor(Bv, bnB_t[:, m2:m2 + 1], Bt, OP.subtract)
        t1 = pt.tile([128, N], F32, name="h1_t1", tag="zt1")
        nc.vector.tensor_scalar(out=t1, in0=ps[:, 0:N], scalar1=Av,
                                scalar2=Bv, op0=OP.mult, op1=OP.add)
        a1m = pa.tile([128, 256], F16, name=f"a1_{m2}")
        nc.vector.memset(a1m[:, N:256].bitcast(F32), 0.0)
        nc.vector.scalar_tensor_tensor(out=a1m[:, 0:N], in0=t1, scalar=0.2,
                                       in1=t1, op0=OP.mult, op1=OP.max)
        dtap("a1", a1m[:, 0:N], row0=m2 * 128, rows=128)
        a1.append(a1m)

    for half in range(4):
        w2st = []
        for kt in range(2):
            w = pstr.tile([128, D // 4], F16, name=f"w2_st{kt}",
                          tag=f"str{kt}")
            dma(out=w, in_=d["w2s"].ap()[kt * 128:(kt + 1) * 128,
                                         half * 512:(half + 1) * 512])
            w2st.append(w)
        for mh in range(4):
            m = half * 4 + mh
            ps = ppt([128, 256], "ps_h2")
            for kt2 in range(2):
                nc.tensor.matmul(ps, w2st[kt2][:, mh * 128:(mh + 1) * 128],
                                 a1[kt2][:, 0:256], start=(kt2 == 0),
                                 stop=(kt2 == 1))
            hp = pt.tile([128, N], F16, name="h2p", tag="zouts")
            nc.vector.tensor_scalar(out=hp, in0=ps[:, 0:N],
                                    scalar1=b2o8_t[:, m:m + 1],
                                    scalar2=None, op0=OP.add)
            dma(out=ar_h2_in[m * 128:(m + 1) * 128, :], in_=hp)
    nc.gpsimd.collective_compute("AllReduce", OP.add, replica_groups=GRP,
                                 ins=[ar_h2_in[:].opt()],
                                 outs=[ar_h2_out[:].opt()])

    # ---- znorm: load z (f16 payload), inst-norm, fin(m, z, t1) writes ----
    def znorm(src_dram, fin, zn, tagset):
        zh = []
        for m in range(NT):
            t = psh.tile([128, 256], F16, name=f"zh{zn}_{m}", tag=f"sh{m}")
            dma(out=t[:, 0:N], in_=src_dram[m * 128:(m + 1) * 128, :])
            nc.vector.memset(t[:, N:256].bitcast(F32), 0.0)
            zh.append(t)
        pstat = ppt_s([1, 512], f"st_{zn}")
        for m in range(NT):
            nc.tensor.matmul(pstat[0:1, 0:256], ones_h, zh[m][:, 0:256],
                             start=(m == 0), stop=(m == NT - 1))
        mu = pt1.tile([1, 256], F32R, name="zmu", tag="zmu")
        nc.vector.tensor_scalar_mul(mu, pstat[0:1, 0:256], 1.0 / D)
        pmu = ppt_h([128, 256], f"mub_{zn}")
        nc.tensor.matmul(pmu, ones1_r, mu, start=True, stop=True)
        z = []
        for m in range(NT):
            # centered z in f32r (f16->f32r conversion fused into subtract)
            t = pa.tile([128, 256], F16, name=f"z{zn}_{m}",
                        tag=f"{tagset}{m}")
            nc.vector.memset(t[:, N:256].bitcast(F32), 0.0)
            nc.vector.tensor_tensor(t[:, 0:N], zh[m][:, 0:N], pmu[:, 0:N],
                                    OP.subtract)
            z.append(t)
            zq = pt.tile([128, 256], F16, name="zq", tag="zq")
            nc.vector.tensor_tensor(zq, t, t, OP.mult)
            nc.tensor.matmul(pstat[0:1, 256:512], ones_h, zq[:, 0:256],
                             start=(m == 0), stop=(m == NT - 1))
        va = pt1.tile([1, 256], F32, name="zva", tag="zva")
        nc.vector.tensor_scalar(out=va, in0=pstat[0:1, 256:512],
                                scalar1=1.0 / D, scalar2=EPS,
                                op0=OP.mult, op1=OP.add)
        ta = pt1.tile([1, 256], F32, name="zta", tag="zmu2")
        nc.scalar.activation(ta, va, AF.Sqrt)
        rsf = pt1.tile([1, 256], F32, name="zrsf", tag="znm")
        nc.vector.reciprocal(out=rsf, in_=ta)
        rs = pt1.tile([1, 256], F32R, name="zrs", tag="zrs")
        nc.vector.tensor_copy(out=rs, in_=rsf)
        prr = ppt_h([128, 256], f"rb_{zn}")
        nc.tensor.matmul(prr, ones1_r, rs, start=True, stop=True)
        for m in range(NT):
            t1 = pt.tile([128, N], F32, name="zt1", tag="zt1")
            nc.vector.tensor_tensor(t1, z[m][:, 0:N], prr[:, 0:N], OP.mult)
            fin(m, z, t1)
        return z

    # h2 -> PVP (leaky), in-place in z set "za"
    def fin_pvp(m, z, t1):
        nc.vector.scalar_tensor_tensor(out=z[m][:, 0:N], in0=t1, scalar=0.2,
                                       in1=t1, op0=OP.mult, op1=OP.max)
    if "h2" in dbg:
        nc.sync.dma_start(out=dbg["h2"].ap()[:, :], in_=ar_h2_out[:, :])
    PVP = znorm(ar_h2_out, fin_pvp, "h2", "za")
    dtap16("PVP", PVP)

    # PVP_n (f32r) on the shared n-major chain
    def transpose_nmajor(src_tiles, name, tagbase, dt):
        out = [pa.tile([128, D], dt, name=f"{name}0", tag=f"{tagbase}0"),
               pa.tile([128, D], dt, name=f"{name}1", tag=f"{tagbase}1")]
        for m in range(NT):
            for jb, (j0, jw) in enumerate(N_MT):
                if dt == F32R:
                    ps = ppt_h([128, 128], "ps_tr", F32)
                    nc.tensor.transpose(
                        ps[0:jw, 0:128],
                        src_tiles[m][:, j0:j0 + jw].bitcast(F32), ident_f)
                else:
                    ps = ppt_h([128, 128], "ps_tr", dt)
                    idm = ident_b if dt == BF16 else ident_h
                    nc.tensor.transpose(ps[0:jw, 0:128],
                                        src_tiles[m][:, j0:j0 + jw], idm)
                nc.vector.tensor_copy(
                    out=out[jb][0:jw, m * 128:(m + 1) * 128],
                    in_=ps[0:jw, 0:128])
        return out

    PVP_n = transpose_nmajor(PVP, "PVP_n", "nmj", F16)

    # =================================================================
    # P3: cos edges (visual f32r, semantic bf16 gram -> f32r edges)
    # =================================================================
    def cos_edge(x_tiles, ksizes, en, rdt, ones_g, rhs_w):
        nkt = len(ksizes)
        pn = ppt_s([1, 256] if rhs_w == 256 else [1, N], f"nrm_{en}")
        for kt, ksz in enumerate(ksizes):
            xq = pt.tile([128, rhs_w], rdt, name="xq", tag="xq")
            nc.vector.tensor_tensor(xq[0:ksz, :], x_tiles[kt][0:ksz, 0:rhs_w],
                                    x_tiles[kt][0:ksz, 0:rhs_w], OP.mult)
            nc.tensor.matmul(pn, ones_g[0:ksz, :], xq[0:ksz, :],
                             start=(kt == 0), stop=(kt == nkt - 1))
        sd = pt.tile([1, N], F32, name="esd", tag="zmu")
        nc.scalar.activation(sd, pn[0:1, 0:N], AF.Sqrt)
        rn_f = pt1.tile([1, 256], F32, name="ern_f", tag="zva")
        nc.vector.memset(rn_f[0:1, N:256], 0.0)
        nc.vector.reciprocal(out=rn_f[0:1, 0:N], in_=sd)
        rn = pt1.tile([1, 256], F32R, name="ern", tag="zAB")
        nc.vector.tensor_copy(out=rn, in_=rn_f)
        prn = ppt_h([128, 256], f"rnb_{en}")
        nc.tensor.matmul(prn, ones1_r, rn, start=True, stop=True)
        rcol = pt.tile([128, 2], F32, name="rc", tag=f"rc_{en}")
        for mt, (i0, iw) in enumerate(N_MT):
            pst = ppt_h([128, 128], "ps_tr3")
            nc.tensor.transpose(pst[0:iw, 0:1], rn_f[0:1, i0:i0 + iw],
                                ident_f[0:1, 0:1])
            nc.vector.tensor_copy(out=rcol[0:iw, mt:mt + 1],
                                  in_=pst[0:iw, 0:1])
        edge = []
        for mt, (i0, iw) in enumerate(N_MT):
            ps = ppt([128, rhs_w], f"ps_{en}")
            for kt, ksz in enumerate(ksizes):
                nc.tensor.matmul(ps[0:iw, :], x_tiles[kt][0:ksz, i0:i0 + iw],
                                 x_tiles[kt][0:ksz, 0:rhs_w],
                                 start=(kt == 0), stop=(kt == nkt - 1))
            s1 = pt.tile([128, N], F32, name="es1", tag="zt1")
            nc.vector.tensor_scalar(out=s1[0:iw, :], in0=ps[0:iw, 0:N],
                                    scalar1=rcol[0:iw, mt:mt + 1],
                                    scalar2=None, op0=OP.mult)
            nc.vector.tensor_tensor(s1[0:iw, :], s1[0:iw, :], prn[0:iw, 0:N],
                                    OP.mult)
            rmx = pt.tile([128, 1], F32, name="ermx", tag="ermx")
            nc.vector.reduce_max(rmx[0:iw, :], s1[0:iw, :], axis=AX.X)
            bia = pt.tile([128, 1], F32, name="ebia", tag="ebia")
            nc.vector.tensor_scalar_mul(bia[0:iw, :], rmx[0:iw, :], -100.0)
            nc.scalar.activation(s1[0:iw, :], s1[0:iw, :], AF.Exp,
                                 bias=bia[0:iw, 0:1], scale=100.0)
            sm = pt.tile([128, 1], F32, name="esm", tag="esm")
            nc.vector.reduce_sum(sm[0:iw, :], s1[0:iw, :], axis=AX.X)
            rr = pt.tile([128, 1], F32, name="err", tag="err")
            nc.vector.reciprocal(out=rr[0:iw, :], in_=sm[0:iw, :])
            ed = pa.tile([128, N], F16, name=f"{en}_{mt}",
                         tag=f"edg_{en}_{mt}")
            nc.vector.tensor_scalar(out=ed[0:iw, :], in0=s1[0:iw, :],
                                    scalar1=rr[0:iw, 0:1], scalar2=None,
                                    op0=OP.mult)
            edge.append(ed)
        edgeT = [pa.tile([128, 256], F16, name=f"{en}T0", tag=f"{en}T0"),
                 pa.tile([128, 256], F16, name=f"{en}T1", tag=f"{en}T1")]
        for jb in range(2):
            nc.vector.memset(edgeT[jb][:].bitcast(F32), 0.0)
        for mt, (i0, iw) in enumerate(N_MT):
            for jb, (j0, jw) in enumerate(N_MT):
                pst = ppt_h([128, 128], "ps_tr4", F16)
                nc.tensor.transpose(pst[0:jw, 0:iw],
                                    edge[mt][0:iw, j0:j0 + jw],
                                    ident_h[0:iw, 0:iw])
                nc.vector.tensor_copy(out=edgeT[jb][0:jw, i0:i0 + iw],
                                      in_=pst[0:jw, 0:iw])
        return edge, edgeT

    ones_b = pa.tile([128, 1], BF16, name="ones_b")
    nc.vector.memset(ones_b, 1.0)
    vedge, vedgeT = cos_edge(PVP, [128] * NT, "ve", F16, ones_h, 256)
    sedge, sedgeT = cos_edge(at_b, S_KT, "se", BF16, ones_b, N)
    if "vedge" in dbg:
        for mt, (i0, iw) in enumerate(N_MT):
            dtap("vedge", vedge[mt][0:iw, :], row0=i0, rows=iw)
            dtap("sedge", sedge[mt][0:iw, :], row0=i0, rows=iw)

    # =================================================================
    # P4: UpdateVisualNode -> AllGather(z_v) -> VP2 (in-place set "zb")
    # =================================================================
    vp_f, y_r = [], []
    for m in range(NT):
        pv = pp.tile([128, 256], F32, name="ps_vp", tag="mm")
        pe = pp.tile([128, 256], F32, name="ps_ev", tag="mm")
        for jb, (j0, jw) in enumerate(N_MT):
            nc.tensor.matmul(pv, PVP_n[jb][0:jw, m * 128:(m + 1) * 128],
                             vedgeT[jb][0:jw, :], start=(jb == 0),
                             stop=(jb == 1))
            nc.tensor.matmul(pe, PVP_n[jb][0:jw, m * 128:(m + 1) * 128],
                             sedgeT[jb][0:jw, :], start=(jb == 0),
                             stop=(jb == 1))
        vf = pa.tile([128, N], F32, name=f"vp_f{m}")
        nc.scalar.copy(out=vf, in_=pv[:, 0:N])
        vp_f.append(vf)
        yb = sh1k(f"y_{m}", m)
        nc.vector.memset(yb[:, N:256].bitcast(F32), 0.0)
        nc.vector.tensor_tensor(yb[:, 0:N], vf, pe[:, 0:N], OP.add)
        y_r.append(yb)
    dtap16("ybf", y_r)

    for m2 in range(2):
        ps = ppt([128, 256], "ps_zv")
        for kt in range(NT):
            nc.tensor.matmul(ps, vnWs_t[kt][:, m2 * 128:(m2 + 1) * 128],
                             y_r[kt][:, 0:256], start=(kt == 0),
                             stop=(kt == NT - 1))
        zc = pt.tile([128, N], F16, name="zvc", tag="zouts")
        nc.vector.tensor_scalar(out=zc, in0=ps[:, 0:N],
                                scalar1=vnbs_t[:, m2:m2 + 1],
                                scalar2=None, op0=OP.add)
        dma(out=ag_zv_in[m2 * 128:(m2 + 1) * 128, :], in_=zc)
    nc.gpsimd.collective_compute("AllGather", OP.bypass, replica_groups=GRP,
                                 ins=[ag_zv_in[:].opt()],
                                 outs=[ag_zv_out[:].opt()])
    if "zv" in dbg:
        nc.sync.dma_start(out=dbg["zv"].ap()[:, :], in_=ag_zv_out[:, :])

    VP2_bf = [pa.tile([128, N], BF16, name=f"VP2b{m}", tag=f"VP2b{m}")
              for m in range(NT)]

    def fin_vp2(m, z, t1):
        nc.vector.scalar_tensor_tensor(out=z[m][:, 0:N], in0=t1, scalar=0.0,
                                       in1=vp_f[m], op0=OP.max, op1=OP.add)
        nc.vector.tensor_copy(out=VP2_bf[m], in_=z[m][:, 0:N])
    VP2 = znorm(ag_zv_out, fin_vp2, "zv", "zb")
    dtap16("VP2", VP2_bf)

    # =================================================================
    # P6: UpdateVisualEdge (i-sharded, batched row stats, bf16)
    #   h1e(i,:) per hidden h: A_j + A_i - 2*C_ij with C via W1-tile scaling
    #   instnorm-over-H stats land as psum ROWS (one-hot partition matmuls)
    # =================================================================
    VP2_n = transpose_nmajor(VP2_bf, "VP2_n", "nmj", BF16)
    negx2my = [None] * NT
    for hh in range(2):
        xmyn = pt1.tile([ISH, D // 2], BF16, name=f"xmyn{hh}", tag="xmyn")
        for ch in range(4):
            ps = ppt([ISH, 256], "ps_xmy")
            for jb, (j0, jw) in enumerate(N_MT):
                nc.tensor.matmul(
                    ps, selv_t[jb][0:jw, :],
                    VP2_n[jb][0:jw, hh * 1024 + ch * 256:
                               hh * 1024 + (ch + 1) * 256],
                    start=(jb == 0), stop=(jb == 1))
            nc.vector.tensor_copy(out=xmyn[:, ch * 256:(ch + 1) * 256],
                                  in_=ps)
        if "xmyn" in dbg:
            nc.sync.dma_start(out=dbg["xmyn"].ap()[:, hh * 1024:
                                                   (hh + 1) * 1024],
                              in_=xmyn[:, :])
        for ktl in range(8):
            kt = hh * 8 + ktl
            pst = ppt_h([128, 128], "ps_tr5", BF16)
            nc.tensor.transpose(pst[0:128, 0:ISH],
                                xmyn[:, ktl * 128:(ktl + 1) * 128],
                                ident_b[0:ISH, 0:ISH])
            ng = pa.tile([128, ISH], F32, name=f"negx2my{kt}")
            nc.vector.tensor_scalar_mul(ng, pst[0:128, 0:ISH], -2.0)
            negx2my[kt] = ng
    pA = ppt([128, N], "ps_A")
    pAm = ppt([128, ISH], "ps_Am")
    for m in range(NT):
        xq = pt.tile([128, N], BF16, name="vsq", tag="xq")
        nc.vector.tensor_tensor(xq, VP2_bf[m], VP2_bf[m], OP.mult)
        nc.tensor.matmul(pA, veW1_sl(m), xq[:, :], start=(m == 0),
                         stop=(m == NT - 1))
        xqm = pt.tile([128, ISH], BF16, name="vsqm", tag="vsqm")
        nc.vector.tensor_tensor(xqm, negx2my[m], negx2my[m], OP.mult)
        nc.tensor.matmul(pAm, veW1_sl(m), xqm[:, :], start=(m == 0),
                         stop=(m == NT - 1))
    # A_T_bf = A_j + b1 (bf16);  A_my = A_i (f32, (-2x)^2/4)
    A_T_bf = pa.tile([128, N], BF16, name="A_T_bf", tag="A_T")
    nc.vector.tensor_scalar(out=A_T_bf, in0=pA[:, 0:N],
                            scalar1=veb1_t[:, 0:1], scalar2=None, op0=OP.add)
    A_my = pa.tile([128, ISH], F32, name="A_my")
    nc.vector.tensor_scalar_mul(A_my, pAm, 0.25)
    if "AT" in dbg:
        atf = pt.tile([128, N], F32, name="atf", tag="zt1")
        nc.vector.tensor_copy(out=atf, in_=A_T_bf)
        dtap("AT", atf[0:128, 0:N])
        amf = pt.tile([128, ISH], F32, name="amf", tag="vsqm")
        nc.vector.tensor_scalar(out=amf, in0=A_my, scalar1=veb1_t[:, 0:1],
                                scalar2=None, op0=OP.add)
        dtap("Amy", amf[0:128, :])
    vedge_my = pa.tile([ISH, N], F32, name="vedge_my", tag="edg_se_0")
    psvm = ppt([ISH, N], "ps_vm")
    for mt, (i0, iw) in enumerate(N_MT):
        vb = pt.tile([128, N], BF16, name="vedgb", tag="xq")
        nc.vector.tensor_copy(out=vb[0:iw, :], in_=vedge[mt][0:iw, :])
        nc.tensor.matmul(psvm, selv_t[mt][0:iw, :], vb[0:iw, :],
                         start=(mt == 0), stop=(mt == 1))
    nc.vector.tensor_copy(out=vedge_my, in_=psvm)

    # lhsT row-placement patterns: slice [ISH-1-k : 2*ISH-1-k] has ones (or
    # veW2) in column k, zeros elsewhere -> matmul accumulates into psum row k
    PAT = pa.tile([128, 2 * ISH - 1], BF16, name="PAT")
    nc.vector.memset(PAT, 0.0)
    nc.vector.memset(PAT[:, ISH - 1:ISH], 1.0)
    W2PAT = pa.tile([128, 2 * ISH - 1], BF16, name="W2PAT")
    nc.vector.memset(W2PAT, 0.0)
    nc.vector.tensor_copy(out=W2PAT[:, ISH - 1:ISH], in_=veW2_t[:, 0:1])
    # mean-over-H broadcast as a constant matmul: mu_b = (1/H) ones @ hsb
    ONESM = pa.tile([128, 128], BF16, name="ONESM")
    nc.vector.memset(ONESM, 1.0 / H)

    cur_ps = pps.tile([ISH, 256], F32, name="cur_ps", tag="cur")
    S_ps = ppt_s([ISH, 512], "st_e")
    for ii in range(ISH):
        psC = ppt([128, 256], "ps_C")
        for kt in range(NT):
            w1i = pstr.tile([128, H], BF16, name=f"w1i{kt}",
                            tag=f"w1i{kt % 4}")
            sc = negx2my[kt][:, ii:ii + 1]
            if kt % 3 == 0:
                nc.vector.tensor_scalar(out=w1i, in0=veW1_sl(kt),
                                        scalar1=sc, scalar2=None,
                                        op0=OP.mult)
            elif kt % 3 == 1:
                nc.gpsimd.tensor_scalar(out=w1i, in0=veW1_sl(kt),
                                        scalar1=sc, scalar2=None,
                                        op0=OP.mult)
            else:
                nc.scalar.activation(w1i, veW1_sl(kt), AF.Copy, scale=sc)
            nc.tensor.matmul(psC[:, 0:N], w1i, VP2_bf[kt],
                             start=(kt == 0), stop=(kt == NT - 1))
        # hsb = (-2C + A_i) + (A_j + b1):  Act (psum in) then DVE bf16 2x
        hp_ = pt.tile([128, N], BF16, name="ehp", tag="zt1")
        nc.scalar.activation(hp_, psC[:, 0:N], AF.Identity,
                             bias=A_my[:, ii:ii + 1], scale=1.0)
        hsb = pt.tile([128, N], BF16, name="ehsb", tag=f"hsb{ii % 2}")
        nc.vector.tensor_tensor(hsb, hp_, A_T_bf, OP.add)
        hsq = pt.tile([128, N], BF16, name="ehsq", tag="xq")
        nc.vector.tensor_tensor(hsq, hsb, hsb, OP.mult)
        psel = PAT[:, ISH - 1 - ii:2 * ISH - 1 - ii]
        nc.tensor.matmul(S_ps[0:ISH, 0:N], psel, hsb,
                         start=(ii == 0), stop=(ii == ISH - 1))
        nc.tensor.matmul(S_ps[0:ISH, 256:256 + N], psel, hsq,
                         start=(ii == 0), stop=(ii == ISH - 1))
        # centered h, relu; rs scaling deferred to cur rows (rs>0)
        pm = ppt_h([128, 256], "mu_b")
        nc.tensor.matmul(pm[:, 0:N], ONESM, hsb, start=True, stop=True)
        t1b = pt.tile([128, N], BF16, name="et1b", tag="zt1")
        nc.vector.tensor_tensor(t1b, hsb, pm[:, 0:N], OP.subtract)
        h2b = pt.tile([128, N], BF16, name="eh2b", tag="h2b")
        nc.vector.tensor_scalar_max(h2b, t1b, 0.0)
        nc.tensor.matmul(cur_ps[0:ISH, 0:N],
                         W2PAT[:, ISH - 1 - ii:2 * ISH - 1 - ii], h2b,
                         start=(ii == 0), stop=(ii == ISH - 1))
    # ---- batched row stats -> rs rows; cur = cur_raw * rs ----
    mu = pt1.tile([ISH, N], F32, name="emu", tag="emu")
    nc.vector.tensor_scalar_mul(mu, S_ps[0:ISH, 0:N], 1.0 / H)
    va = pt1.tile([ISH, N], F32, name="eva", tag="eva")
    nc.vector.tensor_scalar_mul(va, S_ps[0:ISH, 256:256 + N], 1.0 / H)
    musq = pt1.tile([ISH, N], F32, name="emusq", tag="emusq")
    nc.vector.tensor_tensor(musq, mu, mu, OP.mult)
    nc.vector.tensor_tensor(va, va, musq, OP.subtract)
    sdv = pt1.tile([ISH, N], F32, name="esdv", tag="emusq")
    nc.scalar.activation(sdv, va, AF.Sqrt, bias=epsc[0:ISH, 0:1], scale=1.0)
    rsr = pt1.tile([ISH, N], F32, name="ersr", tag="eva")
    nc.vector.reciprocal(out=rsr, in_=sdv)
    curm = pa.tile([ISH, N], F32, name="curm", tag="atf2")
    nc.vector.tensor_tensor(curm, cur_ps[0:ISH, 0:N], rsr, OP.mult)
    dtap("cur", curm[:, :])

    # tanh(cur + b2) * (vedge_my + 1e-8) -> softmax(/10) -> my edge rows
    curt = pa.tile([ISH, N], F32, name="curt", tag="atf0")
    nc.scalar.activation(curt, curm, AF.Tanh,
                         bias=veb2_t[0:ISH, 0:1], scale=1.0)
    ne = pa.tile([ISH, N], F32, name="ne", tag="atf1")
    nc.vector.scalar_tensor_tensor(out=ne, in0=vedge_my, scalar=1e-8,
                                   in1=curt, op0=OP.add, op1=OP.mult)
    rmx = pt.tile([ISH, 1], F32, name="vermx", tag="vermx")
    nc.vector.reduce_max(rmx, ne, axis=AX.X)
    bia = pt.tile([ISH, 1], F32, name="vebia", tag="vebia")
    nc.vector.tensor_scalar_mul(bia, rmx, -0.1)
    ex = pt1.tile([ISH, N], F32, name="veex", tag="veex")
    nc.scalar.activation(ex, ne, AF.Exp, bias=bia[0:ISH, 0:1], scale=0.1)
    sm = pt.tile([ISH, 1], F32, name="vesm", tag="vesm")
    nc.vector.reduce_sum(sm, ex, axis=AX.X)
    rr = pt.tile([ISH, 1], F32, name="verr", tag="verr")
    nc.vector.reciprocal(out=rr, in_=sm)
    vemine = pt1.tile([ISH, N], F16, name="vemine", tag="vemine")
    nc.vector.tensor_scalar(out=vemine, in0=ex, scalar1=rr[0:ISH, 0:1],
                            scalar2=None, op0=OP.mult)
    dma(out=ag_ve_in[:, :], in_=vemine)
    nc.gpsimd.collective_compute("AllGather", OP.bypass, replica_groups=GRP,
                                 ins=[ag_ve_in[:].opt()],
                                 outs=[ag_ve_out[:].opt()])
    if "ve2" in dbg:
        nc.sync.dma_start(out=dbg["ve2"].ap()[:, :], in_=ag_ve_out[:, :])
    ve2 = [pt1.tile([128, N], F16, name="ve2_0", tag="ve2_0"),
           pt1.tile([128, N], F16, name="ve2_1", tag="ve2_1")]
    for mt, (i0, iw) in enumerate(N_MT):
        dma(out=ve2[mt][0:iw, :], in_=ag_ve_out[i0:i0 + iw, :])
    ve2T = [pa.tile([128, 256], F16, name="ve2T0", tag="veT0"),
            pa.tile([128, 256], F16, name="ve2T1", tag="veT1")]
    for mt, (i0, iw) in enumerate(N_MT):
        for jb, (j0, jw) in enumerate(N_MT):
            pst = ppt_h([128, 128], "ps_tr6", F16)
            nc.tensor.transpose(pst[0:jw, 0:iw],
                                ve2[mt][0:iw, j0:j0 + jw],
                                ident_h[0:iw, 0:iw])
            nc.vector.tensor_copy(out=ve2T[jb][0:jw, i0:i0 + iw],
                                  in_=pst[0:jw, 0:iw])

    # =================================================================
    # P5: UpdateSemanticNode -> AllGather(z_s) -> SP2 (in-place set "za")
    # =================================================================
    SP_n = transpose_nmajor(SP, "SP_n", "nmj", F16)
    sp_f, y2_r = [], []
    for m in range(NT):
        psp = pp.tile([128, 256], F32, name="ps_sp", tag="mm")
        pes = pp.tile([128, 256], F32, name="ps_es", tag="mm")
        for jb, (j0, jw) in enumerate(N_MT):
            nc.tensor.matmul(psp, SP_n[jb][0:jw, m * 128:(m + 1) * 128],
                             sedgeT[jb][0:jw, :], start=(jb == 0),
                             stop=(jb == 1))
            nc.tensor.matmul(pes, SP_n[jb][0:jw, m * 128:(m + 1) * 128],
                             ve2T[jb][0:jw, :], start=(jb == 0),
                             stop=(jb == 1))
        sf = pa.tile([128, N], F32, name=f"sp_f{m}", tag=f"vp_f{m}")
        nc.scalar.copy(out=sf, in_=psp[:, 0:N])
        sp_f.append(sf)
        yb = sh1k(f"y2_{m}", m)
        nc.vector.memset(yb[:, N:256].bitcast(F32), 0.0)
        nc.vector.tensor_tensor(yb[:, 0:N], sf, pes[:, 0:N], OP.add)
        y2_r.append(yb)

    for m2 in range(2):
        ps = ppt([128, 256], "ps_zs")
        for kt in range(NT):
            nc.tensor.matmul(ps, snWs_t[kt][:, m2 * 128:(m2 + 1) * 128],
                             y2_r[kt][:, 0:256], start=(kt == 0),
                             stop=(kt == NT - 1))
        zc = pt.tile([128, N], F16, name="zsc", tag="zouts")
        nc.vector.tensor_scalar(out=zc, in0=ps[:, 0:N],
                                scalar1=snbs_t[:, m2:m2 + 1],
                                scalar2=None, op0=OP.add)
        dma(out=ag_zs_in[m2 * 128:(m2 + 1) * 128, :], in_=zc)
    nc.gpsimd.collective_compute("AllGather", OP.bypass, replica_groups=GRP,
                                 ins=[ag_zs_in[:].opt()],
                                 outs=[ag_zs_out[:].opt()])

    def fin_sp2(m, z, t1):
        nc.vector.scalar_tensor_tensor(out=z[m][:, 0:N], in0=t1, scalar=0.0,
                                       in1=sp_f[m], op0=OP.max, op1=OP.add)
    SP2 = znorm(ag_zs_out, fin_sp2, "zs", "za")
    if "SP2" in dbg:
        for m in range(NT):
            nc.sync.dma_start(out=dbg["SP2"].ap()[m * 128:(m + 1) * 128, :],
                              in_=SP2[m][:, 0:N])

    # =================================================================
    # P7: FusionLayer (f32r) -> alpha -> prob
    # =================================================================
    pvu = [ppt_s([1, 256], "ps_vu0"), ppt_s([1, 256], "ps_vu1")]
    for k, srct in enumerate((VP2, SP2)):
        for m2 in range(2):
            ps = ppt([128, 256], "ps_fus")
            for kt in range(NT):
                nc.tensor.matmul(ps, fusWs_t[kt][:, m2 * 128:(m2 + 1) * 128],
                                 srct[kt][:, 0:256], start=(kt == 0),
                                 stop=(kt == NT - 1))
            th = pt.tile([128, 256], F16, name="fth", tag="fth")
            nc.scalar.activation(th, ps, AF.Tanh)
            nc.tensor.matmul(pvu[k], fusUs_t[:, m2:m2 + 1], th[:, :],
                             start=(m2 == 0), stop=(m2 == 1))
    vu_sb = pt1.tile([1, 512], F32R, name="vu_sb", tag="vu_sb")
    nc.vector.memset(vu_sb[:].bitcast(F32), 0.0)
    nc.vector.tensor_copy(out=vu_sb[0:1, 0:N], in_=pvu[0][0:1, 0:N])
    nc.vector.tensor_copy(out=vu_sb[0:1, 256:256 + N], in_=pvu[1][0:1, 0:N])
    dma(out=ag_vu_in[:, :], in_=vu_sb)
    nc.gpsimd.collective_compute("AllGather", OP.bypass, replica_groups=GRP,
                                 ins=[ag_vu_in[:].opt()],
                                 outs=[ag_vu_out[:].opt()])
    vus = pt1.tile([NCORES, 512], F32R, name="vus", tag="vu_sb")
    dma(out=vus, in_=ag_vu_out[:, :])
    pvk = ppt_s([1, 512], "ps_vuk")
    for k in range(2):
        nc.tensor.matmul(pvk[0:1, 256 * k:256 * k + 256],
                         ones8_r, vus[:, 256 * k:256 * k + 256],
                         start=True, stop=True)
    vuf = pt1.tile([1, 512], F32, name="vuf", tag="zstt")
    nc.vector.tensor_copy(out=vuf, in_=pvk)
    dtap("vuf", vuf[:, :])
    mx = pt.tile([1, N], F32, name="amx", tag="amx")
    nc.vector.tensor_tensor(mx, vuf[0:1, 0:N], vuf[0:1, 256:256 + N], OP.max)
    dv = pt1.tile([1, 512], F32R, name="adv", tag="adv")
    nc.vector.memset(dv[:].bitcast(F32), 0.0)
    for k in range(2):
        nc.vector.tensor_tensor(dv[0:1, 256 * k:256 * k + N],
                                vuf[0:1, 256 * k:256 * k + N], mx, OP.subtract)
    nc.scalar.activation(dv, dv, AF.Exp, scale=100.0)
    ssum = pt.tile([1, N], F32, name="assum", tag="assum")
    nc.vector.tensor_tensor(ssum, dv[0:1, 0:N], dv[0:1, 256:256 + N], OP.add)
    rsu = pt.tile([1, N], F32, name="arsu", tag="arsu")
    nc.vector.reciprocal(out=rsu, in_=ssum)
    for k in range(2):
        nc.vector.tensor_tensor(dv[0:1, 256 * k:256 * k + N],
                                dv[0:1, 256 * k:256 * k + N], rsu, OP.mult)
    alro = dv
    if "alpha" in dbg:
        al_f = pt1.tile([1, 512], F32, name="al_f", tag="zstt")
        nc.vector.tensor_copy(out=al_f, in_=alro)
        dtap("alpha", al_f[:, :])
    pal = ppt_h([128, 512], "ab_al")
    nc.tensor.matmul(pal, ones1_r, alro, start=True, stop=True)
    proto_bf = []
    for m in range(NT):
        t1 = pt.tile([128, N], F32, name="pr1", tag="zouts")
        nc.vector.tensor_tensor(t1, VP2[m][:, 0:N], pal[:, 0:N], OP.mult)
        t2 = pt.tile([128, N], F32, name="pr2", tag="zt1")
        nc.vector.tensor_tensor(t2, SP2[m][:, 0:N], pal[:, 256:256 + N],
                                OP.mult)
        pb = pa.tile([128, N], F16, name=f"proto{m}", tag=f"VP2b{m}")
        nc.vector.tensor_tensor(pb, t1, t2, OP.add)
        proto_bf.append(pb)
    for bt in range(2):
        ps = ppt([128, N], "ps_prob")
        for kt in range(NT):
            nc.tensor.matmul(ps, imgT_sl(kt, bt), proto_bf[kt][:, :],
                             start=(kt == 0), stop=(kt == NT - 1))
        t1 = pt.tile([128, N], F32, name="probf", tag="zouts")
        nc.vector.tensor_copy(out=t1, in_=ps)
        dma(out=prob_out.ap()[bt * 128:(bt + 1) * 128, :], in_=t1)


# =====================================================================
# Host side
# =====================================================================
def _prep_inputs(inputs):
    bf = ml_dtypes.bfloat16
    f16 = np.float16
    f32 = np.float32
    att = np.asarray(inputs["attribute"], f32)
    cen = np.asarray(inputs["centers"], f32)
    expW = np.asarray(inputs["expert_W"], f32)
    expB = np.asarray(inputs["expert_b"], f32)
    w1 = np.asarray(inputs["s2v_W1"], f32)
    w2 = np.asarray(inputs["s2v_W2"], f32)
    in_maps = []
    for c in range(NCORES):
        cs = slice(c * DSH, (c + 1) * DSH)
        isl = slice(c * ISH, (c + 1) * ISH)
        bs = slice(c * BSH, (c + 1) * BSH)
        selv = np.zeros((N, ISH), f32)
        selv[np.arange(c * ISH, (c + 1) * ISH), np.arange(ISH)] = 1.0
        m = {
            "attrT": np.ascontiguousarray(att.T).astype(f16),
            "attrTb": np.ascontiguousarray(att.T).astype(bf),
            "centT": np.ascontiguousarray(cen.T),
            "expW": expW.astype(f16),
            "expBT": np.ascontiguousarray(expB.T),
            "w1s": np.ascontiguousarray(w1[:, cs]).astype(f16),
            "bnG": np.ascontiguousarray(np.asarray(inputs["bn_g"], f32)[cs].reshape(2, 128).T),
            "bnB": np.ascontiguousarray(np.asarray(inputs["bn_b"], f32)[cs].reshape(2, 128).T),
            "w2s": np.ascontiguousarray(w2[cs, :]).astype(f16),
            "b2o8": np.ascontiguousarray((np.asarray(inputs["s2v_b2"], f32) / NCORES).reshape(NT, 128).T),
            "vnWs": np.ascontiguousarray(np.asarray(inputs["vn_W"], f32)[:, cs]).astype(f16),
            "vnbs": np.ascontiguousarray(np.asarray(inputs["vn_b"], f32)[cs].reshape(2, 128).T),
            "snWs": np.ascontiguousarray(np.asarray(inputs["sn_W"], f32)[:, cs]).astype(f16),
            "snbs": np.ascontiguousarray(np.asarray(inputs["sn_b"], f32)[cs].reshape(2, 128).T),
            "veW1": np.asarray(inputs["ve_W1"], f32).astype(bf),
            "veb1": np.asarray(inputs["ve_b1"], f32)[:, None],
            "veW2": np.asarray(inputs["ve_W2"], f32).astype(bf),
            "veb2": np.full((ISH, 1), float(np.asarray(inputs["ve_b2"])[0]),
                            f32),
            "fusWs": np.ascontiguousarray(np.asarray(inputs["fus_W"], f32)[:, cs]).astype(f16),
            "fusUs": np.ascontiguousarray(np.asarray(inputs["fus_u"], f32)[cs, 0].reshape(2, 128).T).astype(f16),
            "imgT": np.ascontiguousarray(
                np.asarray(inputs["img_feat"], f32)[bs, :].T).astype(f16),
            "selv": selv.astype(bf),
        }
        in_maps.append(m)
    return in_maps


def kernel(**inputs):
    global _BUILT
    if _BUILT is None:
        _BUILT = build()
    nc = _BUILT
    in_maps = _prep_inputs(inputs)
    res = run_bass_kernel_spmd(nc, in_maps, core_ids=list(range(NCORES)))
    out = np.concatenate([res.results[c]["prob"] for c in range(NCORES)],
                         axis=0)
    return out.astype(np.float32)


def kernel_debug(**inputs):
    nc = build(debug=True)
    in_maps = _prep_inputs(inputs)
    res = run_bass_kernel_spmd(nc, in_maps, core_ids=list(range(NCORES)))
    out = np.concatenate([res.results[c]["prob"] for c in range(NCORES)],
                         axis=0)
    return out.astype(np.float32), res.results


if __name__ == "__main__":
    import reference
    inp = {k: np.asarray(v) for k, v in reference.setup_inputs().items()}
    got = kernel(**inp)
    exp = np.asarray(reference.reference(**reference.setup_inputs()))
    err = np.abs(got - exp).max() / (np.abs(exp).max() + 1e-9)
    print("Relative error:", err)



# revision 44
# speedup vs baseline: 1.3516x; 1.0315x over previous
# Trainium2 Bass kernel for nn_CPPN (gnn_message_passing), 8-core SPMD.
#
# Sharding:
#   - Node-MLP weights (s2v_W1/W2, vn_W, sn_W, fus_W) sharded over the 2048
#     col/row dim (256 per core); stitched with one AllReduce (h2) and three
#     AllGathers (z_v, z_s, vu partials).
#   - Visual edge MLP (200x200 pairwise rows) sharded over i: 25 rows/core,
#     AllGather of the resulting edge rows.  Per-core column selection is via
#     a host-supplied one-hot matrix (SPMD program is identical on all cores;
#     only input data differs).
#   - img_feat batch sharded 256 rows/core for the final prob matmul; host
#     concatenates per-core outputs.
# Layout: activations are d-major [D on partitions (16x128), 200 on free].
# Heavy matmuls bf16; stats/broadcast matmuls f32r.  The reference's second
# edge_update (semantic) is dead code w.r.t. prob and is skipped.

import sys

sys.path.insert(0, "/opt/trn_rl_repo")

import numpy as np
import ml_dtypes

import concourse.bass as bass
import concourse.bacc as bacc
import concourse.tile as tile
from concourse import mybir
from concourse.bass_utils import run_bass_kernel_spmd
from concourse.masks import make_identity

F32 = mybir.dt.float32
F32R = mybir.dt.float32r
BF16 = mybir.dt.bfloat16
F16 = mybir.dt.float16
AF = mybir.ActivationFunctionType
OP = mybir.AluOpType
AX = mybir.AxisListType

NCORES = 8
N = 200
S = 312
D = 2048
H = 128
KEXP = 3
B = 2048
DSH = D // NCORES      # 256
ISH = N // NCORES      # 25
BSH = B // NCORES      # 256
EPS = 1e-5
NT = D // 128          # 16
S_KT = [128, 128, 56]
N_MT = ((0, 128), (128, 72))
G_I = 2                # i-group size for edge t-gen batching

_BUILT = None


def _rep(ap_src, dims):
    """Rebuild AP with explicit free dims [[step,count],...] after partition."""
    return bass.AP(tensor=ap_src.tensor, offset=ap_src.offset,
                   ap=[ap_src.ap[0]] + dims)


def build(debug=False):
    nc = bacc.Bacc("TRN2", target_bir_lowering=False, debug=False,
                   num_devices=NCORES)
    d = {}

    def din(name, shape, dt):
        d[name] = nc.dram_tensor(name, shape, dt, kind="ExternalInput")

    din("attrT", [S, N], F16)
    din("attrTb", [S, N], BF16)
    din("centT", [S, KEXP], F32)
    din("expW", [KEXP, S, D], F16)
    din("expBT", [D, KEXP], F32)
    din("w1s", [D, DSH], F16)
    din("bnG", [128, 2], F32)
    din("bnB", [128, 2], F32)
    din("w2s", [DSH, D], F16)
    din("b2o8", [128, NT], F32)
    din("vnWs", [D, DSH], F16)
    din("vnbs", [128, 2], F32)
    din("snWs", [D, DSH], F16)
    din("snbs", [128, 2], F32)
    din("veW1", [D, H], BF16)
    din("veb1", [H, 1], F32)
    din("veW2", [H, 1], BF16)
    din("veb2", [ISH, 1], F32)
    din("fusWs", [D, DSH], F16)
    din("fusUs", [128, 2], F16)
    din("imgT", [D, BSH], F16)
    din("selv", [N, ISH], BF16)
    prob_out = nc.dram_tensor("prob", [BSH, N], F32, kind="ExternalOutput")
    dbg = {}
    if debug:
        def dout(name, shape, dt):
            dbg[name] = nc.dram_tensor("dbg_" + name, shape, dt,
                                       kind="ExternalOutput")
        dout("SP", [D, N], F16)
        dout("a1", [2 * 128, N], F16)
        dout("h2", [D, N], F16)
        dout("PVP", [D, N], F16)
        dout("vedge", [N, N], F16)
        dout("sedge", [N, N], F16)
        dout("ybf", [D, N], F16)
        dout("zv", [D, N], F16)
        dout("VP2", [D, N], BF16)
        dout("AT", [128, N], F32)
        dout("Amy", [128, ISH], F32)
        dout("xmyn", [ISH, D], BF16)
        dout("cur", [ISH, N], F32)
        dout("ve2", [N, N], F16)
        dout("SP2", [D, N], F16)
        dout("vuf", [1, 512], F32)
        dout("alpha", [1, 512], F32)

    with tile.TileContext(nc) as tc:
        import contextlib
        with contextlib.ExitStack() as ctx, \
                nc.allow_low_precision(reason="bf16 PE transposes (no accum)"):
            _emit(ctx, nc, tc, d, prob_out, dbg)
    nc.compile()
    return nc


def _emit(ctx, nc, tc, d, prob_out, dbg=None):
    dbg = dbg or {}

    def dtap16(key, tiles):     # 16 d-major tiles -> [D, N] dram
        if key in dbg:
            for m in range(NT):
                nc.sync.dma_start(out=dbg[key].ap()[m * 128:(m + 1) * 128, :],
                                  in_=tiles[m][:, 0:N])

    def dtap(key, ap_in, row0=0, rows=None):
        if key in dbg:
            o = dbg[key].ap()
            nc.sync.dma_start(out=o[row0:row0 + (rows or o.shape[0]), :],
                              in_=ap_in)

    pw = ctx.enter_context(tc.tile_pool(name="wts", bufs=1))
    pa = ctx.enter_context(tc.tile_pool(name="acts", bufs=1))
    pt = ctx.enter_context(tc.tile_pool(name="tmp", bufs=2))
    pt1 = ctx.enter_context(tc.tile_pool(name="rows", bufs=1))
    psh = ctx.enter_context(tc.tile_pool(name="sh1k", bufs=1))
    pstr = ctx.enter_context(tc.tile_pool(name="stream", bufs=1))
    pdram = ctx.enter_context(tc.tile_pool(name="dram", bufs=1, space="DRAM"))
    pp = ctx.enter_context(tc.tile_pool(name="ps_mm", bufs=3, space="PSUM"))
    pph = ctx.enter_context(tc.tile_pool(name="ps_hold", bufs=2, space="PSUM"))
    pps = ctx.enter_context(tc.tile_pool(name="ps_st", bufs=1, space="PSUM"))

    dma = nc.sync.dma_start
    GRP = [list(range(NCORES))]

    def ppt(shape, name, dt=F32):
        return pp.tile(shape, dt, name=name, tag="mm")

    def ppt_h(shape, name, dt=F32):
        return pph.tile(shape, dt, name=name, tag="hold")

    def ppt_s(shape, name, dt=F32):
        return pps.tile(shape, dt, name=name, tag="stat")

    def sh1k(name, tagid, dt=F16, shape=None):
        return psh.tile(shape or [128, 256], dt, name=name, tag=f"sh{tagid}")

    # ---------- constants ----------
    ident_b = pa.tile([128, 128], BF16, name="ident_b")
    make_identity(nc, ident_b)
    ident_f = pt1.tile([128, 128], F32, name="ident_f", tag="ident_f")
    make_identity(nc, ident_f)
    ident_h = pa.tile([128, 128], F16, name="ident_h")
    nc.vector.tensor_copy(out=ident_h, in_=ident_f)

    ones_r = pa.tile([128, 1], F32R, name="ones_r")
    nc.vector.memset(ones_r[:].bitcast(F32), 1.0)
    ones_h = pa.tile([128, 1], F16, name="ones_h")
    nc.vector.memset(ones_h, 1.0)
    ones1_r = pa.tile([1, 128], F32R, name="ones1_r")
    nc.vector.memset(ones1_r[:].bitcast(F32), 1.0)

    ones8_r = pa.tile([8, 1], F32R, name="ones8_r")
    nc.vector.memset(ones8_r[:].bitcast(F32), 1.0)
    epsc = pa.tile([128, 1], F32, name="epsc")
    nc.vector.memset(epsc, EPS)

    # ---------- inputs -> SBUF ----------
    at_f, at_b = [], []
    off = 0
    for kt, ksz in enumerate(S_KT):
        tf = psh.tile([128, N], F16, name=f"at_f{kt}", tag=f"sh{9 + kt}")
        tb = pa.tile([128, N], BF16, name=f"at_b{kt}")
        dma(out=tf[0:ksz, :], in_=d["attrT"].ap()[off:off + ksz, :])
        dma(out=tb[0:ksz, :], in_=d["attrTb"].ap()[off:off + ksz, :])
        at_f.append(tf)
        at_b.append(tb)
        off += ksz
    centT_t = []
    off = 0
    for kt, ksz in enumerate(S_KT):
        t = pa.tile([128, KEXP], F32, name=f"centT{kt}")
        dma(out=t[0:ksz, :], in_=d["centT"].ap()[off:off + ksz, :])
        centT_t.append(t)
        off += ksz
    expbt = []
    for m in range(NT):
        t = pw.tile([128, KEXP], F32, name=f"expbt{m}")
        dma(out=t, in_=d["expBT"].ap()[m * 128:(m + 1) * 128, :])
        expbt.append(t)

    def load16(name, key, ncols, dt=F16, tagbase=None):
        out = []
        for kt in range(NT):
            tg = f"{tagbase}{kt}" if tagbase else ""
            t = pw.tile([128, ncols], dt, name=f"{name}{kt}", tag=tg)
            dma(out=t, in_=d[key].ap()[kt * 128:(kt + 1) * 128, :])
            out.append(t)
        return out

    w1s_t = load16("w1s", "w1s", DSH)
    vnWs_t = load16("vnWs", "vnWs", DSH)
    snWs_t = load16("snWs", "snWs", DSH)
    fusWs_t = load16("fusWs", "fusWs", DSH, tagbase="w1s")
    # imgT packed: 2 tiles [128, 8*256] bf16; lhsT slice kt -> [:, (kt%8)*256+...]
    imgT_p = []
    for hh in range(2):
        t = pw.tile([128, 8 * BSH], F16, name=f"imgTp{hh}")
        src_ap = d["imgT"].ap()[hh * 1024:(hh + 1) * 1024, :].rearrange(
            "(kt p) b -> p kt b", p=128)
        nc.sync.dma_start(out=t[:].rearrange("p (kt b) -> p kt b", kt=8),
                          in_=src_ap)
        imgT_p.append(t)

    def imgT_sl(kt, bt):
        return imgT_p[kt // 8][:, (kt % 8) * BSH + bt * 128:
                               (kt % 8) * BSH + (bt + 1) * 128]

    # veW1 packed single [128, 2048] bf16
    veW1_a = pw.tile([128, D], BF16, name="veW1_a")
    nc.sync.dma_start(out=veW1_a[:].rearrange("p (kt h) -> p kt h", kt=NT),
                      in_=d["veW1"].ap().rearrange("(kt p) h -> p kt h", p=128))

    def veW1_sl(kt):
        return veW1_a[:, kt * H:(kt + 1) * H]

    def loadmat(name, key, rows, cols, dt=F32):
        t = pw.tile([rows, cols], dt, name=name)
        dma(out=t, in_=d[key].ap()[:, :])
        return t

    bnG_t = loadmat("bnG_t", "bnG", 128, 2)
    bnB_t = loadmat("bnB_t", "bnB", 128, 2)
    vnbs_t = loadmat("vnbs_t", "vnbs", 128, 2)
    snbs_t = loadmat("snbs_t", "snbs", 128, 2)
    veb1_t = loadmat("veb1_t", "veb1", H, 1)
    veb2_t = loadmat("veb2_t", "veb2", ISH, 1)
    b2o8_t = loadmat("b2o8_t", "b2o8", 128, NT)
    veW2_t = loadmat("veW2_t", "veW2", H, 1, BF16)
    fusUs_t = loadmat("fusUs_t", "fusUs", 128, 2, F16)
    selv_t = []
    for jb, (j0, jw) in enumerate(N_MT):
        t = pw.tile([128, ISH], BF16, name=f"selv{jb}")
        dma(out=t[0:jw, :], in_=d["selv"].ap()[j0:j0 + jw, :])
        selv_t.append(t)

    # collective bounce buffers (fp16 payloads: 10-bit mantissa is enough —
    # verified against reference; halves on-wire bytes)
    ar_h2_in = pdram.tile([D, N], F16, name="ar_h2_in")
    ar_h2_out = pdram.tile([D, N], F16, addr_space="Shared", name="ar_h2_out")
    ag_zv_in = pdram.tile([DSH, N], F16, name="ag_zv_in")
    ag_zv_out = pdram.tile([D, N], F16, addr_space="Shared", name="ag_zv_out")
    ag_zs_in = pdram.tile([DSH, N], F16, name="ag_zs_in")
    ag_zs_out = pdram.tile([D, N], F16, addr_space="Shared", name="ag_zs_out")
    ag_ve_in = pdram.tile([ISH, N], F16, name="ag_ve_in")
    ag_ve_out = pdram.tile([N, N], F16, addr_space="Shared", name="ag_ve_out")
    ag_vu_in = pdram.tile([1, 512], F32R, name="ag_vu_in")
    ag_vu_out = pdram.tile([NCORES, 512], F32R, addr_space="Shared",
                           name="ag_vu_out")

    # =================================================================
    # P1: CooperationModule -> SP (f32r padded, d-major)
    # =================================================================
    offT = []
    for k in range(KEXP):
        row = []
        for kt, ksz in enumerate(S_KT):
            t = sh1k(f"offT{k}_{kt}", 3 * k + kt)
            nc.vector.memset(t[:, N:256].bitcast(F32), 0.0)
            nc.vector.tensor_scalar(
                out=t[0:ksz, 0:N], in0=at_f[kt][0:ksz, :],
                scalar1=centT_t[kt][0:ksz, k:k + 1], scalar2=None,
                op0=OP.subtract)
            row.append(t)
        offT.append(row)

    SP = [pa.tile([128, 256], F16, name=f"SP{m}") for m in range(NT)]
    for m in range(NT):
        nc.vector.memset(SP[m][:, N:256].bitcast(F32), 0.0)
    for k in range(KEXP):
        for half in range(4):
            wst = []
            for kt, ksz in enumerate(S_KT):
                w = pstr.tile([128, D // 4], F16, name=f"expw_st{kt}",
                              tag=f"str{kt}")
                dma(out=w[0:ksz, :],
                    in_=d["expW"].ap()[k, sum(S_KT[:kt]):sum(S_KT[:kt]) + ksz,
                                       half * 512:(half + 1) * 512])
                wst.append(w)
            for mh in range(4):
                m = half * 4 + mh
                ps = ppt([128, 256], "ps_exp")
                for kt, ksz in enumerate(S_KT):
                    nc.tensor.matmul(ps, wst[kt][0:ksz, mh * 128:(mh + 1) * 128],
                                     offT[k][kt][0:ksz, 0:256],
                                     start=(kt == 0),
                                     stop=(kt == len(S_KT) - 1))
                if k == 0:
                    nc.vector.tensor_scalar(out=SP[m][:, 0:N], in0=ps[:, 0:N],
                                            scalar1=expbt[m][:, 0:1],
                                            scalar2=0.0,
                                            op0=OP.add, op1=OP.max)
                else:
                    r1 = pt1.tile([128, N], F16, name="exr", tag="exr")
                    nc.scalar.activation(r1, ps[:, 0:N], AF.Relu,
                                         bias=expbt[m][:, k:k + 1], scale=1.0)
                    nc.vector.tensor_tensor(SP[m][:, 0:N], SP[m][:, 0:N], r1,
                                            OP.add)
    dtap16("SP", SP)

    # =================================================================
    # P2: semantic2visual -> AllReduce(h2)
    # =================================================================
    a1 = []
    for m2 in range(2):
        ps = ppt([128, 256], "ps_h1")
        for kt in range(NT):
            nc.tensor.matmul(ps, w1s_t[kt][:, m2 * 128:(m2 + 1) * 128],
                             SP[kt][:, 0:256], start=(kt == 0),
                             stop=(kt == NT - 1))
        st6 = pt.tile([128, 6], F32, name="bn_st", tag="bn_st")
        mv = pt.tile([128, 2], F32, name="bn_mv", tag="bn_mv")
        nc.vector.bn_stats(out=st6, in_=ps[:, 0:N])
        nc.vector.bn_aggr(out=mv, in_=st6)
        sd = pt.tile([128, 1], F32, name="bn_sd", tag="bn_sd")
        nc.scalar.activation(sd, mv[:, 1:2], AF.Sqrt, bias=epsc[:, 0:1],
                             scale=1.0)
        rs = pt.tile([128, 1], F32, name="bn_rs", tag="bn_rs")
        nc.vector.reciprocal(out=rs, in_=sd)
        Av = pt.tile([128, 1], F32, name="bn_A", tag="bn_A")
        nc.vector.tensor_tensor(Av, rs, bnG_t[:, m2:m2 + 1], OP.mult)
        Bt = pt.tile([128, 1], F32, name="bn_Bt", tag="bn_Bt")
        nc.vector.tensor_tensor(Bt, mv[:, 0:1], Av, OP.mult)
        Bv = pt.tile([128, 1], F32, name="bn_Bv", tag="bn_Bv")
        nc.vector.tensor_tensor(Bv, bnB_t[:, m2:m2 + 1], Bt, OP.subtract)
        t1 = pt.tile([128, N], F32, name="h1_t1", tag="zt1")
        nc.vector.tensor_scalar(out=t1, in0=ps[:, 0:N], scalar1=Av,
                                scalar2=Bv, op0=OP.mult, op1=OP.add)
        a1m = pa.tile([128, 256], F16, name=f"a1_{m2}")
        nc.vector.memset(a1m[:, N:256].bitcast(F32), 0.0)
        nc.vector.scalar_tensor_tensor(out=a1m[:, 0:N], in0=t1, scalar=0.2,
                                       in1=t1, op0=OP.mult, op1=OP.max)
        dtap("a1", a1m[:, 0:N], row0=m2 * 128, rows=128)
        a1.append(a1m)

    for half in range(4):
        w2st = []
        for kt in range(2):
            w = pstr.tile([128, D // 4], F16, name=f"w2_st{kt}",
                          tag=f"str{kt}")
            dma(out=w, in_=d["w2s"].ap()[kt * 128:(kt + 1) * 128,
                                         half * 512:(half + 1) * 512])
            w2st.append(w)
        for mh in range(4):
            m = half * 4 + mh
            ps = ppt([128, 256], "ps_h2")
            for kt2 in range(2):
                nc.tensor.matmul(ps, w2st[kt2][:, mh * 128:(mh + 1) * 128],
                                 a1[kt2][:, 0:256], start=(kt2 == 0),
                                 stop=(kt2 == 1))
            hp = pt.tile([128, N], F16, name="h2p", tag="zouts")
            nc.vector.tensor_scalar(out=hp, in0=ps[:, 0:N],
                                    scalar1=b2o8_t[:, m:m + 1],
                                    scalar2=None, op0=OP.add)
            dma(out=ar_h2_in[m * 128:(m + 1) * 128, :], in_=hp)
    nc.gpsimd.collective_compute("AllReduce", OP.add, replica_groups=GRP,
                                 ins=[ar_h2_in[:].opt()],
                                 outs=[ar_h2_out[:].opt()])

    # PVP_n (f32r) on the shared n-major chain
    def transpose_nmajor(src_tiles, name, tagbase, dt):
        out = [pa.tile([128, D], dt, name=f"{name}0", tag=f"{tagbase}0"),
               pa.tile([128, D], dt, name=f"{name}1", tag=f"{tagbase}1")]
        for m in range(NT):
            for jb, (j0, jw) in enumerate(N_MT):
                if dt == F32R:
                    ps = ppt_h([128, 128], "ps_tr", F32)
                    nc.tensor.transpose(
                        ps[0:jw, 0:128],
                        src_tiles[m][:, j0:j0 + jw].bitcast(F32), ident_f)
                else:
                    ps = ppt_h([128, 128], "ps_tr", dt)
                    idm = ident_b if dt == BF16 else ident_h
                    nc.tensor.transpose(ps[0:jw, 0:128],
                                        src_tiles[m][:, j0:j0 + jw], idm)
                nc.vector.tensor_copy(
                    out=out[jb][0:jw, m * 128:(m + 1) * 128],
                    in_=ps[0:jw, 0:128])
        return out

    def cos_edge(x_tiles, ksizes, en, rdt, ones_g, rhs_w):
        nkt = len(ksizes)
        pn = ppt_s([1, 256] if rhs_w == 256 else [1, N], f"nrm_{en}")
        for kt, ksz in enumerate(ksizes):
            xq = pt.tile([128, rhs_w], rdt, name="xq", tag="xq")
            nc.vector.tensor_tensor(xq[0:ksz, :], x_tiles[kt][0:ksz, 0:rhs_w],
                                    x_tiles[kt][0:ksz, 0:rhs_w], OP.mult)
            nc.tensor.matmul(pn, ones_g[0:ksz, :], xq[0:ksz, :],
                             start=(kt == 0), stop=(kt == nkt - 1))
        sd = pt.tile([1, N], F32, name="esd", tag="zmu")
        nc.scalar.activation(sd, pn[0:1, 0:N], AF.Sqrt)
        rn_f = pt1.tile([1, 256], F32, name="ern_f", tag="zva")
        nc.vector.memset(rn_f[0:1, N:256], 0.0)
        nc.vector.reciprocal(out=rn_f[0:1, 0:N], in_=sd)
        rn = pt1.tile([1, 256], F32R, name="ern", tag="zAB")
        nc.vector.tensor_copy(out=rn, in_=rn_f)
        prn = ppt_h([128, 256], f"rnb_{en}")
        nc.tensor.matmul(prn, ones1_r, rn, start=True, stop=True)
        rcol = pt.tile([128, 2], F32, name="rc", tag=f"rc_{en}")
        for mt, (i0, iw) in enumerate(N_MT):
            pst = ppt_h([128, 128], "ps_tr3")
            nc.tensor.transpose(pst[0:iw, 0:1], rn_f[0:1, i0:i0 + iw],
                                ident_f[0:1, 0:1])
            nc.vector.tensor_copy(out=rcol[0:iw, mt:mt + 1],
                                  in_=pst[0:iw, 0:1])
        edge = []
        for mt, (i0, iw) in enumerate(N_MT):
            ps = ppt([128, rhs_w], f"ps_{en}")
            for kt, ksz in enumerate(ksizes):
                nc.tensor.matmul(ps[0:iw, :], x_tiles[kt][0:ksz, i0:i0 + iw],
                                 x_tiles[kt][0:ksz, 0:rhs_w],
                                 start=(kt == 0), stop=(kt == nkt - 1))
            s1 = pt.tile([128, N], F32, name="es1", tag="zt1")
            nc.vector.tensor_scalar(out=s1[0:iw, :], in0=ps[0:iw, 0:N],
                                    scalar1=rcol[0:iw, mt:mt + 1],
                                    scalar2=None, op0=OP.mult)
            nc.vector.tensor_tensor(s1[0:iw, :], s1[0:iw, :], prn[0:iw, 0:N],
                                    OP.mult)
            rmx = pt.tile([128, 1], F32, name="ermx", tag="ermx")
            nc.vector.reduce_max(rmx[0:iw, :], s1[0:iw, :], axis=AX.X)
            bia = pt.tile([128, 1], F32, name="ebia", tag="ebia")
            nc.vector.tensor_scalar_mul(bia[0:iw, :], rmx[0:iw, :], -100.0)
            nc.scalar.activation(s1[0:iw, :], s1[0:iw, :], AF.Exp,
                                 bias=bia[0:iw, 0:1], scale=100.0)
            sm = pt.tile([128, 1], F32, name="esm", tag="esm")
            nc.vector.reduce_sum(sm[0:iw, :], s1[0:iw, :], axis=AX.X)
            rr = pt.tile([128, 1], F32, name="err", tag="err")
            nc.vector.reciprocal(out=rr[0:iw, :], in_=sm[0:iw, :])
            ed = pa.tile([128, N], F16, name=f"{en}_{mt}",
                         tag=f"edg_{en}_{mt}")
            nc.vector.tensor_scalar(out=ed[0:iw, :], in0=s1[0:iw, :],
                                    scalar1=rr[0:iw, 0:1], scalar2=None,
                                    op0=OP.mult)
            edge.append(ed)
        edgeT = [pa.tile([128, 256], F16, name=f"{en}T0", tag=f"{en}T0"),
                 pa.tile([128, 256], F16, name=f"{en}T1", tag=f"{en}T1")]
        for jb in range(2):
            nc.vector.memset(edgeT[jb][:].bitcast(F32), 0.0)
        for mt, (i0, iw) in enumerate(N_MT):
            for jb, (j0, jw) in enumerate(N_MT):
                pst = ppt_h([128, 128], "ps_tr4", F16)
                nc.tensor.transpose(pst[0:jw, 0:iw],
                                    edge[mt][0:iw, j0:j0 + jw],
                                    ident_h[0:iw, 0:iw])
                nc.vector.tensor_copy(out=edgeT[jb][0:jw, i0:i0 + iw],
                                      in_=pst[0:jw, 0:iw])
        return edge, edgeT

    ones_b = pa.tile([128, 1], BF16, name="ones_b")
    nc.vector.memset(ones_b, 1.0)

    # ---- AR(h2) shadow: everything independent of h2 ----
    sedge, sedgeT = cos_edge(at_b, S_KT, "se", BF16, ones_b, N)
    SP_n = transpose_nmajor(SP, "SP_n", "nmj", F16)
    sp_f = []
    for m in range(NT):
        psp = pp.tile([128, 256], F32, name="ps_sp", tag="mm")
        for jb, (j0, jw) in enumerate(N_MT):
            nc.tensor.matmul(psp[:, 0:N],
                             SP_n[jb][0:jw, m * 128:(m + 1) * 128],
                             sedgeT[jb][0:jw, 0:N], start=(jb == 0),
                             stop=(jb == 1))
        sf = pa.tile([128, N], F16, name=f"sp_f{m}")
        nc.scalar.copy(out=sf, in_=psp[:, 0:N])
        sp_f.append(sf)
    # zs_a = sps @ snW + snb (the esp part joins after AG(ve))
    zsa = []
    for m2 in range(2):
        ps = ppt([128, 256], "ps_zsa")
        for kt in range(NT):
            nc.tensor.matmul(ps[:, 0:N], snWs_t[kt][:, m2 * 128:(m2 + 1) * 128],
                             sp_f[kt][:, 0:N], start=(kt == 0),
                             stop=(kt == NT - 1))
        za_t = pa.tile([128, N], F32, name=f"zsa{m2}")
        nc.vector.tensor_scalar(out=za_t, in0=ps[:, 0:N],
                                scalar1=snbs_t[:, m2:m2 + 1],
                                scalar2=None, op0=OP.add)
        zsa.append(za_t)

    # ---- znorm: load z (f16 payload), inst-norm, fin(m, z, t1) writes ----
    def znorm(src_dram, fin, zn, tagset):
        zh = []
        for m in range(NT):
            t = psh.tile([128, 256], F16, name=f"zh{zn}_{m}", tag=f"sh{m}")
            dma(out=t[:, 0:N], in_=src_dram[m * 128:(m + 1) * 128, :])
            nc.vector.memset(t[:, N:256].bitcast(F32), 0.0)
            zh.append(t)
        pstat = ppt_s([1, 512], f"st_{zn}")
        for m in range(NT):
            nc.tensor.matmul(pstat[0:1, 0:256], ones_h, zh[m][:, 0:256],
                             start=(m == 0), stop=(m == NT - 1))
        mu = pt1.tile([1, 256], F32R, name="zmu", tag="zmu")
        nc.vector.tensor_scalar_mul(mu, pstat[0:1, 0:256], 1.0 / D)
        pmu = ppt_h([128, 256], f"mub_{zn}")
        nc.tensor.matmul(pmu, ones1_r, mu, start=True, stop=True)
        z = []
        for m in range(NT):
            # centered z in f32r (f16->f32r conversion fused into subtract)
            t = pa.tile([128, 256], F16, name=f"z{zn}_{m}",
                        tag=f"{tagset}{m}")
            nc.vector.memset(t[:, N:256].bitcast(F32), 0.0)
            nc.vector.tensor_tensor(t[:, 0:N], zh[m][:, 0:N], pmu[:, 0:N],
                                    OP.subtract)
            z.append(t)
            zq = pt.tile([128, 256], F16, name="zq", tag="zq")
            nc.vector.tensor_tensor(zq, t, t, OP.mult)
            nc.tensor.matmul(pstat[0:1, 256:512], ones_h, zq[:, 0:256],
                             start=(m == 0), stop=(m == NT - 1))
        va = pt1.tile([1, 256], F32, name="zva", tag="zva")
        nc.vector.tensor_scalar(out=va, in0=pstat[0:1, 256:512],
                                scalar1=1.0 / D, scalar2=EPS,
                                op0=OP.mult, op1=OP.add)
        ta = pt1.tile([1, 256], F32, name="zta", tag="zmu2")
        nc.scalar.activation(ta, va, AF.Sqrt)
        rsf = pt1.tile([1, 256], F32, name="zrsf", tag="znm")
        nc.vector.reciprocal(out=rsf, in_=ta)
        rs = pt1.tile([1, 256], F32R, name="zrs", tag="zrs")
        nc.vector.tensor_copy(out=rs, in_=rsf)
        prr = ppt_h([128, 256], f"rb_{zn}")
        nc.tensor.matmul(prr, ones1_r, rs, start=True, stop=True)
        for m in range(NT):
            t1 = pt.tile([128, N], F32, name="zt1", tag="zt1")
            nc.vector.tensor_tensor(t1, z[m][:, 0:N], prr[:, 0:N], OP.mult)
            fin(m, z, t1)
        return z

    # h2 -> PVP (leaky), in-place in z set "za"
    def fin_pvp(m, z, t1):
        nc.vector.scalar_tensor_tensor(out=z[m][:, 0:N], in0=t1, scalar=0.2,
                                       in1=t1, op0=OP.mult, op1=OP.max)
    if "h2" in dbg:
        nc.sync.dma_start(out=dbg["h2"].ap()[:, :], in_=ar_h2_out[:, :])
    PVP = znorm(ar_h2_out, fin_pvp, "h2", "za")
    dtap16("PVP", PVP)

    PVP_n = transpose_nmajor(PVP, "PVP_n", "nmj", F16)

    vedge, vedgeT = cos_edge(PVP, [128] * NT, "ve", F16, ones_h, 256)
    if "vedge" in dbg:
        for mt, (i0, iw) in enumerate(N_MT):
            dtap("vedge", vedge[mt][0:iw, :], row0=i0, rows=iw)
            dtap("sedge", sedge[mt][0:iw, :], row0=i0, rows=iw)

    # =================================================================
    # P4: UpdateVisualNode -> AllGather(z_v) -> VP2 (in-place set "zb")
    # =================================================================
    vp_f, y_r = [], []
    for m in range(NT):
        pv = pp.tile([128, 256], F32, name="ps_vp", tag="mm")
        pe = pp.tile([128, 256], F32, name="ps_ev", tag="mm")
        for jb, (j0, jw) in enumerate(N_MT):
            nc.tensor.matmul(pv, PVP_n[jb][0:jw, m * 128:(m + 1) * 128],
                             vedgeT[jb][0:jw, :], start=(jb == 0),
                             stop=(jb == 1))
            nc.tensor.matmul(pe, PVP_n[jb][0:jw, m * 128:(m + 1) * 128],
                             sedgeT[jb][0:jw, :], start=(jb == 0),
                             stop=(jb == 1))
        vf = pa.tile([128, N], F32, name=f"vp_f{m}")
        nc.scalar.copy(out=vf, in_=pv[:, 0:N])
        vp_f.append(vf)
        yb = sh1k(f"y_{m}", m)
        nc.vector.memset(yb[:, N:256].bitcast(F32), 0.0)
        nc.vector.tensor_tensor(yb[:, 0:N], vf, pe[:, 0:N], OP.add)
        y_r.append(yb)
    dtap16("ybf", y_r)

    for m2 in range(2):
        ps = ppt([128, 256], "ps_zv")
        for kt in range(NT):
            nc.tensor.matmul(ps, vnWs_t[kt][:, m2 * 128:(m2 + 1) * 128],
                             y_r[kt][:, 0:256], start=(kt == 0),
                             stop=(kt == NT - 1))
        zc = pt.tile([128, N], F16, name="zvc", tag="zouts")
        nc.vector.tensor_scalar(out=zc, in0=ps[:, 0:N],
                                scalar1=vnbs_t[:, m2:m2 + 1],
                                scalar2=None, op0=OP.add)
        dma(out=ag_zv_in[m2 * 128:(m2 + 1) * 128, :], in_=zc)
    nc.gpsimd.collective_compute("AllGather", OP.bypass, replica_groups=GRP,
                                 ins=[ag_zv_in[:].opt()],
                                 outs=[ag_zv_out[:].opt()])
    if "zv" in dbg:
        nc.sync.dma_start(out=dbg["zv"].ap()[:, :], in_=ag_zv_out[:, :])

    VP2_bf = [pa.tile([128, N], BF16, name=f"VP2b{m}", tag=f"VP2b{m}")
              for m in range(NT)]

    def fin_vp2(m, z, t1):
        nc.vector.scalar_tensor_tensor(out=z[m][:, 0:N], in0=t1, scalar=0.0,
                                       in1=vp_f[m], op0=OP.max, op1=OP.add)
        nc.vector.tensor_copy(out=VP2_bf[m], in_=z[m][:, 0:N])
    VP2 = znorm(ag_zv_out, fin_vp2, "zv", "zb")
    dtap16("VP2", VP2_bf)

    # =================================================================
    # P6: UpdateVisualEdge (i-sharded, batched row stats, bf16)
    #   h1e(i,:) per hidden h: A_j + A_i - 2*C_ij with C via W1-tile scaling
    #   instnorm-over-H stats land as psum ROWS (one-hot partition matmuls)
    # =================================================================
    VP2_n = transpose_nmajor(VP2_bf, "VP2_n", "nmj", BF16)
    negx2my = [None] * NT
    for hh in range(2):
        xmyn = pt1.tile([ISH, D // 2], BF16, name=f"xmyn{hh}", tag="xmyn")
        for ch in range(4):
            ps = ppt([ISH, 256], "ps_xmy")
            for jb, (j0, jw) in enumerate(N_MT):
                nc.tensor.matmul(
                    ps, selv_t[jb][0:jw, :],
                    VP2_n[jb][0:jw, hh * 1024 + ch * 256:
                               hh * 1024 + (ch + 1) * 256],
                    start=(jb == 0), stop=(jb == 1))
            nc.vector.tensor_copy(out=xmyn[:, ch * 256:(ch + 1) * 256],
                                  in_=ps)
        if "xmyn" in dbg:
            nc.sync.dma_start(out=dbg["xmyn"].ap()[:, hh * 1024:
                                                   (hh + 1) * 1024],
                              in_=xmyn[:, :])
        for ktl in range(8):
            kt = hh * 8 + ktl
            pst = ppt_h([128, 128], "ps_tr5", BF16)
            nc.tensor.transpose(pst[0:128, 0:ISH],
                                xmyn[:, ktl * 128:(ktl + 1) * 128],
                                ident_b[0:ISH, 0:ISH])
            ng = pa.tile([128, ISH], F32, name=f"negx2my{kt}")
            nc.vector.tensor_scalar_mul(ng, pst[0:128, 0:ISH], -2.0)
            negx2my[kt] = ng
    pA = ppt([128, N], "ps_A")
    pAm = ppt([128, ISH], "ps_Am")
    for m in range(NT):
        xq = pt.tile([128, N], BF16, name="vsq", tag="xq")
        nc.vector.tensor_tensor(xq, VP2_bf[m], VP2_bf[m], OP.mult)
        nc.tensor.matmul(pA, veW1_sl(m), xq[:, :], start=(m == 0),
                         stop=(m == NT - 1))
        xqm = pt.tile([128, ISH], BF16, name="vsqm", tag="vsqm")
        nc.vector.tensor_tensor(xqm, negx2my[m], negx2my[m], OP.mult)
        nc.tensor.matmul(pAm, veW1_sl(m), xqm[:, :], start=(m == 0),
                         stop=(m == NT - 1))
    # A_T_bf = A_j + b1 (bf16);  A_my = A_i (f32, (-2x)^2/4)
    A_T_bf = pa.tile([128, N], BF16, name="A_T_bf", tag="A_T")
    nc.vector.tensor_scalar(out=A_T_bf, in0=pA[:, 0:N],
                            scalar1=veb1_t[:, 0:1], scalar2=None, op0=OP.add)
    A_my = pa.tile([128, ISH], F32, name="A_my")
    nc.vector.tensor_scalar_mul(A_my, pAm, 0.25)
    if "AT" in dbg:
        atf = pt.tile([128, N], F32, name="atf", tag="zt1")
        nc.vector.tensor_copy(out=atf, in_=A_T_bf)
        dtap("AT", atf[0:128, 0:N])
        amf = pt.tile([128, ISH], F32, name="amf", tag="vsqm")
        nc.vector.tensor_scalar(out=amf, in0=A_my, scalar1=veb1_t[:, 0:1],
                                scalar2=None, op0=OP.add)
        dtap("Amy", amf[0:128, :])
    vedge_my = pa.tile([ISH, N], F32, name="vedge_my", tag="edg_se_0")
    psvm = ppt([ISH, N], "ps_vm")
    for mt, (i0, iw) in enumerate(N_MT):
        vb = pt.tile([128, N], BF16, name="vedgb", tag="xq")
        nc.vector.tensor_copy(out=vb[0:iw, :], in_=vedge[mt][0:iw, :])
        nc.tensor.matmul(psvm, selv_t[mt][0:iw, :], vb[0:iw, :],
                         start=(mt == 0), stop=(mt == 1))
    nc.vector.tensor_copy(out=vedge_my, in_=psvm)

    # lhsT row-placement patterns: slice [ISH-1-k : 2*ISH-1-k] has ones (or
    # veW2) in column k, zeros elsewhere -> matmul accumulates into psum row k
    PAT = pa.tile([128, 2 * ISH - 1], BF16, name="PAT")
    nc.vector.memset(PAT, 0.0)
    nc.vector.memset(PAT[:, ISH - 1:ISH], 1.0)
    W2PAT = pa.tile([128, 2 * ISH - 1], BF16, name="W2PAT")
    nc.vector.memset(W2PAT, 0.0)
    nc.vector.tensor_copy(out=W2PAT[:, ISH - 1:ISH], in_=veW2_t[:, 0:1])
    # mean-over-H broadcast as a constant matmul: mu_b = (1/H) ones @ hsb
    ONESM = pa.tile([128, 128], BF16, name="ONESM")
    nc.vector.memset(ONESM, 1.0 / H)

    cur_ps = pps.tile([ISH, 256], F32, name="cur_ps", tag="cur")
    S_ps = ppt_s([ISH, 512], "st_e")
    for ii in range(ISH):
        psC = ppt([128, 256], "ps_C")
        for kt in range(NT):
            w1i = pstr.tile([128, H], BF16, name=f"w1i{kt}",
                            tag=f"w1i{kt % 4}")
            sc = negx2my[kt][:, ii:ii + 1]
            if kt % 3 == 0:
                nc.vector.tensor_scalar(out=w1i, in0=veW1_sl(kt),
                                        scalar1=sc, scalar2=None,
                                        op0=OP.mult)
            elif kt % 3 == 1:
                nc.gpsimd.tensor_scalar(out=w1i, in0=veW1_sl(kt),
                                        scalar1=sc, scalar2=None,
                                        op0=OP.mult)
            else:
                nc.scalar.activation(w1i, veW1_sl(kt), AF.Copy, scale=sc)
            nc.tensor.matmul(psC[:, 0:N], w1i, VP2_bf[kt],
                             start=(kt == 0), stop=(kt == NT - 1))
        # hsb = (-2C + A_i) + (A_j + b1):  Act (psum in) then DVE bf16 2x
        hp_ = pt.tile([128, N], BF16, name="ehp", tag="zt1")
        nc.scalar.activation(hp_, psC[:, 0:N], AF.Identity,
                             bias=A_my[:, ii:ii + 1], scale=1.0)
        hsb = pt.tile([128, N], BF16, name="ehsb", tag=f"hsb{ii % 2}")
        nc.vector.tensor_tensor(hsb, hp_, A_T_bf, OP.add)
        hsq = pt.tile([128, N], BF16, name="ehsq", tag="xq")
        nc.vector.tensor_tensor(hsq, hsb, hsb, OP.mult)
        psel = PAT[:, ISH - 1 - ii:2 * ISH - 1 - ii]
        nc.tensor.matmul(S_ps[0:ISH, 0:N], psel, hsb,
                         start=(ii == 0), stop=(ii == ISH - 1))
        nc.tensor.matmul(S_ps[0:ISH, 256:256 + N], psel, hsq,
                         start=(ii == 0), stop=(ii == ISH - 1))
        # centered h, relu; rs scaling deferred to cur rows (rs>0)
        pm = ppt_h([128, 256], "mu_b")
        nc.tensor.matmul(pm[:, 0:N], ONESM, hsb, start=True, stop=True)
        t1b = pt.tile([128, N], BF16, name="et1b", tag="zt1")
        nc.vector.tensor_tensor(t1b, hsb, pm[:, 0:N], OP.subtract)
        h2b = pt.tile([128, N], BF16, name="eh2b", tag="h2b")
        nc.vector.tensor_scalar_max(h2b, t1b, 0.0)
        nc.tensor.matmul(cur_ps[0:ISH, 0:N],
                         W2PAT[:, ISH - 1 - ii:2 * ISH - 1 - ii], h2b,
                         start=(ii == 0), stop=(ii == ISH - 1))
    # ---- batched row stats -> rs rows; cur = cur_raw * rs ----
    mu = pt1.tile([ISH, N], F32, name="emu", tag="emu")
    nc.vector.tensor_scalar_mul(mu, S_ps[0:ISH, 0:N], 1.0 / H)
    va = pt1.tile([ISH, N], F32, name="eva", tag="eva")
    nc.vector.tensor_scalar_mul(va, S_ps[0:ISH, 256:256 + N], 1.0 / H)
    musq = pt1.tile([ISH, N], F32, name="emusq", tag="emusq")
    nc.vector.tensor_tensor(musq, mu, mu, OP.mult)
    nc.vector.tensor_tensor(va, va, musq, OP.subtract)
    sdv = pt1.tile([ISH, N], F32, name="esdv", tag="emusq")
    nc.scalar.activation(sdv, va, AF.Sqrt, bias=epsc[0:ISH, 0:1], scale=1.0)
    rsr = pt1.tile([ISH, N], F32, name="ersr", tag="eva")
    nc.vector.reciprocal(out=rsr, in_=sdv)
    curm = pa.tile([ISH, N], F32, name="curm", tag="atf2")
    nc.vector.tensor_tensor(curm, cur_ps[0:ISH, 0:N], rsr, OP.mult)
    dtap("cur", curm[:, :])

    # tanh(cur + b2) * (vedge_my + 1e-8) -> softmax(/10) -> my edge rows
    curt = pa.tile([ISH, N], F32, name="curt", tag="atf0")
    nc.scalar.activation(curt, curm, AF.Tanh,
                         bias=veb2_t[0:ISH, 0:1], scale=1.0)
    ne = pa.tile([ISH, N], F32, name="ne", tag="atf1")
    nc.vector.scalar_tensor_tensor(out=ne, in0=vedge_my, scalar=1e-8,
                                   in1=curt, op0=OP.add, op1=OP.mult)
    rmx = pt.tile([ISH, 1], F32, name="vermx", tag="vermx")
    nc.vector.reduce_max(rmx, ne, axis=AX.X)
    bia = pt.tile([ISH, 1], F32, name="vebia", tag="vebia")
    nc.vector.tensor_scalar_mul(bia, rmx, -0.1)
    ex = pt1.tile([ISH, N], F32, name="veex", tag="veex")
    nc.scalar.activation(ex, ne, AF.Exp, bias=bia[0:ISH, 0:1], scale=0.1)
    sm = pt.tile([ISH, 1], F32, name="vesm", tag="vesm")
    nc.vector.reduce_sum(sm, ex, axis=AX.X)
    rr = pt.tile([ISH, 1], F32, name="verr", tag="verr")
    nc.vector.reciprocal(out=rr, in_=sm)
    vemine = pt1.tile([ISH, N], F16, name="vemine", tag="vemine")
    nc.vector.tensor_scalar(out=vemine, in0=ex, scalar1=rr[0:ISH, 0:1],
                            scalar2=None, op0=OP.mult)
    dma(out=ag_ve_in[:, :], in_=vemine)
    nc.gpsimd.collective_compute("AllGather", OP.bypass, replica_groups=GRP,
                                 ins=[ag_ve_in[:].opt()],
                                 outs=[ag_ve_out[:].opt()])

    # ---- AG(ve) shadow: fusion (VP2 half) + img @ VP2^T partials ----
    vu_sb = pt1.tile([1, 512], F32R, name="vu_sb", tag="vu_sb")
    nc.vector.memset(vu_sb[:].bitcast(F32), 0.0)
    pvu0 = ppt_s([1, 256], "ps_vu0")
    for m2 in range(2):
        ps = ppt([128, 256], "ps_fus")
        for kt in range(NT):
            nc.tensor.matmul(ps[:, 0:N],
                             fusWs_t[kt][:, m2 * 128:(m2 + 1) * 128],
                             VP2[kt][:, 0:N], start=(kt == 0),
                             stop=(kt == NT - 1))
        th = pt.tile([128, 256], F16, name="fth", tag="fth")
        nc.scalar.activation(th[:, 0:N], ps[:, 0:N], AF.Tanh)
        nc.tensor.matmul(pvu0[0:1, 0:N], fusUs_t[:, m2:m2 + 1], th[:, 0:N],
                         start=(m2 == 0), stop=(m2 == 1))
    nc.vector.tensor_copy(out=vu_sb[0:1, 0:N], in_=pvu0[0:1, 0:N])
    probv = []
    for bt in range(2):
        ps = ppt([128, N], "ps_prob")
        for kt in range(NT):
            nc.tensor.matmul(ps, imgT_sl(kt, bt), VP2[kt][:, 0:N],
                             start=(kt == 0), stop=(kt == NT - 1))
        pv_t = pa.tile([128, N], F32, name=f"probv{bt}")
        nc.scalar.copy(out=pv_t, in_=ps)
        probv.append(pv_t)
    if "ve2" in dbg:
        nc.sync.dma_start(out=dbg["ve2"].ap()[:, :], in_=ag_ve_out[:, :])
    ve2 = [pt1.tile([128, N], F16, name="ve2_0", tag="ve2_0"),
           pt1.tile([128, N], F16, name="ve2_1", tag="ve2_1")]
    for mt, (i0, iw) in enumerate(N_MT):
        dma(out=ve2[mt][0:iw, :], in_=ag_ve_out[i0:i0 + iw, :])
    ve2T = [pa.tile([128, 256], F16, name="ve2T0", tag="veT0"),
            pa.tile([128, 256], F16, name="ve2T1", tag="veT1")]
    for mt, (i0, iw) in enumerate(N_MT):
        for jb, (j0, jw) in enumerate(N_MT):
            pst = ppt_h([128, 128], "ps_tr6", F16)
            nc.tensor.transpose(pst[0:jw, 0:iw],
                                ve2[mt][0:iw, j0:j0 + jw],
                                ident_h[0:iw, 0:iw])
            nc.vector.tensor_copy(out=ve2T[jb][0:jw, i0:i0 + iw],
                                  in_=pst[0:jw, 0:iw])

    # =================================================================
    # P5: UpdateSemanticNode -> AllGather(z_s) -> SP2 (in-place set "za")
    #   (sps/zs_a were precomputed in the AR(h2) shadow)
    # =================================================================
    SP_n2 = transpose_nmajor(SP, "SP_n2", "nmj", F16)
    y2_r = []
    for m in range(NT):
        pes = pp.tile([128, 256], F32, name="ps_es", tag="mm")
        for jb, (j0, jw) in enumerate(N_MT):
            nc.tensor.matmul(pes[:, 0:N],
                             SP_n2[jb][0:jw, m * 128:(m + 1) * 128],
                             ve2T[jb][0:jw, 0:N], start=(jb == 0),
                             stop=(jb == 1))
        yb = sh1k(f"y2_{m}", m)
        nc.scalar.copy(out=yb[:, 0:N], in_=pes[:, 0:N])
        y2_r.append(yb)

    for m2 in range(2):
        ps = ppt([128, 256], "ps_zs")
        for kt in range(NT):
            nc.tensor.matmul(ps[:, 0:N], snWs_t[kt][:, m2 * 128:(m2 + 1) * 128],
                             y2_r[kt][:, 0:N], start=(kt == 0),
                             stop=(kt == NT - 1))
        zc = pt.tile([128, N], F16, name="zsc", tag="zouts")
        nc.vector.tensor_tensor(zc, ps[:, 0:N], zsa[m2], OP.add)
        dma(out=ag_zs_in[m2 * 128:(m2 + 1) * 128, :], in_=zc)
    nc.gpsimd.collective_compute("AllGather", OP.bypass, replica_groups=GRP,
                                 ins=[ag_zs_in[:].opt()],
                                 outs=[ag_zs_out[:].opt()])

    def fin_sp2(m, z, t1):
        nc.vector.scalar_tensor_tensor(out=z[m][:, 0:N], in0=t1, scalar=0.0,
                                       in1=sp_f[m], op0=OP.max, op1=OP.add)
    SP2 = znorm(ag_zs_out, fin_sp2, "zs", "za")
    if "SP2" in dbg:
        for m in range(NT):
            nc.sync.dma_start(out=dbg["SP2"].ap()[m * 128:(m + 1) * 128, :],
                              in_=SP2[m][:, 0:N])

    # =================================================================
    # P7: fusion (SP2 half) -> AG(vu) || img @ SP2^T -> alpha -> prob
    # =================================================================
    pvu1 = ppt_s([1, 256], "ps_vu1")
    for m2 in range(2):
        ps = ppt([128, 256], "ps_fus")
        for kt in range(NT):
            nc.tensor.matmul(ps[:, 0:N],
                             fusWs_t[kt][:, m2 * 128:(m2 + 1) * 128],
                             SP2[kt][:, 0:N], start=(kt == 0),
                             stop=(kt == NT - 1))
        th = pt.tile([128, 256], F16, name="fth", tag="fth")
        nc.scalar.activation(th[:, 0:N], ps[:, 0:N], AF.Tanh)
        nc.tensor.matmul(pvu1[0:1, 0:N], fusUs_t[:, m2:m2 + 1], th[:, 0:N],
                         start=(m2 == 0), stop=(m2 == 1))
    nc.vector.tensor_copy(out=vu_sb[0:1, 256:256 + N], in_=pvu1[0:1, 0:N])
    dma(out=ag_vu_in[:, :], in_=vu_sb)
    nc.gpsimd.collective_compute("AllGather", OP.bypass, replica_groups=GRP,
                                 ins=[ag_vu_in[:].opt()],
                                 outs=[ag_vu_out[:].opt()])
    # AG(vu) shadow: img @ SP2^T partials
    probs = []
    for bt in range(2):
        ps = ppt([128, N], "ps_prob")
        for kt in range(NT):
            nc.tensor.matmul(ps, imgT_sl(kt, bt), SP2[kt][:, 0:N],
                             start=(kt == 0), stop=(kt == NT - 1))
        ps_t = pa.tile([128, N], F32, name=f"probs{bt}")
        nc.scalar.copy(out=ps_t, in_=ps)
        probs.append(ps_t)
    vus = pt1.tile([NCORES, 512], F32R, name="vus", tag="vu_sb")
    dma(out=vus, in_=ag_vu_out[:, :])
    pvk = ppt_s([1, 512], "ps_vuk")
    for k in range(2):
        nc.tensor.matmul(pvk[0:1, 256 * k:256 * k + 256],
                         ones8_r, vus[:, 256 * k:256 * k + 256],
                         start=True, stop=True)
    vuf = pt1.tile([1, 512], F32, name="vuf", tag="zstt")
    nc.vector.tensor_copy(out=vuf, in_=pvk)
    dtap("vuf", vuf[:, :])
    mx = pt.tile([1, N], F32, name="amx", tag="amx")
    nc.vector.tensor_tensor(mx, vuf[0:1, 0:N], vuf[0:1, 256:256 + N], OP.max)
    dv = pt1.tile([1, 512], F32R, name="adv", tag="adv")
    nc.vector.memset(dv[:].bitcast(F32), 0.0)
    for k in range(2):
        nc.vector.tensor_tensor(dv[0:1, 256 * k:256 * k + N],
                                vuf[0:1, 256 * k:256 * k + N], mx, OP.subtract)
    nc.scalar.activation(dv, dv, AF.Exp, scale=100.0)
    ssum = pt.tile([1, N], F32, name="assum", tag="assum")
    nc.vector.tensor_tensor(ssum, dv[0:1, 0:N], dv[0:1, 256:256 + N], OP.add)
    rsu = pt.tile([1, N], F32, name="arsu", tag="arsu")
    nc.vector.reciprocal(out=rsu, in_=ssum)
    for k in range(2):
        nc.vector.tensor_tensor(dv[0:1, 256 * k:256 * k + N],
                                dv[0:1, 256 * k:256 * k + N], rsu, OP.mult)
    alro = dv
    if "alpha" in dbg:
        al_f = pt1.tile([1, 512], F32, name="al_f", tag="zstt")
        nc.vector.tensor_copy(out=al_f, in_=alro)
        dtap("alpha", al_f[:, :])
    pal = ppt_h([128, 512], "ab_al")
    nc.tensor.matmul(pal, ones1_r, alro, start=True, stop=True)
    # prob = alpha_v * (img@VP2^T) + alpha_s * (img@SP2^T)
    for bt in range(2):
        t1 = pt.tile([128, N], F32, name="pr1", tag="zouts")
        nc.vector.tensor_tensor(t1, probv[bt], pal[:, 0:N], OP.mult)
        t2 = pt.tile([128, N], F32, name="pr2", tag="zt1")
        nc.vector.tensor_tensor(t2, probs[bt], pal[:, 256:256 + N], OP.mult)
        t3 = pt.tile([128, N], F32, name="probf", tag="zouts")
        nc.vector.tensor_tensor(t3, t1, t2, OP.add)
        dma(out=prob_out.ap()[bt * 128:(bt + 1) * 128, :], in_=t3)


# =====================================================================
# Host side
# =====================================================================
def _prep_inputs(inputs):
    bf = ml_dtypes.bfloat16
    f16 = np.float16
    f32 = np.float32
    att = np.asarray(inputs["attribute"], f32)
    cen = np.asarray(inputs["centers"], f32)
    expW = np.asarray(inputs["expert_W"], f32)
    expB = np.asarray(inputs["expert_b"], f32)
    w1 = np.asarray(inputs["s2v_W1"], f32)
    w2 = np.asarray(inputs["s2v_W2"], f32)
    in_maps = []
    for c in range(NCORES):
        cs = slice(c * DSH, (c + 1) * DSH)
        isl = slice(c * ISH, (c + 1) * ISH)
        bs = slice(c * BSH, (c + 1) * BSH)
        selv = np.zeros((N, ISH), f32)
        selv[np.arange(c * ISH, (c + 1) * ISH), np.arange(ISH)] = 1.0
        m = {
            "attrT": np.ascontiguousarray(att.T).astype(f16),
            "attrTb": np.ascontiguousarray(att.T).astype(bf),
            "centT": np.ascontiguousarray(cen.T),
            "expW": expW.astype(f16),
            "expBT": np.ascontiguousarray(expB.T),
            "w1s": np.ascontiguousarray(w1[:, cs]).astype(f16),
            "bnG": np.ascontiguousarray(np.asarray(inputs["bn_g"], f32)[cs].reshape(2, 128).T),
            "bnB": np.ascontiguousarray(np.asarray(inputs["bn_b"], f32)[cs].reshape(2, 128).T),
            "w2s": np.ascontiguousarray(w2[cs, :]).astype(f16),
            "b2o8": np.ascontiguousarray((np.asarray(inputs["s2v_b2"], f32) / NCORES).reshape(NT, 128).T),
            "vnWs": np.ascontiguousarray(np.asarray(inputs["vn_W"], f32)[:, cs]).astype(f16),
            "vnbs": np.ascontiguousarray(np.asarray(inputs["vn_b"], f32)[cs].reshape(2, 128).T),
            "snWs": np.ascontiguousarray(np.asarray(inputs["sn_W"], f32)[:, cs]).astype(f16),
            "snbs": np.ascontiguousarray(np.asarray(inputs["sn_b"], f32)[cs].reshape(2, 128).T),
            "veW1": np.asarray(inputs["ve_W1"], f32).astype(bf),
            "veb1": np.asarray(inputs["ve_b1"], f32)[:, None],
            "veW2": np.asarray(inputs["ve_W2"], f32).astype(bf),
            "veb2": np.full((ISH, 1), float(np.asarray(inputs["ve_b2"])[0]),
                            f32),
            "fusWs": np.ascontiguousarray(np.asarray(inputs["fus_W"], f32)[:, cs]).astype(f16),
            "fusUs": np.ascontiguousarray(np.asarray(inputs["fus_u"], f32)[cs, 0].reshape(2, 128).T).astype(f16),
            "imgT": np.ascontiguousarray(
                np.asarray(inputs["img_feat"], f32)[bs, :].T).astype(f16),
            "selv": selv.astype(bf),
        }
        in_maps.append(m)
    return in_maps


def kernel(**inputs):
    global _BUILT
    if _BUILT is None:
        _BUILT = build()
    nc = _BUILT
    in_maps = _prep_inputs(inputs)
    res = run_bass_kernel_spmd(nc, in_maps, core_ids=list(range(NCORES)))
    out = np.concatenate([res.results[c]["prob"] for c in range(NCORES)],
                         axis=0)
    return out.astype(np.float32)


def kernel_debug(**inputs):
    nc = build(debug=True)
    in_maps = _prep_inputs(inputs)
    res = run_bass_kernel_spmd(nc, in_maps, core_ids=list(range(NCORES)))
    out = np.concatenate([res.results[c]["prob"] for c in range(NCORES)],
                         axis=0)
    return out.astype(np.float32), res.results


if __name__ == "__main__":
    import reference
    inp = {k: np.asarray(v) for k, v in reference.setup_inputs().items()}
    got = kernel(**inp)
    exp = np.asarray(reference.reference(**reference.setup_inputs()))
    err = np.abs(got - exp).max() / (np.abs(exp).max() + 1e-9)
    print("Relative error:", err)



# revision 46
# speedup vs baseline: 1.4541x; 1.0759x over previous
# Trainium2 Bass kernel for nn_CPPN (gnn_message_passing), 8-core SPMD.
#
# Sharding:
#   - Node-MLP weights (s2v_W1/W2, vn_W, sn_W, fus_W) sharded over the 2048
#     col/row dim (256 per core); stitched with one AllReduce (h2) and three
#     AllGathers (z_v, z_s, vu partials).
#   - Visual edge MLP (200x200 pairwise rows) sharded over i: 25 rows/core,
#     AllGather of the resulting edge rows.  Per-core column selection is via
#     a host-supplied one-hot matrix (SPMD program is identical on all cores;
#     only input data differs).
#   - img_feat batch sharded 256 rows/core for the final prob matmul; host
#     concatenates per-core outputs.
# Layout: activations are d-major [D on partitions (16x128), 200 on free].
# Heavy matmuls bf16; stats/broadcast matmuls f32r.  The reference's second
# edge_update (semantic) is dead code w.r.t. prob and is skipped.

import sys

sys.path.insert(0, "/opt/trn_rl_repo")

import numpy as np
import ml_dtypes

import concourse.bass as bass
import concourse.bacc as bacc
import concourse.tile as tile
from concourse import mybir
from concourse.bass_utils import run_bass_kernel_spmd
from concourse.masks import make_identity

F32 = mybir.dt.float32
F32R = mybir.dt.float32r
BF16 = mybir.dt.bfloat16
F16 = mybir.dt.float16
AF = mybir.ActivationFunctionType
OP = mybir.AluOpType
AX = mybir.AxisListType

NCORES = 8
N = 200
S = 312
D = 2048
H = 128
KEXP = 3
B = 2048
DSH = D // NCORES      # 256
ISH = N // NCORES      # 25
BSH = B // NCORES      # 256
EPS = 1e-5
NT = D // 128          # 16
S_KT = [128, 128, 56]
N_MT = ((0, 128), (128, 72))
G_I = 2                # i-group size for edge t-gen batching

_BUILT = None


def _rep(ap_src, dims):
    """Rebuild AP with explicit free dims [[step,count],...] after partition."""
    return bass.AP(tensor=ap_src.tensor, offset=ap_src.offset,
                   ap=[ap_src.ap[0]] + dims)


def build(debug=False):
    nc = bacc.Bacc("TRN2", target_bir_lowering=False, debug=False,
                   num_devices=NCORES)
    d = {}

    def din(name, shape, dt):
        d[name] = nc.dram_tensor(name, shape, dt, kind="ExternalInput")

    din("attrT", [S, N], F16)
    din("attrTb", [S, N], BF16)
    din("centT", [S, KEXP], F32)
    din("expW", [KEXP, S, D], F16)
    din("expBT", [D, KEXP], F32)
    din("w1s", [D, DSH], F16)
    din("bnG", [128, 2], F32)
    din("bnB", [128, 2], F32)
    din("w2s", [DSH, D], F16)
    din("b2o8", [128, NT], F32)
    din("vnWs", [D, DSH], F16)
    din("vnbs", [128, 2], F32)
    din("snWs", [D, DSH], F16)
    din("snbs", [128, 2], F32)
    din("veW1", [D, H], BF16)
    din("veb1", [H, 1], F32)
    din("veW2", [H, 1], BF16)
    din("veb2", [ISH, 1], F32)
    din("fusWs", [D, DSH], F16)
    din("fusUs", [128, 2], F16)
    din("imgT", [D, BSH], F16)
    din("selv", [N, ISH], BF16)
    prob_out = nc.dram_tensor("prob", [BSH, N], F32, kind="ExternalOutput")
    dbg = {}
    if debug:
        def dout(name, shape, dt):
            dbg[name] = nc.dram_tensor("dbg_" + name, shape, dt,
                                       kind="ExternalOutput")
        dout("SP", [D, N], F16)
        dout("a1", [2 * 128, N], F16)
        dout("h2", [D, N], F16)
        dout("PVP", [D, N], F16)
        dout("vedge", [N, N], F16)
        dout("sedge", [N, N], F16)
        dout("ybf", [D, N], F16)
        dout("zv", [D, N], F16)
        dout("VP2", [D, N], BF16)
        dout("AT", [128, N], F32)
        dout("Amy", [128, ISH], F32)
        dout("xmyn", [ISH, D], BF16)
        dout("cur", [ISH, N], F32)
        dout("ve2", [N, N], F16)
        dout("SP2", [D, N], F16)
        dout("vuf", [1, 512], F32)
        dout("alpha", [1, 512], F32)

    with tile.TileContext(nc) as tc:
        import contextlib
        with contextlib.ExitStack() as ctx, \
                nc.allow_low_precision(reason="bf16 PE transposes (no accum)"):
            _emit(ctx, nc, tc, d, prob_out, dbg)
    nc.compile()
    return nc


def _emit(ctx, nc, tc, d, prob_out, dbg=None):
    dbg = dbg or {}

    def dtap16(key, tiles):     # 16 d-major tiles -> [D, N] dram
        if key in dbg:
            for m in range(NT):
                nc.sync.dma_start(out=dbg[key].ap()[m * 128:(m + 1) * 128, :],
                                  in_=tiles[m][:, 0:N])

    def dtap(key, ap_in, row0=0, rows=None):
        if key in dbg:
            o = dbg[key].ap()
            nc.sync.dma_start(out=o[row0:row0 + (rows or o.shape[0]), :],
                              in_=ap_in)

    pw = ctx.enter_context(tc.tile_pool(name="wts", bufs=1))
    pa = ctx.enter_context(tc.tile_pool(name="acts", bufs=1))
    pt = ctx.enter_context(tc.tile_pool(name="tmp", bufs=2))
    pt1 = ctx.enter_context(tc.tile_pool(name="rows", bufs=1))
    psh = ctx.enter_context(tc.tile_pool(name="sh1k", bufs=1))
    pstr = ctx.enter_context(tc.tile_pool(name="stream", bufs=1))
    pdram = ctx.enter_context(tc.tile_pool(name="dram", bufs=1, space="DRAM"))
    pp = ctx.enter_context(tc.tile_pool(name="ps_mm", bufs=3, space="PSUM"))
    pph = ctx.enter_context(tc.tile_pool(name="ps_hold", bufs=2, space="PSUM"))
    pps = ctx.enter_context(tc.tile_pool(name="ps_st", bufs=1, space="PSUM"))

    dma = nc.sync.dma_start
    GRP = [list(range(NCORES))]

    def ppt(shape, name, dt=F32):
        return pp.tile(shape, dt, name=name, tag="mm")

    def ppt_h(shape, name, dt=F32):
        return pph.tile(shape, dt, name=name, tag="hold")

    def ppt_s(shape, name, dt=F32):
        return pps.tile(shape, dt, name=name, tag="stat")

    def sh1k(name, tagid, dt=F16, shape=None):
        return psh.tile(shape or [128, 256], dt, name=name, tag=f"sh{tagid}")

    # ---------- constants ----------
    ident_b = pa.tile([128, 128], BF16, name="ident_b")
    make_identity(nc, ident_b)
    ident_f = pt1.tile([128, 128], F32, name="ident_f", tag="ident_f")
    make_identity(nc, ident_f)
    ident_h = pa.tile([128, 128], F16, name="ident_h")
    nc.vector.tensor_copy(out=ident_h, in_=ident_f)

    ones_r = pa.tile([128, 1], F32R, name="ones_r")
    nc.vector.memset(ones_r[:].bitcast(F32), 1.0)
    ones_h = pa.tile([128, 1], F16, name="ones_h")
    nc.vector.memset(ones_h, 1.0)
    ones1_r = pa.tile([1, 128], F32R, name="ones1_r")
    nc.vector.memset(ones1_r[:].bitcast(F32), 1.0)

    ones8_r = pa.tile([8, 1], F32R, name="ones8_r")
    nc.vector.memset(ones8_r[:].bitcast(F32), 1.0)
    epsc = pa.tile([128, 1], F32, name="epsc")
    nc.vector.memset(epsc, EPS)

    # ---------- inputs -> SBUF ----------
    at_f, at_b = [], []
    off = 0
    for kt, ksz in enumerate(S_KT):
        tf = psh.tile([128, N], F16, name=f"at_f{kt}", tag=f"sh{9 + kt}")
        tb = pa.tile([128, N], BF16, name=f"at_b{kt}")
        dma(out=tf[0:ksz, :], in_=d["attrT"].ap()[off:off + ksz, :])
        dma(out=tb[0:ksz, :], in_=d["attrTb"].ap()[off:off + ksz, :])
        at_f.append(tf)
        at_b.append(tb)
        off += ksz
    centT_t = []
    off = 0
    for kt, ksz in enumerate(S_KT):
        t = pa.tile([128, KEXP], F32, name=f"centT{kt}")
        dma(out=t[0:ksz, :], in_=d["centT"].ap()[off:off + ksz, :])
        centT_t.append(t)
        off += ksz
    expbt_a = pw.tile([128, NT * KEXP], F32, name="expbt_a")
    dma(out=expbt_a[:].rearrange("p (m k) -> p m k", m=NT),
        in_=d["expBT"].ap().rearrange("(m p) k -> p m k", p=128))

    def expbt_sl(m, k):
        return expbt_a[:, m * KEXP + k:m * KEXP + k + 1]

    def load4(name, key, tagbase=None):
        """[D, DSH] weight as 4 tiles [128, 4*DSH]; one DMA per tile."""
        out = []
        for g in range(4):
            tg = f"{tagbase}{g}" if tagbase else ""
            t = pw.tile([128, 4 * DSH], F16, name=f"{name}{g}", tag=tg)
            dma(out=t[:].rearrange("p (kt c) -> p kt c", kt=4),
                in_=d[key].ap()[g * 512:(g + 1) * 512, :].rearrange(
                    "(kt p) c -> p kt c", p=128))
            out.append(t)
        return out

    def sl4(tiles):
        def f(kt, c0, c1):
            return tiles[kt // 4][:, (kt % 4) * DSH + c0:(kt % 4) * DSH + c1]
        return f

    w1s_4 = load4("w1s", "w1s")
    w1s_sl = sl4(w1s_4)
    vnWs_4 = load4("vnWs", "vnWs")
    vnWs_sl = sl4(vnWs_4)
    snWs_4 = load4("snWs", "snWs")
    snWs_sl = sl4(snWs_4)

    def late_loads():
        """Weights only needed after AR(h2): emit their DMAs in its shadow."""
        g = {}
        fusWs_4 = load4("fusWs", "fusWs", tagbase="w1s")
        g["fusWs_sl"] = sl4(fusWs_4)
        imgT_p = []
        for hh in range(2):
            t = pw.tile([128, 8 * BSH], F16, name=f"imgTp{hh}")
            src_ap = d["imgT"].ap()[hh * 1024:(hh + 1) * 1024, :].rearrange(
                "(kt p) b -> p kt b", p=128)
            nc.sync.dma_start(out=t[:].rearrange("p (kt b) -> p kt b", kt=8),
                              in_=src_ap)
            imgT_p.append(t)
        g["imgT_p"] = imgT_p
        veW1_a = pw.tile([128, D], BF16, name="veW1_a")
        nc.sync.dma_start(
            out=veW1_a[:].rearrange("p (kt h) -> p kt h", kt=NT),
            in_=d["veW1"].ap().rearrange("(kt p) h -> p kt h", p=128))
        g["veW1_a"] = veW1_a
        selv_t = []
        for jb, (j0, jw) in enumerate(N_MT):
            t = pw.tile([128, ISH], BF16, name=f"selv{jb}")
            dma(out=t[0:jw, :], in_=d["selv"].ap()[j0:j0 + jw, :])
            selv_t.append(t)
        g["selv_t"] = selv_t
        return g

    def imgT_sl(kt, bt):
        return LATE["imgT_p"][kt // 8][:, (kt % 8) * BSH + bt * 128:
                                       (kt % 8) * BSH + (bt + 1) * 128]

    def veW1_sl(kt):
        return LATE["veW1_a"][:, kt * H:(kt + 1) * H]

    def loadmat(name, key, rows, cols, dt=F32):
        t = pw.tile([rows, cols], dt, name=name)
        dma(out=t, in_=d[key].ap()[:, :])
        return t

    bnG_t = loadmat("bnG_t", "bnG", 128, 2)
    bnB_t = loadmat("bnB_t", "bnB", 128, 2)
    vnbs_t = loadmat("vnbs_t", "vnbs", 128, 2)
    snbs_t = loadmat("snbs_t", "snbs", 128, 2)
    veb1_t = loadmat("veb1_t", "veb1", H, 1)
    veb2_t = loadmat("veb2_t", "veb2", ISH, 1)
    b2o8_t = loadmat("b2o8_t", "b2o8", 128, NT)
    veW2_t = loadmat("veW2_t", "veW2", H, 1, BF16)
    fusUs_t = loadmat("fusUs_t", "fusUs", 128, 2, F16)

    # collective bounce buffers (fp16 payloads: 10-bit mantissa is enough —
    # verified against reference; halves on-wire bytes)
    ar_h2_in = pdram.tile([D, N], F16, name="ar_h2_in")
    ar_h2_out = pdram.tile([D, N], F16, addr_space="Shared", name="ar_h2_out")
    ag_zv_in = pdram.tile([DSH, N], F16, name="ag_zv_in")
    ag_zv_out = pdram.tile([D, N], F16, addr_space="Shared", name="ag_zv_out")
    ag_zs_in = pdram.tile([DSH, N], F16, name="ag_zs_in")
    ag_zs_out = pdram.tile([D, N], F16, addr_space="Shared", name="ag_zs_out")
    ag_ve_in = pdram.tile([ISH, N], F16, name="ag_ve_in")
    ag_ve_out = pdram.tile([N, N], F16, addr_space="Shared", name="ag_ve_out")
    ag_vu_in = pdram.tile([1, 512], F32R, name="ag_vu_in")
    ag_vu_out = pdram.tile([NCORES, 512], F32R, addr_space="Shared",
                           name="ag_vu_out")

    # =================================================================
    # P1: CooperationModule -> SP (f32r padded, d-major)
    # =================================================================
    offT = []
    for k in range(KEXP):
        row = []
        for kt, ksz in enumerate(S_KT):
            t = sh1k(f"offT{k}_{kt}", 3 * k + kt)
            nc.vector.memset(t[:, N:256].bitcast(F32), 0.0)
            nc.vector.tensor_scalar(
                out=t[0:ksz, 0:N], in0=at_f[kt][0:ksz, :],
                scalar1=centT_t[kt][0:ksz, k:k + 1], scalar2=None,
                op0=OP.subtract)
            row.append(t)
        offT.append(row)

    SP = [pa.tile([128, 256], F16, name=f"SP{m}") for m in range(NT)]
    for m in range(NT):
        nc.vector.memset(SP[m][:, N:256].bitcast(F32), 0.0)
    for k in range(KEXP):
        for half in range(4):
            wst = []
            for kt, ksz in enumerate(S_KT):
                w = pstr.tile([128, D // 4], F16, name=f"expw_st{kt}",
                              tag=f"str{kt}")
                dma(out=w[0:ksz, :],
                    in_=d["expW"].ap()[k, sum(S_KT[:kt]):sum(S_KT[:kt]) + ksz,
                                       half * 512:(half + 1) * 512])
                wst.append(w)
            for mh in range(4):
                m = half * 4 + mh
                ps = ppt([128, 256], "ps_exp")
                for kt, ksz in enumerate(S_KT):
                    nc.tensor.matmul(ps, wst[kt][0:ksz, mh * 128:(mh + 1) * 128],
                                     offT[k][kt][0:ksz, 0:256],
                                     start=(kt == 0),
                                     stop=(kt == len(S_KT) - 1))
                if k == 0:
                    nc.vector.tensor_scalar(out=SP[m][:, 0:N], in0=ps[:, 0:N],
                                            scalar1=expbt_sl(m, 0),
                                            scalar2=0.0,
                                            op0=OP.add, op1=OP.max)
                else:
                    r1 = pt1.tile([128, N], F16, name="exr", tag="exr")
                    nc.scalar.activation(r1, ps[:, 0:N], AF.Relu,
                                         bias=expbt_sl(m, k), scale=1.0)
                    nc.vector.tensor_tensor(SP[m][:, 0:N], SP[m][:, 0:N], r1,
                                            OP.add)
    dtap16("SP", SP)

    # =================================================================
    # P2: semantic2visual -> AllReduce(h2)
    # =================================================================
    a1 = []
    for m2 in range(2):
        ps = ppt([128, 256], "ps_h1")
        for kt in range(NT):
            nc.tensor.matmul(ps[:, 0:N], w1s_sl(kt, m2 * 128, (m2 + 1) * 128),
                             SP[kt][:, 0:N], start=(kt == 0),
                             stop=(kt == NT - 1))
        st6 = pt.tile([128, 6], F32, name="bn_st", tag="bn_st")
        mv = pt.tile([128, 2], F32, name="bn_mv", tag="bn_mv")
        nc.vector.bn_stats(out=st6, in_=ps[:, 0:N])
        nc.vector.bn_aggr(out=mv, in_=st6)
        sd = pt.tile([128, 1], F32, name="bn_sd", tag="bn_sd")
        nc.scalar.activation(sd, mv[:, 1:2], AF.Sqrt, bias=epsc[:, 0:1],
                             scale=1.0)
        rs = pt.tile([128, 1], F32, name="bn_rs", tag="bn_rs")
        nc.vector.reciprocal(out=rs, in_=sd)
        Av = pt.tile([128, 1], F32, name="bn_A", tag="bn_A")
        nc.vector.tensor_tensor(Av, rs, bnG_t[:, m2:m2 + 1], OP.mult)
        Bt = pt.tile([128, 1], F32, name="bn_Bt", tag="bn_Bt")
        nc.vector.tensor_tensor(Bt, mv[:, 0:1], Av, OP.mult)
        Bv = pt.tile([128, 1], F32, name="bn_Bv", tag="bn_Bv")
        nc.vector.tensor_tensor(Bv, bnB_t[:, m2:m2 + 1], Bt, OP.subtract)
        t1 = pt.tile([128, N], F32, name="h1_t1", tag="zt1")
        nc.vector.tensor_scalar(out=t1, in0=ps[:, 0:N], scalar1=Av,
                                scalar2=Bv, op0=OP.mult, op1=OP.add)
        a1m = pa.tile([128, 256], F16, name=f"a1_{m2}")
        nc.vector.memset(a1m[:, N:256].bitcast(F32), 0.0)
        nc.vector.scalar_tensor_tensor(out=a1m[:, 0:N], in0=t1, scalar=0.2,
                                       in1=t1, op0=OP.mult, op1=OP.max)
        dtap("a1", a1m[:, 0:N], row0=m2 * 128, rows=128)
        a1.append(a1m)

    for half in range(4):
        w2st = []
        for kt in range(2):
            w = pstr.tile([128, D // 4], F16, name=f"w2_st{kt}",
                          tag=f"str{kt}")
            dma(out=w, in_=d["w2s"].ap()[kt * 128:(kt + 1) * 128,
                                         half * 512:(half + 1) * 512])
            w2st.append(w)
        for mh in range(4):
            m = half * 4 + mh
            ps = ppt([128, 256], "ps_h2")
            for kt2 in range(2):
                nc.tensor.matmul(ps, w2st[kt2][:, mh * 128:(mh + 1) * 128],
                                 a1[kt2][:, 0:256], start=(kt2 == 0),
                                 stop=(kt2 == 1))
            hp = pt.tile([128, N], F16, name="h2p", tag="zouts")
            nc.vector.tensor_scalar(out=hp, in0=ps[:, 0:N],
                                    scalar1=b2o8_t[:, m:m + 1],
                                    scalar2=None, op0=OP.add)
            dma(out=ar_h2_in[m * 128:(m + 1) * 128, :], in_=hp)
    nc.gpsimd.collective_compute("AllReduce", OP.add, replica_groups=GRP,
                                 ins=[ar_h2_in[:].opt()],
                                 outs=[ar_h2_out[:].opt()])

    # PVP_n (f32r) on the shared n-major chain
    def transpose_nmajor(src_tiles, name, tagbase, dt):
        out = [pa.tile([128, D], dt, name=f"{name}0", tag=f"{tagbase}0"),
               pa.tile([128, D], dt, name=f"{name}1", tag=f"{tagbase}1")]
        for m in range(NT):
            for jb, (j0, jw) in enumerate(N_MT):
                if dt == F32R:
                    ps = ppt_h([128, 128], "ps_tr", F32)
                    nc.tensor.transpose(
                        ps[0:jw, 0:128],
                        src_tiles[m][:, j0:j0 + jw].bitcast(F32), ident_f)
                else:
                    ps = ppt_h([128, 128], "ps_tr", dt)
                    idm = ident_b if dt == BF16 else ident_h
                    nc.tensor.transpose(ps[0:jw, 0:128],
                                        src_tiles[m][:, j0:j0 + jw], idm)
                nc.vector.tensor_copy(
                    out=out[jb][0:jw, m * 128:(m + 1) * 128],
                    in_=ps[0:jw, 0:128])
        return out

    def cos_edge(x_tiles, ksizes, en, rdt, ones_g, rhs_w):
        nkt = len(ksizes)
        pn = ppt_s([1, 256] if rhs_w == 256 else [1, N], f"nrm_{en}")
        for kt, ksz in enumerate(ksizes):
            xq = pt.tile([128, rhs_w], rdt, name="xq", tag="xq")
            nc.vector.tensor_tensor(xq[0:ksz, :], x_tiles[kt][0:ksz, 0:rhs_w],
                                    x_tiles[kt][0:ksz, 0:rhs_w], OP.mult)
            nc.tensor.matmul(pn, ones_g[0:ksz, :], xq[0:ksz, :],
                             start=(kt == 0), stop=(kt == nkt - 1))
        sd = pt.tile([1, N], F32, name="esd", tag="zmu")
        nc.scalar.activation(sd, pn[0:1, 0:N], AF.Sqrt)
        rn_f = pt1.tile([1, 256], F32, name="ern_f", tag="zva")
        nc.vector.memset(rn_f[0:1, N:256], 0.0)
        nc.vector.reciprocal(out=rn_f[0:1, 0:N], in_=sd)
        rn = pt1.tile([1, 256], F32R, name="ern", tag="zAB")
        nc.vector.tensor_copy(out=rn, in_=rn_f)
        prn = ppt_h([128, 256], f"rnb_{en}")
        nc.tensor.matmul(prn, ones1_r, rn, start=True, stop=True)
        rcol = pt.tile([128, 2], F32, name="rc", tag=f"rc_{en}")
        for mt, (i0, iw) in enumerate(N_MT):
            pst = ppt_h([128, 128], "ps_tr3")
            nc.tensor.transpose(pst[0:iw, 0:1], rn_f[0:1, i0:i0 + iw],
                                ident_f[0:1, 0:1])
            nc.vector.tensor_copy(out=rcol[0:iw, mt:mt + 1],
                                  in_=pst[0:iw, 0:1])
        edge = []
        for mt, (i0, iw) in enumerate(N_MT):
            ps = ppt([128, rhs_w], f"ps_{en}")
            for kt, ksz in enumerate(ksizes):
                nc.tensor.matmul(ps[0:iw, :], x_tiles[kt][0:ksz, i0:i0 + iw],
                                 x_tiles[kt][0:ksz, 0:rhs_w],
                                 start=(kt == 0), stop=(kt == nkt - 1))
            s1 = pt.tile([128, N], F32, name="es1", tag="zt1")
            nc.vector.tensor_scalar(out=s1[0:iw, :], in0=ps[0:iw, 0:N],
                                    scalar1=rcol[0:iw, mt:mt + 1],
                                    scalar2=None, op0=OP.mult)
            nc.vector.tensor_tensor(s1[0:iw, :], s1[0:iw, :], prn[0:iw, 0:N],
                                    OP.mult)
            rmx = pt.tile([128, 1], F32, name="ermx", tag="ermx")
            nc.vector.reduce_max(rmx[0:iw, :], s1[0:iw, :], axis=AX.X)
            bia = pt.tile([128, 1], F32, name="ebia", tag="ebia")
            nc.vector.tensor_scalar_mul(bia[0:iw, :], rmx[0:iw, :], -100.0)
            nc.scalar.activation(s1[0:iw, :], s1[0:iw, :], AF.Exp,
                                 bias=bia[0:iw, 0:1], scale=100.0)
            sm = pt.tile([128, 1], F32, name="esm", tag="esm")
            nc.vector.reduce_sum(sm[0:iw, :], s1[0:iw, :], axis=AX.X)
            rr = pt.tile([128, 1], F32, name="err", tag="err")
            nc.vector.reciprocal(out=rr[0:iw, :], in_=sm[0:iw, :])
            ed = pa.tile([128, N], F16, name=f"{en}_{mt}",
                         tag=f"edg_{en}_{mt}")
            nc.vector.tensor_scalar(out=ed[0:iw, :], in0=s1[0:iw, :],
                                    scalar1=rr[0:iw, 0:1], scalar2=None,
                                    op0=OP.mult)
            edge.append(ed)
        edgeT = [pa.tile([128, 256], F16, name=f"{en}T0", tag=f"{en}T0"),
                 pa.tile([128, 256], F16, name=f"{en}T1", tag=f"{en}T1")]
        for jb in range(2):
            nc.vector.memset(edgeT[jb][:].bitcast(F32), 0.0)
        for mt, (i0, iw) in enumerate(N_MT):
            for jb, (j0, jw) in enumerate(N_MT):
                pst = ppt_h([128, 128], "ps_tr4", F16)
                nc.tensor.transpose(pst[0:jw, 0:iw],
                                    edge[mt][0:iw, j0:j0 + jw],
                                    ident_h[0:iw, 0:iw])
                nc.vector.tensor_copy(out=edgeT[jb][0:jw, i0:i0 + iw],
                                      in_=pst[0:jw, 0:iw])
        return edge, edgeT

    ones_b = pa.tile([128, 1], BF16, name="ones_b")
    nc.vector.memset(ones_b, 1.0)

    # ---- AR(h2) shadow: everything independent of h2 ----
    LATE = late_loads()
    sedge, sedgeT = cos_edge(at_b, S_KT, "se", BF16, ones_b, N)
    SP_n = transpose_nmajor(SP, "SP_n", "nmj", F16)
    sp_f = []
    for m in range(NT):
        psp = pp.tile([128, 256], F32, name="ps_sp", tag="mm")
        for jb, (j0, jw) in enumerate(N_MT):
            nc.tensor.matmul(psp[:, 0:N],
                             SP_n[jb][0:jw, m * 128:(m + 1) * 128],
                             sedgeT[jb][0:jw, 0:N], start=(jb == 0),
                             stop=(jb == 1))
        sf = pa.tile([128, N], F16, name=f"sp_f{m}")
        nc.scalar.copy(out=sf, in_=psp[:, 0:N])
        sp_f.append(sf)
    # zs_a = sps @ snW + snb (the esp part joins after AG(ve))
    zsa = []
    for m2 in range(2):
        ps = ppt([128, 256], "ps_zsa")
        for kt in range(NT):
            nc.tensor.matmul(ps[:, 0:N], snWs_sl(kt, m2 * 128, (m2 + 1) * 128),
                             sp_f[kt][:, 0:N], start=(kt == 0),
                             stop=(kt == NT - 1))
        za_t = pa.tile([128, N], F32, name=f"zsa{m2}")
        nc.vector.tensor_scalar(out=za_t, in0=ps[:, 0:N],
                                scalar1=snbs_t[:, m2:m2 + 1],
                                scalar2=None, op0=OP.add)
        zsa.append(za_t)

    # ---- znorm: load z (f16 payload), inst-norm, fin(m, z, t1) writes ----
    def znorm(src_dram, fin, zn, tagset):
        zh = []
        for m in range(NT):
            t = psh.tile([128, 256], F16, name=f"zh{zn}_{m}", tag=f"sh{m}")
            dma(out=t[:, 0:N], in_=src_dram[m * 128:(m + 1) * 128, :])
            nc.vector.memset(t[:, N:256].bitcast(F32), 0.0)
            zh.append(t)
        pstat = ppt_s([1, 512], f"st_{zn}")
        for m in range(NT):
            nc.tensor.matmul(pstat[0:1, 0:256], ones_h, zh[m][:, 0:256],
                             start=(m == 0), stop=(m == NT - 1))
        mu = pt1.tile([1, 256], F32R, name="zmu", tag="zmu")
        nc.vector.tensor_scalar_mul(mu, pstat[0:1, 0:256], 1.0 / D)
        pmu = ppt_h([128, 256], f"mub_{zn}")
        nc.tensor.matmul(pmu, ones1_r, mu, start=True, stop=True)
        z = []
        for m in range(NT):
            # centered z in f32r (f16->f32r conversion fused into subtract)
            t = pa.tile([128, 256], F16, name=f"z{zn}_{m}",
                        tag=f"{tagset}{m}")
            nc.vector.memset(t[:, N:256].bitcast(F32), 0.0)
            nc.vector.tensor_tensor(t[:, 0:N], zh[m][:, 0:N], pmu[:, 0:N],
                                    OP.subtract)
            z.append(t)
            zq = pt.tile([128, 256], F16, name="zq", tag="zq")
            nc.vector.tensor_tensor(zq, t, t, OP.mult)
            nc.tensor.matmul(pstat[0:1, 256:512], ones_h, zq[:, 0:256],
                             start=(m == 0), stop=(m == NT - 1))
        va = pt1.tile([1, 256], F32, name="zva", tag="zva")
        nc.vector.tensor_scalar(out=va, in0=pstat[0:1, 256:512],
                                scalar1=1.0 / D, scalar2=EPS,
                                op0=OP.mult, op1=OP.add)
        ta = pt1.tile([1, 256], F32, name="zta", tag="zmu2")
        nc.scalar.activation(ta, va, AF.Sqrt)
        rsf = pt1.tile([1, 256], F32, name="zrsf", tag="znm")
        nc.vector.reciprocal(out=rsf, in_=ta)
        rs = pt1.tile([1, 256], F32R, name="zrs", tag="zrs")
        nc.vector.tensor_copy(out=rs, in_=rsf)
        prr = ppt_h([128, 256], f"rb_{zn}")
        nc.tensor.matmul(prr, ones1_r, rs, start=True, stop=True)
        for m in range(NT):
            t1 = pt.tile([128, N], F32, name="zt1", tag="zt1")
            nc.vector.tensor_tensor(t1, z[m][:, 0:N], prr[:, 0:N], OP.mult)
            fin(m, z, t1)
        return z

    # h2 -> PVP (leaky), in-place in z set "za"
    def fin_pvp(m, z, t1):
        nc.vector.scalar_tensor_tensor(out=z[m][:, 0:N], in0=t1, scalar=0.2,
                                       in1=t1, op0=OP.mult, op1=OP.max)
    if "h2" in dbg:
        nc.sync.dma_start(out=dbg["h2"].ap()[:, :], in_=ar_h2_out[:, :])
    PVP = znorm(ar_h2_out, fin_pvp, "h2", "za")
    dtap16("PVP", PVP)

    PVP_n = transpose_nmajor(PVP, "PVP_n", "nmj", F16)

    vedge, vedgeT = cos_edge(PVP, [128] * NT, "ve", F16, ones_h, 256)
    if "vedge" in dbg:
        for mt, (i0, iw) in enumerate(N_MT):
            dtap("vedge", vedge[mt][0:iw, :], row0=i0, rows=iw)
            dtap("sedge", sedge[mt][0:iw, :], row0=i0, rows=iw)

    # =================================================================
    # P4: UpdateVisualNode -> AllGather(z_v) -> VP2 (in-place set "zb")
    # =================================================================
    vp_f, y_r = [], []
    for m in range(NT):
        pv = pp.tile([128, 256], F32, name="ps_vp", tag="mm")
        pe = pp.tile([128, 256], F32, name="ps_ev", tag="mm")
        for jb, (j0, jw) in enumerate(N_MT):
            nc.tensor.matmul(pv, PVP_n[jb][0:jw, m * 128:(m + 1) * 128],
                             vedgeT[jb][0:jw, :], start=(jb == 0),
                             stop=(jb == 1))
            nc.tensor.matmul(pe, PVP_n[jb][0:jw, m * 128:(m + 1) * 128],
                             sedgeT[jb][0:jw, :], start=(jb == 0),
                             stop=(jb == 1))
        vf = pa.tile([128, N], F32, name=f"vp_f{m}")
        nc.scalar.copy(out=vf, in_=pv[:, 0:N])
        vp_f.append(vf)
        yb = sh1k(f"y_{m}", m)
        nc.vector.memset(yb[:, N:256].bitcast(F32), 0.0)
        nc.vector.tensor_tensor(yb[:, 0:N], vf, pe[:, 0:N], OP.add)
        y_r.append(yb)
    dtap16("ybf", y_r)

    for m2 in range(2):
        ps = ppt([128, 256], "ps_zv")
        for kt in range(NT):
            nc.tensor.matmul(ps[:, 0:N], vnWs_sl(kt, m2 * 128, (m2 + 1) * 128),
                             y_r[kt][:, 0:N], start=(kt == 0),
                             stop=(kt == NT - 1))
        zc = pt.tile([128, N], F16, name="zvc", tag="zouts")
        nc.vector.tensor_scalar(out=zc, in0=ps[:, 0:N],
                                scalar1=vnbs_t[:, m2:m2 + 1],
                                scalar2=None, op0=OP.add)
        dma(out=ag_zv_in[m2 * 128:(m2 + 1) * 128, :], in_=zc)
    nc.gpsimd.collective_compute("AllGather", OP.bypass, replica_groups=GRP,
                                 ins=[ag_zv_in[:].opt()],
                                 outs=[ag_zv_out[:].opt()])
    if "zv" in dbg:
        nc.sync.dma_start(out=dbg["zv"].ap()[:, :], in_=ag_zv_out[:, :])

    VP2_bf = [pa.tile([128, N], BF16, name=f"VP2b{m}", tag=f"VP2b{m}")
              for m in range(NT)]

    def fin_vp2(m, z, t1):
        nc.vector.scalar_tensor_tensor(out=z[m][:, 0:N], in0=t1, scalar=0.0,
                                       in1=vp_f[m], op0=OP.max, op1=OP.add)
        nc.vector.tensor_copy(out=VP2_bf[m], in_=z[m][:, 0:N])
    VP2 = znorm(ag_zv_out, fin_vp2, "zv", "zb")
    dtap16("VP2", VP2_bf)

    # =================================================================
    # P6: UpdateVisualEdge (i-sharded, batched row stats, bf16)
    #   h1e(i,:) per hidden h: A_j + A_i - 2*C_ij with C via W1-tile scaling
    #   instnorm-over-H stats land as psum ROWS (one-hot partition matmuls)
    # =================================================================
    VP2_n = transpose_nmajor(VP2_bf, "VP2_n", "nmj", BF16)
    negx2my = [None] * NT
    for hh in range(2):
        xmyn = pt1.tile([ISH, D // 2], BF16, name=f"xmyn{hh}", tag="xmyn")
        for ch in range(4):
            ps = ppt([ISH, 256], "ps_xmy")
            for jb, (j0, jw) in enumerate(N_MT):
                nc.tensor.matmul(
                    ps, LATE["selv_t"][jb][0:jw, :],
                    VP2_n[jb][0:jw, hh * 1024 + ch * 256:
                               hh * 1024 + (ch + 1) * 256],
                    start=(jb == 0), stop=(jb == 1))
            nc.vector.tensor_copy(out=xmyn[:, ch * 256:(ch + 1) * 256],
                                  in_=ps)
        if "xmyn" in dbg:
            nc.sync.dma_start(out=dbg["xmyn"].ap()[:, hh * 1024:
                                                   (hh + 1) * 1024],
                              in_=xmyn[:, :])
        for ktl in range(8):
            kt = hh * 8 + ktl
            pst = ppt_h([128, 128], "ps_tr5", BF16)
            nc.tensor.transpose(pst[0:128, 0:ISH],
                                xmyn[:, ktl * 128:(ktl + 1) * 128],
                                ident_b[0:ISH, 0:ISH])
            ng = pa.tile([128, ISH], F32, name=f"negx2my{kt}")
            nc.vector.tensor_scalar_mul(ng, pst[0:128, 0:ISH], -2.0)
            negx2my[kt] = ng
    pA = ppt([128, N], "ps_A")
    pAm = ppt([128, ISH], "ps_Am")
    for m in range(NT):
        xq = pt.tile([128, N], BF16, name="vsq", tag="xq")
        nc.vector.tensor_tensor(xq, VP2_bf[m], VP2_bf[m], OP.mult)
        nc.tensor.matmul(pA, veW1_sl(m), xq[:, :], start=(m == 0),
                         stop=(m == NT - 1))
        xqm = pt.tile([128, ISH], BF16, name="vsqm", tag="vsqm")
        nc.vector.tensor_tensor(xqm, negx2my[m], negx2my[m], OP.mult)
        nc.tensor.matmul(pAm, veW1_sl(m), xqm[:, :], start=(m == 0),
                         stop=(m == NT - 1))
    # A_T_bf = A_j + b1 (bf16);  A_my = A_i (f32, (-2x)^2/4)
    A_T_bf = pa.tile([128, N], BF16, name="A_T_bf", tag="A_T")
    nc.vector.tensor_scalar(out=A_T_bf, in0=pA[:, 0:N],
                            scalar1=veb1_t[:, 0:1], scalar2=None, op0=OP.add)
    A_my = pa.tile([128, ISH], F32, name="A_my")
    nc.vector.tensor_scalar_mul(A_my, pAm, 0.25)
    if "AT" in dbg:
        atf = pt.tile([128, N], F32, name="atf", tag="zt1")
        nc.vector.tensor_copy(out=atf, in_=A_T_bf)
        dtap("AT", atf[0:128, 0:N])
        amf = pt.tile([128, ISH], F32, name="amf", tag="vsqm")
        nc.vector.tensor_scalar(out=amf, in0=A_my, scalar1=veb1_t[:, 0:1],
                                scalar2=None, op0=OP.add)
        dtap("Amy", amf[0:128, :])
    vedge_my = pa.tile([ISH, N], F32, name="vedge_my", tag="edg_se_0")
    psvm = ppt([ISH, N], "ps_vm")
    for mt, (i0, iw) in enumerate(N_MT):
        vb = pt.tile([128, N], BF16, name="vedgb", tag="xq")
        nc.vector.tensor_copy(out=vb[0:iw, :], in_=vedge[mt][0:iw, :])
        nc.tensor.matmul(psvm, LATE["selv_t"][mt][0:iw, :], vb[0:iw, :],
                         start=(mt == 0), stop=(mt == 1))
    nc.vector.tensor_copy(out=vedge_my, in_=psvm)

    # lhsT row-placement patterns: slice [ISH-1-k : 2*ISH-1-k] has ones (or
    # veW2) in column k, zeros elsewhere -> matmul accumulates into psum row k
    PAT = pa.tile([128, 2 * ISH - 1], BF16, name="PAT")
    nc.vector.memset(PAT, 0.0)
    nc.vector.memset(PAT[:, ISH - 1:ISH], 1.0)
    W2PAT = pa.tile([128, 2 * ISH - 1], BF16, name="W2PAT")
    nc.vector.memset(W2PAT, 0.0)
    nc.vector.tensor_copy(out=W2PAT[:, ISH - 1:ISH], in_=veW2_t[:, 0:1])
    # mean-over-H broadcast as a constant matmul: mu_b = (1/H) ones @ hsb
    ONESM = pa.tile([128, 128], BF16, name="ONESM")
    nc.vector.memset(ONESM, 1.0 / H)

    cur_ps = pps.tile([ISH, 256], F32, name="cur_ps", tag="cur")
    S_ps = ppt_s([ISH, 512], "st_e")
    for ii in range(ISH):
        psC = ppt([128, 256], "ps_C")
        for kt in range(NT):
            w1i = pstr.tile([128, H], BF16, name=f"w1i{kt}",
                            tag=f"w1i{kt % 4}")
            sc = negx2my[kt][:, ii:ii + 1]
            if kt % 3 == 0:
                nc.vector.tensor_scalar(out=w1i, in0=veW1_sl(kt),
                                        scalar1=sc, scalar2=None,
                                        op0=OP.mult)
            elif kt % 3 == 1:
                nc.gpsimd.tensor_scalar(out=w1i, in0=veW1_sl(kt),
                                        scalar1=sc, scalar2=None,
                                        op0=OP.mult)
            else:
                nc.scalar.activation(w1i, veW1_sl(kt), AF.Copy, scale=sc)
            nc.tensor.matmul(psC[:, 0:N], w1i, VP2_bf[kt],
                             start=(kt == 0), stop=(kt == NT - 1))
        # hsb = (-2C + A_i) + (A_j + b1):  Act (psum in) then DVE bf16 2x
        hp_ = pt.tile([128, N], BF16, name="ehp", tag="zt1")
        nc.scalar.activation(hp_, psC[:, 0:N], AF.Identity,
                             bias=A_my[:, ii:ii + 1], scale=1.0)
        hsb = pt.tile([128, N], BF16, name="ehsb", tag=f"hsb{ii % 2}")
        nc.vector.tensor_tensor(hsb, hp_, A_T_bf, OP.add)
        hsq = pt.tile([128, N], BF16, name="ehsq", tag="xq")
        nc.vector.tensor_tensor(hsq, hsb, hsb, OP.mult)
        psel = PAT[:, ISH - 1 - ii:2 * ISH - 1 - ii]
        nc.tensor.matmul(S_ps[0:ISH, 0:N], psel, hsb,
                         start=(ii == 0), stop=(ii == ISH - 1))
        nc.tensor.matmul(S_ps[0:ISH, 256:256 + N], psel, hsq,
                         start=(ii == 0), stop=(ii == ISH - 1))
        # centered h, relu; rs scaling deferred to cur rows (rs>0)
        pm = ppt_h([128, 256], "mu_b")
        nc.tensor.matmul(pm[:, 0:N], ONESM, hsb, start=True, stop=True)
        t1b = pt.tile([128, N], BF16, name="et1b", tag="zt1")
        nc.vector.tensor_tensor(t1b, hsb, pm[:, 0:N], OP.subtract)
        h2b = pt.tile([128, N], BF16, name="eh2b", tag="h2b")
        nc.vector.tensor_scalar_max(h2b, t1b, 0.0)
        nc.tensor.matmul(cur_ps[0:ISH, 0:N],
                         W2PAT[:, ISH - 1 - ii:2 * ISH - 1 - ii], h2b,
                         start=(ii == 0), stop=(ii == ISH - 1))
    # ---- batched row stats -> rs rows; cur = cur_raw * rs ----
    mu = pt1.tile([ISH, N], F32, name="emu", tag="emu")
    nc.vector.tensor_scalar_mul(mu, S_ps[0:ISH, 0:N], 1.0 / H)
    va = pt1.tile([ISH, N], F32, name="eva", tag="eva")
    nc.vector.tensor_scalar_mul(va, S_ps[0:ISH, 256:256 + N], 1.0 / H)
    musq = pt1.tile([ISH, N], F32, name="emusq", tag="emusq")
    nc.vector.tensor_tensor(musq, mu, mu, OP.mult)
    nc.vector.tensor_tensor(va, va, musq, OP.subtract)
    sdv = pt1.tile([ISH, N], F32, name="esdv", tag="emusq")
    nc.scalar.activation(sdv, va, AF.Sqrt, bias=epsc[0:ISH, 0:1], scale=1.0)
    rsr = pt1.tile([ISH, N], F32, name="ersr", tag="eva")
    nc.vector.reciprocal(out=rsr, in_=sdv)
    curm = pa.tile([ISH, N], F32, name="curm", tag="atf2")
    nc.vector.tensor_tensor(curm, cur_ps[0:ISH, 0:N], rsr, OP.mult)
    dtap("cur", curm[:, :])

    # tanh(cur + b2) * (vedge_my + 1e-8) -> softmax(/10) -> my edge rows
    curt = pa.tile([ISH, N], F32, name="curt", tag="atf0")
    nc.scalar.activation(curt, curm, AF.Tanh,
                         bias=veb2_t[0:ISH, 0:1], scale=1.0)
    ne = pa.tile([ISH, N], F32, name="ne", tag="atf1")
    nc.vector.scalar_tensor_tensor(out=ne, in0=vedge_my, scalar=1e-8,
                                   in1=curt, op0=OP.add, op1=OP.mult)
    rmx = pt.tile([ISH, 1], F32, name="vermx", tag="vermx")
    nc.vector.reduce_max(rmx, ne, axis=AX.X)
    bia = pt.tile([ISH, 1], F32, name="vebia", tag="vebia")
    nc.vector.tensor_scalar_mul(bia, rmx, -0.1)
    ex = pt1.tile([ISH, N], F32, name="veex", tag="veex")
    nc.scalar.activation(ex, ne, AF.Exp, bias=bia[0:ISH, 0:1], scale=0.1)
    sm = pt.tile([ISH, 1], F32, name="vesm", tag="vesm")
    nc.vector.reduce_sum(sm, ex, axis=AX.X)
    rr = pt.tile([ISH, 1], F32, name="verr", tag="verr")
    nc.vector.reciprocal(out=rr, in_=sm)
    vemine = pt1.tile([ISH, N], F16, name="vemine", tag="vemine")
    nc.vector.tensor_scalar(out=vemine, in0=ex, scalar1=rr[0:ISH, 0:1],
                            scalar2=None, op0=OP.mult)
    dma(out=ag_ve_in[:, :], in_=vemine)
    nc.gpsimd.collective_compute("AllGather", OP.bypass, replica_groups=GRP,
                                 ins=[ag_ve_in[:].opt()],
                                 outs=[ag_ve_out[:].opt()])

    # ---- AG(ve) shadow: fusion (VP2 half) + img @ VP2^T partials ----
    vu_sb = pt1.tile([1, 512], F32R, name="vu_sb", tag="vu_sb")
    nc.vector.memset(vu_sb[:].bitcast(F32), 0.0)
    pvu0 = ppt_s([1, 256], "ps_vu0")
    for m2 in range(2):
        ps = ppt([128, 256], "ps_fus")
        for kt in range(NT):
            nc.tensor.matmul(ps[:, 0:N],
                             LATE["fusWs_sl"](kt, m2 * 128, (m2 + 1) * 128),
                             VP2[kt][:, 0:N], start=(kt == 0),
                             stop=(kt == NT - 1))
        th = pt.tile([128, 256], F16, name="fth", tag="fth")
        nc.scalar.activation(th[:, 0:N], ps[:, 0:N], AF.Tanh)
        nc.tensor.matmul(pvu0[0:1, 0:N], fusUs_t[:, m2:m2 + 1], th[:, 0:N],
                         start=(m2 == 0), stop=(m2 == 1))
    nc.vector.tensor_copy(out=vu_sb[0:1, 0:N], in_=pvu0[0:1, 0:N])
    probv = []
    for bt in range(2):
        ps = ppt([128, N], "ps_prob")
        for kt in range(NT):
            nc.tensor.matmul(ps, imgT_sl(kt, bt), VP2[kt][:, 0:N],
                             start=(kt == 0), stop=(kt == NT - 1))
        pv_t = pa.tile([128, N], F32, name=f"probv{bt}")
        nc.scalar.copy(out=pv_t, in_=ps)
        probv.append(pv_t)
    if "ve2" in dbg:
        nc.sync.dma_start(out=dbg["ve2"].ap()[:, :], in_=ag_ve_out[:, :])
    ve2 = [pt1.tile([128, N], F16, name="ve2_0", tag="ve2_0"),
           pt1.tile([128, N], F16, name="ve2_1", tag="ve2_1")]
    for mt, (i0, iw) in enumerate(N_MT):
        dma(out=ve2[mt][0:iw, :], in_=ag_ve_out[i0:i0 + iw, :])
    ve2T = [pa.tile([128, 256], F16, name="ve2T0", tag="veT0"),
            pa.tile([128, 256], F16, name="ve2T1", tag="veT1")]
    for mt, (i0, iw) in enumerate(N_MT):
        for jb, (j0, jw) in enumerate(N_MT):
            pst = ppt_h([128, 128], "ps_tr6", F16)
            nc.tensor.transpose(pst[0:jw, 0:iw],
                                ve2[mt][0:iw, j0:j0 + jw],
                                ident_h[0:iw, 0:iw])
            nc.vector.tensor_copy(out=ve2T[jb][0:jw, i0:i0 + iw],
                                  in_=pst[0:jw, 0:iw])

    # =================================================================
    # P5: UpdateSemanticNode -> AllGather(z_s) -> SP2 (in-place set "za")
    #   (sps/zs_a were precomputed in the AR(h2) shadow)
    # =================================================================
    SP_n2 = transpose_nmajor(SP, "SP_n2", "nmj", F16)
    y2_r = []
    for m in range(NT):
        pes = pp.tile([128, 256], F32, name="ps_es", tag="mm")
        for jb, (j0, jw) in enumerate(N_MT):
            nc.tensor.matmul(pes[:, 0:N],
                             SP_n2[jb][0:jw, m * 128:(m + 1) * 128],
                             ve2T[jb][0:jw, 0:N], start=(jb == 0),
                             stop=(jb == 1))
        yb = sh1k(f"y2_{m}", m)
        nc.scalar.copy(out=yb[:, 0:N], in_=pes[:, 0:N])
        y2_r.append(yb)

    for m2 in range(2):
        ps = ppt([128, 256], "ps_zs")
        for kt in range(NT):
            nc.tensor.matmul(ps[:, 0:N], snWs_sl(kt, m2 * 128, (m2 + 1) * 128),
                             y2_r[kt][:, 0:N], start=(kt == 0),
                             stop=(kt == NT - 1))
        zc = pt.tile([128, N], F16, name="zsc", tag="zouts")
        nc.vector.tensor_tensor(zc, ps[:, 0:N], zsa[m2], OP.add)
        dma(out=ag_zs_in[m2 * 128:(m2 + 1) * 128, :], in_=zc)
    nc.gpsimd.collective_compute("AllGather", OP.bypass, replica_groups=GRP,
                                 ins=[ag_zs_in[:].opt()],
                                 outs=[ag_zs_out[:].opt()])

    def fin_sp2(m, z, t1):
        nc.vector.scalar_tensor_tensor(out=z[m][:, 0:N], in0=t1, scalar=0.0,
                                       in1=sp_f[m], op0=OP.max, op1=OP.add)
    SP2 = znorm(ag_zs_out, fin_sp2, "zs", "za")
    if "SP2" in dbg:
        for m in range(NT):
            nc.sync.dma_start(out=dbg["SP2"].ap()[m * 128:(m + 1) * 128, :],
                              in_=SP2[m][:, 0:N])

    # =================================================================
    # P7: fusion (SP2 half) -> AG(vu) || img @ SP2^T -> alpha -> prob
    # =================================================================
    pvu1 = ppt_s([1, 256], "ps_vu1")
    for m2 in range(2):
        ps = ppt([128, 256], "ps_fus")
        for kt in range(NT):
            nc.tensor.matmul(ps[:, 0:N],
                             LATE["fusWs_sl"](kt, m2 * 128, (m2 + 1) * 128),
                             SP2[kt][:, 0:N], start=(kt == 0),
                             stop=(kt == NT - 1))
        th = pt.tile([128, 256], F16, name="fth", tag="fth")
        nc.scalar.activation(th[:, 0:N], ps[:, 0:N], AF.Tanh)
        nc.tensor.matmul(pvu1[0:1, 0:N], fusUs_t[:, m2:m2 + 1], th[:, 0:N],
                         start=(m2 == 0), stop=(m2 == 1))
    nc.vector.tensor_copy(out=vu_sb[0:1, 256:256 + N], in_=pvu1[0:1, 0:N])
    dma(out=ag_vu_in[:, :], in_=vu_sb)
    nc.gpsimd.collective_compute("AllGather", OP.bypass, replica_groups=GRP,
                                 ins=[ag_vu_in[:].opt()],
                                 outs=[ag_vu_out[:].opt()])
    # AG(vu) shadow: img @ SP2^T partials
    probs = []
    for bt in range(2):
        ps = ppt([128, N], "ps_prob")
        for kt in range(NT):
            nc.tensor.matmul(ps, imgT_sl(kt, bt), SP2[kt][:, 0:N],
                             start=(kt == 0), stop=(kt == NT - 1))
        ps_t = pa.tile([128, N], F32, name=f"probs{bt}")
        nc.scalar.copy(out=ps_t, in_=ps)
        probs.append(ps_t)
    vus = pt1.tile([NCORES, 512], F32R, name="vus", tag="vu_sb")
    dma(out=vus, in_=ag_vu_out[:, :])
    pvk = ppt_s([1, 512], "ps_vuk")
    for k in range(2):
        nc.tensor.matmul(pvk[0:1, 256 * k:256 * k + 256],
                         ones8_r, vus[:, 256 * k:256 * k + 256],
                         start=True, stop=True)
    vuf = pt1.tile([1, 512], F32, name="vuf", tag="zstt")
    nc.vector.tensor_copy(out=vuf, in_=pvk)
    dtap("vuf", vuf[:, :])
    mx = pt.tile([1, N], F32, name="amx", tag="amx")
    nc.vector.tensor_tensor(mx, vuf[0:1, 0:N], vuf[0:1, 256:256 + N], OP.max)
    dv = pt1.tile([1, 512], F32R, name="adv", tag="adv")
    nc.vector.memset(dv[:].bitcast(F32), 0.0)
    for k in range(2):
        nc.vector.tensor_tensor(dv[0:1, 256 * k:256 * k + N],
                                vuf[0:1, 256 * k:256 * k + N], mx, OP.subtract)
    nc.scalar.activation(dv, dv, AF.Exp, scale=100.0)
    ssum = pt.tile([1, N], F32, name="assum", tag="assum")
    nc.vector.tensor_tensor(ssum, dv[0:1, 0:N], dv[0:1, 256:256 + N], OP.add)
    rsu = pt.tile([1, N], F32, name="arsu", tag="arsu")
    nc.vector.reciprocal(out=rsu, in_=ssum)
    for k in range(2):
        nc.vector.tensor_tensor(dv[0:1, 256 * k:256 * k + N],
                                dv[0:1, 256 * k:256 * k + N], rsu, OP.mult)
    alro = dv
    if "alpha" in dbg:
        al_f = pt1.tile([1, 512], F32, name="al_f", tag="zstt")
        nc.vector.tensor_copy(out=al_f, in_=alro)
        dtap("alpha", al_f[:, :])
    pal = ppt_h([128, 512], "ab_al")
    nc.tensor.matmul(pal, ones1_r, alro, start=True, stop=True)
    # prob = alpha_v * (img@VP2^T) + alpha_s * (img@SP2^T)
    for bt in range(2):
        t1 = pt.tile([128, N], F32, name="pr1", tag="zouts")
        nc.vector.tensor_tensor(t1, probv[bt], pal[:, 0:N], OP.mult)
        t2 = pt.tile([128, N], F32, name="pr2", tag="zt1")
        nc.vector.tensor_tensor(t2, probs[bt], pal[:, 256:256 + N], OP.mult)
        t3 = pt.tile([128, N], F32, name="probf", tag="zouts")
        nc.vector.tensor_tensor(t3, t1, t2, OP.add)
        dma(out=prob_out.ap()[bt * 128:(bt + 1) * 128, :], in_=t3)


# =====================================================================
# Host side
# =====================================================================
def _prep_inputs(inputs):
    bf = ml_dtypes.bfloat16
    f16 = np.float16
    f32 = np.float32
    att = np.asarray(inputs["attribute"], f32)
    cen = np.asarray(inputs["centers"], f32)
    expW = np.asarray(inputs["expert_W"], f32)
    expB = np.asarray(inputs["expert_b"], f32)
    w1 = np.asarray(inputs["s2v_W1"], f32)
    w2 = np.asarray(inputs["s2v_W2"], f32)
    in_maps = []
    for c in range(NCORES):
        cs = slice(c * DSH, (c + 1) * DSH)
        isl = slice(c * ISH, (c + 1) * ISH)
        bs = slice(c * BSH, (c + 1) * BSH)
        selv = np.zeros((N, ISH), f32)
        selv[np.arange(c * ISH, (c + 1) * ISH), np.arange(ISH)] = 1.0
        m = {
            "attrT": np.ascontiguousarray(att.T).astype(f16),
            "attrTb": np.ascontiguousarray(att.T).astype(bf),
            "centT": np.ascontiguousarray(cen.T),
            "expW": expW.astype(f16),
            "expBT": np.ascontiguousarray(expB.T),
            "w1s": np.ascontiguousarray(w1[:, cs]).astype(f16),
            "bnG": np.ascontiguousarray(np.asarray(inputs["bn_g"], f32)[cs].reshape(2, 128).T),
            "bnB": np.ascontiguousarray(np.asarray(inputs["bn_b"], f32)[cs].reshape(2, 128).T),
            "w2s": np.ascontiguousarray(w2[cs, :]).astype(f16),
            "b2o8": np.ascontiguousarray((np.asarray(inputs["s2v_b2"], f32) / NCORES).reshape(NT, 128).T),
            "vnWs": np.ascontiguousarray(np.asarray(inputs["vn_W"], f32)[:, cs]).astype(f16),
            "vnbs": np.ascontiguousarray(np.asarray(inputs["vn_b"], f32)[cs].reshape(2, 128).T),
            "snWs": np.ascontiguousarray(np.asarray(inputs["sn_W"], f32)[:, cs]).astype(f16),
            "snbs": np.ascontiguousarray(np.asarray(inputs["sn_b"], f32)[cs].reshape(2, 128).T),
            "veW1": np.asarray(inputs["ve_W1"], f32).astype(bf),
            "veb1": np.asarray(inputs["ve_b1"], f32)[:, None],
            "veW2": np.asarray(inputs["ve_W2"], f32).astype(bf),
            "veb2": np.full((ISH, 1), float(np.asarray(inputs["ve_b2"])[0]),
                            f32),
            "fusWs": np.ascontiguousarray(np.asarray(inputs["fus_W"], f32)[:, cs]).astype(f16),
            "fusUs": np.ascontiguousarray(np.asarray(inputs["fus_u"], f32)[cs, 0].reshape(2, 128).T).astype(f16),
            "imgT": np.ascontiguousarray(
                np.asarray(inputs["img_feat"], f32)[bs, :].T).astype(f16),
            "selv": selv.astype(bf),
        }
        in_maps.append(m)
    return in_maps


def kernel(**inputs):
    global _BUILT
    if _BUILT is None:
        _BUILT = build()
    nc = _BUILT
    in_maps = _prep_inputs(inputs)
    res = run_bass_kernel_spmd(nc, in_maps, core_ids=list(range(NCORES)))
    out = np.concatenate([res.results[c]["prob"] for c in range(NCORES)],
                         axis=0)
    return out.astype(np.float32)


def kernel_debug(**inputs):
    nc = build(debug=True)
    in_maps = _prep_inputs(inputs)
    res = run_bass_kernel_spmd(nc, in_maps, core_ids=list(range(NCORES)))
    out = np.concatenate([res.results[c]["prob"] for c in range(NCORES)],
                         axis=0)
    return out.astype(np.float32), res.results


if __name__ == "__main__":
    import reference
    inp = {k: np.asarray(v) for k, v in reference.setup_inputs().items()}
    got = kernel(**inp)
    exp = np.asarray(reference.reference(**reference.setup_inputs()))
    err = np.abs(got - exp).max() / (np.abs(exp).max() + 1e-9)
    print("Relative error:", err)



# revision 47
# speedup vs baseline: 1.5518x; 1.0672x over previous
# Trainium2 Bass kernel for nn_CPPN (gnn_message_passing), 8-core SPMD.
#
# Sharding:
#   - Node-MLP weights (s2v_W1/W2, vn_W, sn_W, fus_W) sharded over the 2048
#     col/row dim (256 per core); stitched with one AllReduce (h2) and three
#     AllGathers (z_v, z_s, vu partials).
#   - Visual edge MLP (200x200 pairwise rows) sharded over i: 25 rows/core,
#     AllGather of the resulting edge rows.  Per-core column selection is via
#     a host-supplied one-hot matrix (SPMD program is identical on all cores;
#     only input data differs).
#   - img_feat batch sharded 256 rows/core for the final prob matmul; host
#     concatenates per-core outputs.
# Layout: activations are d-major [D on partitions (16x128), 200 on free].
# Heavy matmuls bf16; stats/broadcast matmuls f32r.  The reference's second
# edge_update (semantic) is dead code w.r.t. prob and is skipped.

import sys

sys.path.insert(0, "/opt/trn_rl_repo")

import numpy as np
import ml_dtypes

import concourse.bass as bass
import concourse.bacc as bacc
import concourse.tile as tile
from concourse import mybir
from concourse.bass_utils import run_bass_kernel_spmd
from concourse.masks import make_identity

F32 = mybir.dt.float32
F32R = mybir.dt.float32r
BF16 = mybir.dt.bfloat16
F16 = mybir.dt.float16
AF = mybir.ActivationFunctionType
OP = mybir.AluOpType
AX = mybir.AxisListType

NCORES = 8
N = 200
S = 312
D = 2048
H = 128
KEXP = 3
B = 2048
DSH = D // NCORES      # 256
ISH = N // NCORES      # 25
BSH = B // NCORES      # 256
EPS = 1e-5
NT = D // 128          # 16
S_KT = [128, 128, 56]
N_MT = ((0, 128), (128, 72))
G_I = 2                # i-group size for edge t-gen batching

_BUILT = None


def _rep(ap_src, dims):
    """Rebuild AP with explicit free dims [[step,count],...] after partition."""
    return bass.AP(tensor=ap_src.tensor, offset=ap_src.offset,
                   ap=[ap_src.ap[0]] + dims)


def build(debug=False):
    nc = bacc.Bacc("TRN2", target_bir_lowering=False, debug=False,
                   num_devices=NCORES)
    d = {}

    def din(name, shape, dt):
        d[name] = nc.dram_tensor(name, shape, dt, kind="ExternalInput")

    din("attrT", [S, N], F16)
    din("attrTb", [S, N], BF16)
    din("centT", [S, KEXP], F32)
    din("expW", [KEXP, S, D], F16)
    din("expBT", [D, KEXP], F32)
    din("w1s", [D, DSH], F16)
    din("bnG", [128, 2], F32)
    din("bnB", [128, 2], F32)
    din("w2s", [DSH, D], F16)
    din("b2o8", [128, NT], F32)
    din("vnWs", [D, DSH], F16)
    din("vnbs", [128, 2], F32)
    din("snWs", [D, DSH], F16)
    din("snbs", [128, 2], F32)
    din("veW1", [D, H], BF16)
    din("veb1", [H, 1], F32)
    din("veW2", [H, 1], BF16)
    din("veb2", [ISH, 1], F32)
    din("fusWs", [D, DSH], F16)
    din("fusUs", [128, 2], F16)
    din("imgT", [D, BSH], F16)
    din("selv", [N, ISH], BF16)
    prob_out = nc.dram_tensor("prob", [BSH, N], F32, kind="ExternalOutput")
    dbg = {}
    if debug:
        def dout(name, shape, dt):
            dbg[name] = nc.dram_tensor("dbg_" + name, shape, dt,
                                       kind="ExternalOutput")
        dout("SP", [D, N], F16)
        dout("a1", [2 * 128, N], F16)
        dout("h2", [D, N], F16)
        dout("PVP", [D, N], F16)
        dout("vedge", [N, N], F16)
        dout("sedge", [N, N], F16)
        dout("ybf", [D, N], F16)
        dout("zv", [D, N], F16)
        dout("VP2", [D, N], BF16)
        dout("AT", [128, N], F32)
        dout("Amy", [128, ISH], F32)
        dout("xmyn", [ISH, D], BF16)
        dout("cur", [ISH, N], F32)
        dout("ve2", [N, N], F16)
        dout("SP2", [D, N], F16)
        dout("vuf", [1, 512], F32)
        dout("alpha", [1, 512], F32)

    with tile.TileContext(nc) as tc:
        import contextlib
        with contextlib.ExitStack() as ctx, \
                nc.allow_low_precision(reason="bf16 PE transposes (no accum)"):
            _emit(ctx, nc, tc, d, prob_out, dbg)
    nc.compile()
    return nc


def _emit(ctx, nc, tc, d, prob_out, dbg=None):
    dbg = dbg or {}

    def dtap16(key, tiles):     # 16 d-major tiles -> [D, N] dram
        if key in dbg:
            for m in range(NT):
                nc.sync.dma_start(out=dbg[key].ap()[m * 128:(m + 1) * 128, :],
                                  in_=tiles[m][:, 0:N])

    def dtap(key, ap_in, row0=0, rows=None):
        if key in dbg:
            o = dbg[key].ap()
            nc.sync.dma_start(out=o[row0:row0 + (rows or o.shape[0]), :],
                              in_=ap_in)

    pw = ctx.enter_context(tc.tile_pool(name="wts", bufs=1))
    pa = ctx.enter_context(tc.tile_pool(name="acts", bufs=1))
    pt = ctx.enter_context(tc.tile_pool(name="tmp", bufs=2))
    pt1 = ctx.enter_context(tc.tile_pool(name="rows", bufs=1))
    psh = ctx.enter_context(tc.tile_pool(name="sh1k", bufs=1))
    pstr = ctx.enter_context(tc.tile_pool(name="stream", bufs=1))
    pdram = ctx.enter_context(tc.tile_pool(name="dram", bufs=1, space="DRAM"))
    pp = ctx.enter_context(tc.tile_pool(name="ps_mm", bufs=3, space="PSUM"))
    pph = ctx.enter_context(tc.tile_pool(name="ps_hold", bufs=2, space="PSUM"))
    pps = ctx.enter_context(tc.tile_pool(name="ps_st", bufs=1, space="PSUM"))

    dma = nc.sync.dma_start
    GRP = [list(range(NCORES))]

    def ppt(shape, name, dt=F32):
        return pp.tile(shape, dt, name=name, tag="mm")

    def ppt_h(shape, name, dt=F32):
        return pph.tile(shape, dt, name=name, tag="hold")

    def ppt_s(shape, name, dt=F32):
        return pps.tile(shape, dt, name=name, tag="stat")

    def sh1k(name, tagid, dt=F16, shape=None):
        return psh.tile(shape or [128, 256], dt, name=name, tag=f"sh{tagid}")

    # ---------- constants ----------
    ident_b = pa.tile([128, 128], BF16, name="ident_b")
    make_identity(nc, ident_b)
    ident_f = pt1.tile([128, 128], F32, name="ident_f", tag="ident_f")
    make_identity(nc, ident_f)
    ident_h = pa.tile([128, 128], F16, name="ident_h")
    nc.vector.tensor_copy(out=ident_h, in_=ident_f)

    ones_r = pa.tile([128, 1], F32R, name="ones_r")
    nc.vector.memset(ones_r[:].bitcast(F32), 1.0)
    ones_h = pa.tile([128, 1], F16, name="ones_h")
    nc.vector.memset(ones_h, 1.0)
    ones1_r = pa.tile([1, 128], F32R, name="ones1_r")
    nc.vector.memset(ones1_r[:].bitcast(F32), 1.0)

    ones8_r = pa.tile([8, 1], F32R, name="ones8_r")
    nc.vector.memset(ones8_r[:].bitcast(F32), 1.0)
    epsc = pa.tile([128, 1], F32, name="epsc")
    nc.vector.memset(epsc, EPS)

    # ---------- inputs -> SBUF ----------
    at_f, at_b = [], []
    off = 0
    for kt, ksz in enumerate(S_KT):
        tf = psh.tile([128, N], F16, name=f"at_f{kt}", tag=f"sh{9 + kt}")
        tb = pa.tile([128, N], BF16, name=f"at_b{kt}")
        dma(out=tf[0:ksz, :], in_=d["attrT"].ap()[off:off + ksz, :])
        dma(out=tb[0:ksz, :], in_=d["attrTb"].ap()[off:off + ksz, :])
        at_f.append(tf)
        at_b.append(tb)
        off += ksz
    centT_t = []
    off = 0
    for kt, ksz in enumerate(S_KT):
        t = pa.tile([128, KEXP], F32, name=f"centT{kt}")
        dma(out=t[0:ksz, :], in_=d["centT"].ap()[off:off + ksz, :])
        centT_t.append(t)
        off += ksz
    expbt_a = pw.tile([128, NT * KEXP], F32, name="expbt_a")
    dma(out=expbt_a[:].rearrange("p (m k) -> p m k", m=NT),
        in_=d["expBT"].ap().rearrange("(m p) k -> p m k", p=128))

    def expbt_sl(m, k):
        return expbt_a[:, m * KEXP + k:m * KEXP + k + 1]

    def load4(name, key, tagbase=None):
        """[D, DSH] weight as 4 tiles [128, 4*DSH]; one DMA per tile."""
        out = []
        for g in range(4):
            tg = f"{tagbase}{g}" if tagbase else ""
            t = pw.tile([128, 4 * DSH], F16, name=f"{name}{g}", tag=tg)
            dma(out=t[:].rearrange("p (kt c) -> p kt c", kt=4),
                in_=d[key].ap()[g * 512:(g + 1) * 512, :].rearrange(
                    "(kt p) c -> p kt c", p=128))
            out.append(t)
        return out

    def sl4(tiles):
        def f(kt, c0, c1):
            return tiles[kt // 4][:, (kt % 4) * DSH + c0:(kt % 4) * DSH + c1]
        return f

    w1s_4 = load4("w1s", "w1s")
    w1s_sl = sl4(w1s_4)
    snWs_4 = load4("snWs", "snWs")
    snWs_sl = sl4(snWs_4)

    def late_loads():
        """Weights only needed after AR(h2): emit their DMAs in its shadow."""
        g = {}
        vnWs_4 = load4("vnWs", "vnWs")
        g["vnWs_sl"] = sl4(vnWs_4)
        fusWs_4 = load4("fusWs", "fusWs", tagbase="w1s")
        g["fusWs_sl"] = sl4(fusWs_4)
        imgT_p = []
        for hh in range(2):
            t = pw.tile([128, 8 * BSH], F16, name=f"imgTp{hh}")
            src_ap = d["imgT"].ap()[hh * 1024:(hh + 1) * 1024, :].rearrange(
                "(kt p) b -> p kt b", p=128)
            nc.sync.dma_start(out=t[:].rearrange("p (kt b) -> p kt b", kt=8),
                              in_=src_ap)
            imgT_p.append(t)
        g["imgT_p"] = imgT_p
        veW1_a = pw.tile([128, D], BF16, name="veW1_a")
        nc.sync.dma_start(
            out=veW1_a[:].rearrange("p (kt h) -> p kt h", kt=NT),
            in_=d["veW1"].ap().rearrange("(kt p) h -> p kt h", p=128))
        g["veW1_a"] = veW1_a
        selv_t = []
        for jb, (j0, jw) in enumerate(N_MT):
            t = pw.tile([128, ISH], BF16, name=f"selv{jb}")
            dma(out=t[0:jw, :], in_=d["selv"].ap()[j0:j0 + jw, :])
            selv_t.append(t)
        g["selv_t"] = selv_t
        return g

    def imgT_sl(kt, bt):
        return LATE["imgT_p"][kt // 8][:, (kt % 8) * BSH + bt * 128:
                                       (kt % 8) * BSH + (bt + 1) * 128]

    def veW1_sl(kt):
        return LATE["veW1_a"][:, kt * H:(kt + 1) * H]

    def loadmat(name, key, rows, cols, dt=F32):
        t = pw.tile([rows, cols], dt, name=name)
        dma(out=t, in_=d[key].ap()[:, :])
        return t

    bnG_t = loadmat("bnG_t", "bnG", 128, 2)
    bnB_t = loadmat("bnB_t", "bnB", 128, 2)
    vnbs_t = loadmat("vnbs_t", "vnbs", 128, 2)
    snbs_t = loadmat("snbs_t", "snbs", 128, 2)
    veb1_t = loadmat("veb1_t", "veb1", H, 1)
    veb2_t = loadmat("veb2_t", "veb2", ISH, 1)
    b2o8_t = loadmat("b2o8_t", "b2o8", 128, NT)
    veW2_t = loadmat("veW2_t", "veW2", H, 1, BF16)
    fusUs_t = loadmat("fusUs_t", "fusUs", 128, 2, F16)

    # collective bounce buffers (fp16 payloads: 10-bit mantissa is enough —
    # verified against reference; halves on-wire bytes)
    ar_h2_in = pdram.tile([D, N], F16, name="ar_h2_in")
    ar_h2_out = pdram.tile([D, N], F16, addr_space="Shared", name="ar_h2_out")
    ag_zv_in = pdram.tile([DSH, N], F16, name="ag_zv_in")
    ag_zv_out = pdram.tile([D, N], F16, addr_space="Shared", name="ag_zv_out")
    ag_zs_in = pdram.tile([DSH, N], F16, name="ag_zs_in")
    ag_zs_out = pdram.tile([D, N], F16, addr_space="Shared", name="ag_zs_out")
    ag_ve_in = pdram.tile([ISH, N], F16, name="ag_ve_in")
    ag_ve_out = pdram.tile([N, N], F16, addr_space="Shared", name="ag_ve_out")
    ag_vu_in = pdram.tile([1, 512], F32R, name="ag_vu_in")
    ag_vu_out = pdram.tile([NCORES, 512], F32R, addr_space="Shared",
                           name="ag_vu_out")

    # =================================================================
    # P1: CooperationModule -> SP (f32r padded, d-major)
    # =================================================================
    offT = []
    for k in range(KEXP):
        row = []
        for kt, ksz in enumerate(S_KT):
            t = sh1k(f"offT{k}_{kt}", 3 * k + kt)
            nc.vector.memset(t[:, N:256].bitcast(F32), 0.0)
            nc.vector.tensor_scalar(
                out=t[0:ksz, 0:N], in0=at_f[kt][0:ksz, :],
                scalar1=centT_t[kt][0:ksz, k:k + 1], scalar2=None,
                op0=OP.subtract)
            row.append(t)
        offT.append(row)

    SP = [pa.tile([128, 256], F16, name=f"SP{m}") for m in range(NT)]
    for m in range(NT):
        nc.vector.memset(SP[m][:, N:256].bitcast(F32), 0.0)
    for k in range(KEXP):
        for half in range(4):
            wst = []
            for kt, ksz in enumerate(S_KT):
                w = pstr.tile([128, D // 4], F16, name=f"expw_st{kt}",
                              tag=f"str{kt}")
                dma(out=w[0:ksz, :],
                    in_=d["expW"].ap()[k, sum(S_KT[:kt]):sum(S_KT[:kt]) + ksz,
                                       half * 512:(half + 1) * 512])
                wst.append(w)
            for mh in range(4):
                m = half * 4 + mh
                ps = ppt([128, 256], "ps_exp")
                for kt, ksz in enumerate(S_KT):
                    nc.tensor.matmul(ps, wst[kt][0:ksz, mh * 128:(mh + 1) * 128],
                                     offT[k][kt][0:ksz, 0:256],
                                     start=(kt == 0),
                                     stop=(kt == len(S_KT) - 1))
                if k == 0:
                    nc.vector.tensor_scalar(out=SP[m][:, 0:N], in0=ps[:, 0:N],
                                            scalar1=expbt_sl(m, 0),
                                            scalar2=0.0,
                                            op0=OP.add, op1=OP.max)
                else:
                    r1 = pt1.tile([128, N], F16, name="exr", tag="exr")
                    nc.scalar.activation(r1, ps[:, 0:N], AF.Relu,
                                         bias=expbt_sl(m, k), scale=1.0)
                    nc.vector.tensor_tensor(SP[m][:, 0:N], SP[m][:, 0:N], r1,
                                            OP.add)
    dtap16("SP", SP)

    # =================================================================
    # P2: semantic2visual -> AllReduce(h2)
    # =================================================================
    a1 = []
    for m2 in range(2):
        ps = ppt([128, 256], "ps_h1")
        for kt in range(NT):
            nc.tensor.matmul(ps[:, 0:N], w1s_sl(kt, m2 * 128, (m2 + 1) * 128),
                             SP[kt][:, 0:N], start=(kt == 0),
                             stop=(kt == NT - 1))
        st6 = pt.tile([128, 6], F32, name="bn_st", tag="bn_st")
        mv = pt.tile([128, 2], F32, name="bn_mv", tag="bn_mv")
        nc.vector.bn_stats(out=st6, in_=ps[:, 0:N])
        nc.vector.bn_aggr(out=mv, in_=st6)
        sd = pt.tile([128, 1], F32, name="bn_sd", tag="bn_sd")
        nc.scalar.activation(sd, mv[:, 1:2], AF.Sqrt, bias=epsc[:, 0:1],
                             scale=1.0)
        rs = pt.tile([128, 1], F32, name="bn_rs", tag="bn_rs")
        nc.vector.reciprocal(out=rs, in_=sd)
        Av = pt.tile([128, 1], F32, name="bn_A", tag="bn_A")
        nc.vector.tensor_tensor(Av, rs, bnG_t[:, m2:m2 + 1], OP.mult)
        Bt = pt.tile([128, 1], F32, name="bn_Bt", tag="bn_Bt")
        nc.vector.tensor_tensor(Bt, mv[:, 0:1], Av, OP.mult)
        Bv = pt.tile([128, 1], F32, name="bn_Bv", tag="bn_Bv")
        nc.vector.tensor_tensor(Bv, bnB_t[:, m2:m2 + 1], Bt, OP.subtract)
        t1 = pt.tile([128, N], F32, name="h1_t1", tag="zt1")
        nc.vector.tensor_scalar(out=t1, in0=ps[:, 0:N], scalar1=Av,
                                scalar2=Bv, op0=OP.mult, op1=OP.add)
        a1m = pa.tile([128, 256], F16, name=f"a1_{m2}")
        nc.vector.memset(a1m[:, N:256].bitcast(F32), 0.0)
        nc.vector.scalar_tensor_tensor(out=a1m[:, 0:N], in0=t1, scalar=0.2,
                                       in1=t1, op0=OP.mult, op1=OP.max)
        dtap("a1", a1m[:, 0:N], row0=m2 * 128, rows=128)
        a1.append(a1m)

    for half in range(4):
        w2st = []
        for kt in range(2):
            w = pstr.tile([128, D // 4], F16, name=f"w2_st{kt}",
                          tag=f"str{kt}")
            dma(out=w, in_=d["w2s"].ap()[kt * 128:(kt + 1) * 128,
                                         half * 512:(half + 1) * 512])
            w2st.append(w)
        for mh in range(4):
            m = half * 4 + mh
            ps = ppt([128, 256], "ps_h2")
            for kt2 in range(2):
                nc.tensor.matmul(ps, w2st[kt2][:, mh * 128:(mh + 1) * 128],
                                 a1[kt2][:, 0:256], start=(kt2 == 0),
                                 stop=(kt2 == 1))
            hp = pt.tile([128, N], F16, name="h2p", tag="zouts")
            nc.vector.tensor_scalar(out=hp, in0=ps[:, 0:N],
                                    scalar1=b2o8_t[:, m:m + 1],
                                    scalar2=None, op0=OP.add)
            dma(out=ar_h2_in[m * 128:(m + 1) * 128, :], in_=hp)
    nc.gpsimd.collective_compute("AllReduce", OP.add, replica_groups=GRP,
                                 ins=[ar_h2_in[:].opt()],
                                 outs=[ar_h2_out[:].opt()])

    # PVP_n (f32r) on the shared n-major chain
    def transpose_nmajor(src_tiles, name, tagbase, dt):
        out = [pa.tile([128, D], dt, name=f"{name}0", tag=f"{tagbase}0"),
               pa.tile([128, D], dt, name=f"{name}1", tag=f"{tagbase}1")]
        for m in range(NT):
            for jb, (j0, jw) in enumerate(N_MT):
                if dt == F32R:
                    ps = ppt_h([128, 128], "ps_tr", F32)
                    nc.tensor.transpose(
                        ps[0:jw, 0:128],
                        src_tiles[m][:, j0:j0 + jw].bitcast(F32), ident_f)
                else:
                    ps = ppt_h([128, 128], "ps_tr", dt)
                    idm = ident_b if dt == BF16 else ident_h
                    nc.tensor.transpose(ps[0:jw, 0:128],
                                        src_tiles[m][:, j0:j0 + jw], idm)
                nc.vector.tensor_copy(
                    out=out[jb][0:jw, m * 128:(m + 1) * 128],
                    in_=ps[0:jw, 0:128])
        return out

    def cos_edge(x_tiles, ksizes, en, rdt, ones_g, rhs_w):
        nkt = len(ksizes)
        pn = ppt_s([1, 256] if rhs_w == 256 else [1, N], f"nrm_{en}")
        for kt, ksz in enumerate(ksizes):
            xq = pt.tile([128, rhs_w], rdt, name="xq", tag="xq")
            nc.vector.tensor_tensor(xq[0:ksz, :], x_tiles[kt][0:ksz, 0:rhs_w],
                                    x_tiles[kt][0:ksz, 0:rhs_w], OP.mult)
            nc.tensor.matmul(pn, ones_g[0:ksz, :], xq[0:ksz, :],
                             start=(kt == 0), stop=(kt == nkt - 1))
        sd = pt.tile([1, N], F32, name="esd", tag="zmu")
        nc.scalar.activation(sd, pn[0:1, 0:N], AF.Sqrt)
        rn_f = pt1.tile([1, 256], F32, name="ern_f", tag="zva")
        nc.vector.memset(rn_f[0:1, N:256], 0.0)
        nc.vector.reciprocal(out=rn_f[0:1, 0:N], in_=sd)
        rn = pt1.tile([1, 256], F32R, name="ern", tag="zAB")
        nc.vector.tensor_copy(out=rn, in_=rn_f)
        prn = ppt_h([128, 256], f"rnb_{en}")
        nc.tensor.matmul(prn, ones1_r, rn, start=True, stop=True)
        rcol = pt.tile([128, 2], F32, name="rc", tag=f"rc_{en}")
        for mt, (i0, iw) in enumerate(N_MT):
            pst = ppt_h([128, 128], "ps_tr3")
            nc.tensor.transpose(pst[0:iw, 0:1], rn_f[0:1, i0:i0 + iw],
                                ident_f[0:1, 0:1])
            nc.vector.tensor_copy(out=rcol[0:iw, mt:mt + 1],
                                  in_=pst[0:iw, 0:1])
        edge = []
        for mt, (i0, iw) in enumerate(N_MT):
            ps = ppt([128, rhs_w], f"ps_{en}")
            for kt, ksz in enumerate(ksizes):
                nc.tensor.matmul(ps[0:iw, :], x_tiles[kt][0:ksz, i0:i0 + iw],
                                 x_tiles[kt][0:ksz, 0:rhs_w],
                                 start=(kt == 0), stop=(kt == nkt - 1))
            s1 = pt.tile([128, N], F32, name="es1", tag="zt1")
            nc.vector.tensor_scalar(out=s1[0:iw, :], in0=ps[0:iw, 0:N],
                                    scalar1=rcol[0:iw, mt:mt + 1],
                                    scalar2=None, op0=OP.mult)
            nc.vector.tensor_tensor(s1[0:iw, :], s1[0:iw, :], prn[0:iw, 0:N],
                                    OP.mult)
            rmx = pt.tile([128, 1], F32, name="ermx", tag="ermx")
            nc.vector.reduce_max(rmx[0:iw, :], s1[0:iw, :], axis=AX.X)
            bia = pt.tile([128, 1], F32, name="ebia", tag="ebia")
            nc.vector.tensor_scalar_mul(bia[0:iw, :], rmx[0:iw, :], -100.0)
            nc.scalar.activation(s1[0:iw, :], s1[0:iw, :], AF.Exp,
                                 bias=bia[0:iw, 0:1], scale=100.0)
            sm = pt.tile([128, 1], F32, name="esm", tag="esm")
            nc.vector.reduce_sum(sm[0:iw, :], s1[0:iw, :], axis=AX.X)
            rr = pt.tile([128, 1], F32, name="err", tag="err")
            nc.vector.reciprocal(out=rr[0:iw, :], in_=sm[0:iw, :])
            ed = pa.tile([128, N], F16, name=f"{en}_{mt}",
                         tag=f"edg_{en}_{mt}")
            nc.vector.tensor_scalar(out=ed[0:iw, :], in0=s1[0:iw, :],
                                    scalar1=rr[0:iw, 0:1], scalar2=None,
                                    op0=OP.mult)
            edge.append(ed)
        edgeT = [pa.tile([128, 256], F16, name=f"{en}T0", tag=f"{en}T0"),
                 pa.tile([128, 256], F16, name=f"{en}T1", tag=f"{en}T1")]
        for jb in range(2):
            nc.vector.memset(edgeT[jb][:].bitcast(F32), 0.0)
        for mt, (i0, iw) in enumerate(N_MT):
            for jb, (j0, jw) in enumerate(N_MT):
                pst = ppt_h([128, 128], "ps_tr4", F16)
                nc.tensor.transpose(pst[0:jw, 0:iw],
                                    edge[mt][0:iw, j0:j0 + jw],
                                    ident_h[0:iw, 0:iw])
                nc.vector.tensor_copy(out=edgeT[jb][0:jw, i0:i0 + iw],
                                      in_=pst[0:jw, 0:iw])
        return edge, edgeT

    ones_b = pa.tile([128, 1], BF16, name="ones_b")
    nc.vector.memset(ones_b, 1.0)

    # ---- AR(h2) shadow: everything independent of h2 ----
    LATE = late_loads()
    sedge, sedgeT = cos_edge(at_b, S_KT, "se", BF16, ones_b, N)
    SP_n = transpose_nmajor(SP, "SP_n", "nmj", F16)
    sp_f = []
    for m in range(NT):
        psp = pp.tile([128, 256], F32, name="ps_sp", tag="mm")
        for jb, (j0, jw) in enumerate(N_MT):
            nc.tensor.matmul(psp[:, 0:N],
                             SP_n[jb][0:jw, m * 128:(m + 1) * 128],
                             sedgeT[jb][0:jw, 0:N], start=(jb == 0),
                             stop=(jb == 1))
        sf = pa.tile([128, N], F16, name=f"sp_f{m}")
        nc.scalar.copy(out=sf, in_=psp[:, 0:N])
        sp_f.append(sf)
    # zs_a = sps @ snW + snb (the esp part joins after AG(ve))
    zsa = []
    for m2 in range(2):
        ps = ppt([128, 256], "ps_zsa")
        for kt in range(NT):
            nc.tensor.matmul(ps[:, 0:N], snWs_sl(kt, m2 * 128, (m2 + 1) * 128),
                             sp_f[kt][:, 0:N], start=(kt == 0),
                             stop=(kt == NT - 1))
        za_t = pa.tile([128, N], F32, name=f"zsa{m2}")
        nc.vector.tensor_scalar(out=za_t, in0=ps[:, 0:N],
                                scalar1=snbs_t[:, m2:m2 + 1],
                                scalar2=None, op0=OP.add)
        zsa.append(za_t)

    # ---- znorm: load z (f16 payload), inst-norm, fin(m, z, t1) writes ----
    def znorm(src_dram, fin, zn, tagset):
        zh = []
        for m in range(NT):
            t = psh.tile([128, 256], F16, name=f"zh{zn}_{m}", tag=f"sh{m}")
            dma(out=t[:, 0:N], in_=src_dram[m * 128:(m + 1) * 128, :])
            nc.vector.memset(t[:, N:256].bitcast(F32), 0.0)
            zh.append(t)
        pstat = ppt_s([1, 512], f"st_{zn}")
        for m in range(NT):
            nc.tensor.matmul(pstat[0:1, 0:256], ones_h, zh[m][:, 0:256],
                             start=(m == 0), stop=(m == NT - 1))
        mu = pt1.tile([1, 256], F32R, name="zmu", tag="zmu")
        nc.vector.tensor_scalar_mul(mu, pstat[0:1, 0:256], 1.0 / D)
        pmu = ppt_h([128, 256], f"mub_{zn}")
        nc.tensor.matmul(pmu, ones1_r, mu, start=True, stop=True)
        z = []
        for m in range(NT):
            # centered z in f32r (f16->f32r conversion fused into subtract)
            t = pa.tile([128, 256], F16, name=f"z{zn}_{m}",
                        tag=f"{tagset}{m}")
            nc.gpsimd.memset(t[:, N:256].bitcast(F32), 0.0)
            nc.vector.tensor_tensor(t[:, 0:N], zh[m][:, 0:N], pmu[:, 0:N],
                                    OP.subtract)
            z.append(t)
            zq = pt.tile([128, 256], F16, name="zq", tag="zq")
            eng = nc.gpsimd if m % 2 == 0 else nc.vector
            eng.tensor_tensor(zq, t, t, OP.mult)
            nc.tensor.matmul(pstat[0:1, 256:512], ones_h, zq[:, 0:256],
                             start=(m == 0), stop=(m == NT - 1))
        va = pt1.tile([1, 256], F32, name="zva", tag="zva")
        nc.vector.tensor_scalar(out=va, in0=pstat[0:1, 256:512],
                                scalar1=1.0 / D, scalar2=EPS,
                                op0=OP.mult, op1=OP.add)
        ta = pt1.tile([1, 256], F32, name="zta", tag="zmu2")
        nc.scalar.activation(ta, va, AF.Sqrt)
        rsf = pt1.tile([1, 256], F32, name="zrsf", tag="znm")
        nc.vector.reciprocal(out=rsf, in_=ta)
        rs = pt1.tile([1, 256], F32R, name="zrs", tag="zrs")
        nc.vector.tensor_copy(out=rs, in_=rsf)
        prr = ppt_h([128, 256], f"rb_{zn}")
        nc.tensor.matmul(prr, ones1_r, rs, start=True, stop=True)
        for m in range(NT):
            t1 = pt.tile([128, N], F32, name="zt1", tag="zt1")
            nc.vector.tensor_tensor(t1, z[m][:, 0:N], prr[:, 0:N], OP.mult)
            fin(m, z, t1)
        return z

    # h2 -> PVP (leaky), in-place in z set "za"
    def fin_pvp(m, z, t1):
        nc.vector.scalar_tensor_tensor(out=z[m][:, 0:N], in0=t1, scalar=0.2,
                                       in1=t1, op0=OP.mult, op1=OP.max)
    if "h2" in dbg:
        nc.sync.dma_start(out=dbg["h2"].ap()[:, :], in_=ar_h2_out[:, :])
    PVP = znorm(ar_h2_out, fin_pvp, "h2", "za")
    dtap16("PVP", PVP)

    PVP_n = transpose_nmajor(PVP, "PVP_n", "nmj", F16)

    vedge, vedgeT = cos_edge(PVP, [128] * NT, "ve", F16, ones_h, 256)
    if "vedge" in dbg:
        for mt, (i0, iw) in enumerate(N_MT):
            dtap("vedge", vedge[mt][0:iw, :], row0=i0, rows=iw)
            dtap("sedge", sedge[mt][0:iw, :], row0=i0, rows=iw)

    # =================================================================
    # P4: UpdateVisualNode -> AllGather(z_v) -> VP2 (in-place set "zb")
    # =================================================================
    vp_f, y_r = [], []
    for m in range(NT):
        pv = pp.tile([128, 256], F32, name="ps_vp", tag="mm")
        pe = pp.tile([128, 256], F32, name="ps_ev", tag="mm")
        for jb, (j0, jw) in enumerate(N_MT):
            nc.tensor.matmul(pv, PVP_n[jb][0:jw, m * 128:(m + 1) * 128],
                             vedgeT[jb][0:jw, :], start=(jb == 0),
                             stop=(jb == 1))
            nc.tensor.matmul(pe, PVP_n[jb][0:jw, m * 128:(m + 1) * 128],
                             sedgeT[jb][0:jw, :], start=(jb == 0),
                             stop=(jb == 1))
        vf = pa.tile([128, N], F32, name=f"vp_f{m}")
        nc.scalar.copy(out=vf, in_=pv[:, 0:N])
        vp_f.append(vf)
        yb = sh1k(f"y_{m}", m)
        nc.vector.memset(yb[:, N:256].bitcast(F32), 0.0)
        nc.vector.tensor_tensor(yb[:, 0:N], vf, pe[:, 0:N], OP.add)
        y_r.append(yb)
    dtap16("ybf", y_r)

    for m2 in range(2):
        ps = ppt([128, 256], "ps_zv")
        for kt in range(NT):
            nc.tensor.matmul(ps[:, 0:N], LATE["vnWs_sl"](kt, m2 * 128, (m2 + 1) * 128),
                             y_r[kt][:, 0:N], start=(kt == 0),
                             stop=(kt == NT - 1))
        zc = pt.tile([128, N], F16, name="zvc", tag="zouts")
        nc.vector.tensor_scalar(out=zc, in0=ps[:, 0:N],
                                scalar1=vnbs_t[:, m2:m2 + 1],
                                scalar2=None, op0=OP.add)
        dma(out=ag_zv_in[m2 * 128:(m2 + 1) * 128, :], in_=zc)
    nc.gpsimd.collective_compute("AllGather", OP.bypass, replica_groups=GRP,
                                 ins=[ag_zv_in[:].opt()],
                                 outs=[ag_zv_out[:].opt()])
    if "zv" in dbg:
        nc.sync.dma_start(out=dbg["zv"].ap()[:, :], in_=ag_zv_out[:, :])

    VP2_bf = [pa.tile([128, N], BF16, name=f"VP2b{m}", tag=f"VP2b{m}")
              for m in range(NT)]

    def fin_vp2(m, z, t1):
        nc.vector.scalar_tensor_tensor(out=z[m][:, 0:N], in0=t1, scalar=0.0,
                                       in1=vp_f[m], op0=OP.max, op1=OP.add)
        nc.vector.tensor_copy(out=VP2_bf[m], in_=z[m][:, 0:N])
    VP2 = znorm(ag_zv_out, fin_vp2, "zv", "zb")
    dtap16("VP2", VP2_bf)

    # =================================================================
    # P6: UpdateVisualEdge (i-sharded, batched row stats, bf16)
    #   h1e(i,:) per hidden h: A_j + A_i - 2*C_ij with C via W1-tile scaling
    #   instnorm-over-H stats land as psum ROWS (one-hot partition matmuls)
    # =================================================================
    VP2_n = transpose_nmajor(VP2_bf, "VP2_n", "nmj", BF16)
    negx2my = [None] * NT
    for hh in range(2):
        xmyn = pt1.tile([ISH, D // 2], BF16, name=f"xmyn{hh}", tag="xmyn")
        for ch in range(4):
            ps = ppt([ISH, 256], "ps_xmy")
            for jb, (j0, jw) in enumerate(N_MT):
                nc.tensor.matmul(
                    ps, LATE["selv_t"][jb][0:jw, :],
                    VP2_n[jb][0:jw, hh * 1024 + ch * 256:
                               hh * 1024 + (ch + 1) * 256],
                    start=(jb == 0), stop=(jb == 1))
            nc.vector.tensor_copy(out=xmyn[:, ch * 256:(ch + 1) * 256],
                                  in_=ps)
        if "xmyn" in dbg:
            nc.sync.dma_start(out=dbg["xmyn"].ap()[:, hh * 1024:
                                                   (hh + 1) * 1024],
                              in_=xmyn[:, :])
        for ktl in range(8):
            kt = hh * 8 + ktl
            pst = ppt_h([128, 128], "ps_tr5", BF16)
            nc.tensor.transpose(pst[0:128, 0:ISH],
                                xmyn[:, ktl * 128:(ktl + 1) * 128],
                                ident_b[0:ISH, 0:ISH])
            ng = pa.tile([128, ISH], F32, name=f"negx2my{kt}")
            nc.vector.tensor_scalar_mul(ng, pst[0:128, 0:ISH], -2.0)
            negx2my[kt] = ng
    pA = ppt([128, N], "ps_A")
    pAm = ppt([128, ISH], "ps_Am")
    for m in range(NT):
        xq = pt.tile([128, N], BF16, name="vsq", tag="xq")
        nc.vector.tensor_tensor(xq, VP2_bf[m], VP2_bf[m], OP.mult)
        nc.tensor.matmul(pA, veW1_sl(m), xq[:, :], start=(m == 0),
                         stop=(m == NT - 1))
        xqm = pt.tile([128, ISH], BF16, name="vsqm", tag="vsqm")
        nc.vector.tensor_tensor(xqm, negx2my[m], negx2my[m], OP.mult)
        nc.tensor.matmul(pAm, veW1_sl(m), xqm[:, :], start=(m == 0),
                         stop=(m == NT - 1))
    # A_T_bf = A_j + b1 (bf16);  A_my = A_i (f32, (-2x)^2/4)
    A_T_bf = pa.tile([128, N], BF16, name="A_T_bf", tag="A_T")
    nc.vector.tensor_scalar(out=A_T_bf, in0=pA[:, 0:N],
                            scalar1=veb1_t[:, 0:1], scalar2=None, op0=OP.add)
    A_my = pa.tile([128, ISH], F32, name="A_my")
    nc.vector.tensor_scalar_mul(A_my, pAm, 0.25)
    if "AT" in dbg:
        atf = pt.tile([128, N], F32, name="atf", tag="zt1")
        nc.vector.tensor_copy(out=atf, in_=A_T_bf)
        dtap("AT", atf[0:128, 0:N])
        amf = pt.tile([128, ISH], F32, name="amf", tag="vsqm")
        nc.vector.tensor_scalar(out=amf, in0=A_my, scalar1=veb1_t[:, 0:1],
                                scalar2=None, op0=OP.add)
        dtap("Amy", amf[0:128, :])
    vedge_my = pa.tile([ISH, N], F32, name="vedge_my", tag="edg_se_0")
    psvm = ppt([ISH, N], "ps_vm")
    for mt, (i0, iw) in enumerate(N_MT):
        vb = pt.tile([128, N], BF16, name="vedgb", tag="xq")
        nc.vector.tensor_copy(out=vb[0:iw, :], in_=vedge[mt][0:iw, :])
        nc.tensor.matmul(psvm, LATE["selv_t"][mt][0:iw, :], vb[0:iw, :],
                         start=(mt == 0), stop=(mt == 1))
    nc.vector.tensor_copy(out=vedge_my, in_=psvm)

    # lhsT row-placement patterns: slice [ISH-1-k : 2*ISH-1-k] has ones (or
    # veW2) in column k, zeros elsewhere -> matmul accumulates into psum row k
    PAT = pa.tile([128, 2 * ISH - 1], BF16, name="PAT")
    nc.vector.memset(PAT, 0.0)
    nc.vector.memset(PAT[:, ISH - 1:ISH], 1.0)
    W2PAT = pa.tile([128, 2 * ISH - 1], BF16, name="W2PAT")
    nc.vector.memset(W2PAT, 0.0)
    nc.vector.tensor_copy(out=W2PAT[:, ISH - 1:ISH], in_=veW2_t[:, 0:1])
    # mean-over-H broadcast as a constant matmul: mu_b = (1/H) ones @ hsb
    ONESM = pa.tile([128, 128], BF16, name="ONESM")
    nc.vector.memset(ONESM, 1.0 / H)

    cur_ps = pps.tile([ISH, 256], F32, name="cur_ps", tag="cur")
    S_ps = ppt_s([ISH, 512], "st_e")
    for ii in range(ISH):
        psC = ppt([128, 256], "ps_C")
        for kt in range(NT):
            w1i = pstr.tile([128, H], BF16, name=f"w1i{kt}",
                            tag=f"w1i{kt % 8}")
            sc = negx2my[kt][:, ii:ii + 1]
            if kt % 3 == 0:
                nc.vector.tensor_scalar(out=w1i, in0=veW1_sl(kt),
                                        scalar1=sc, scalar2=None,
                                        op0=OP.mult)
            elif kt % 3 == 1:
                nc.gpsimd.tensor_scalar(out=w1i, in0=veW1_sl(kt),
                                        scalar1=sc, scalar2=None,
                                        op0=OP.mult)
            else:
                nc.scalar.activation(w1i, veW1_sl(kt), AF.Copy, scale=sc)
            nc.tensor.matmul(psC[:, 0:N], w1i, VP2_bf[kt],
                             start=(kt == 0), stop=(kt == NT - 1))
        # hsb = (-2C + A_i) + (A_j + b1):  Act (psum in) then DVE bf16 2x
        hp_ = pt.tile([128, N], BF16, name="ehp", tag="zt1")
        nc.scalar.activation(hp_, psC[:, 0:N], AF.Identity,
                             bias=A_my[:, ii:ii + 1], scale=1.0)
        hsb = pt.tile([128, N], BF16, name="ehsb", tag=f"hsb{ii % 2}")
        nc.vector.tensor_tensor(hsb, hp_, A_T_bf, OP.add)
        hsq = pt.tile([128, N], BF16, name="ehsq", tag="xq")
        nc.vector.tensor_tensor(hsq, hsb, hsb, OP.mult)
        psel = PAT[:, ISH - 1 - ii:2 * ISH - 1 - ii]
        nc.tensor.matmul(S_ps[0:ISH, 0:N], psel, hsb,
                         start=(ii == 0), stop=(ii == ISH - 1))
        nc.tensor.matmul(S_ps[0:ISH, 256:256 + N], psel, hsq,
                         start=(ii == 0), stop=(ii == ISH - 1))
        # centered h, relu; rs scaling deferred to cur rows (rs>0)
        pm = ppt_h([128, 256], "mu_b")
        nc.tensor.matmul(pm[:, 0:N], ONESM, hsb, start=True, stop=True)
        t1b = pt.tile([128, N], BF16, name="et1b", tag="zt1")
        nc.vector.tensor_tensor(t1b, hsb, pm[:, 0:N], OP.subtract)
        h2b = pt.tile([128, N], BF16, name="eh2b", tag="h2b")
        nc.vector.tensor_scalar_max(h2b, t1b, 0.0)
        nc.tensor.matmul(cur_ps[0:ISH, 0:N],
                         W2PAT[:, ISH - 1 - ii:2 * ISH - 1 - ii], h2b,
                         start=(ii == 0), stop=(ii == ISH - 1))
    # ---- batched row stats -> rs rows; cur = cur_raw * rs ----
    mu = pt1.tile([ISH, N], F32, name="emu", tag="emu")
    nc.vector.tensor_scalar_mul(mu, S_ps[0:ISH, 0:N], 1.0 / H)
    va = pt1.tile([ISH, N], F32, name="eva", tag="eva")
    nc.vector.tensor_scalar_mul(va, S_ps[0:ISH, 256:256 + N], 1.0 / H)
    musq = pt1.tile([ISH, N], F32, name="emusq", tag="emusq")
    nc.vector.tensor_tensor(musq, mu, mu, OP.mult)
    nc.vector.tensor_tensor(va, va, musq, OP.subtract)
    sdv = pt1.tile([ISH, N], F32, name="esdv", tag="emusq")
    nc.scalar.activation(sdv, va, AF.Sqrt, bias=epsc[0:ISH, 0:1], scale=1.0)
    rsr = pt1.tile([ISH, N], F32, name="ersr", tag="eva")
    nc.vector.reciprocal(out=rsr, in_=sdv)
    curm = pa.tile([ISH, N], F32, name="curm", tag="atf2")
    nc.vector.tensor_tensor(curm, cur_ps[0:ISH, 0:N], rsr, OP.mult)
    dtap("cur", curm[:, :])

    # tanh(cur + b2) * (vedge_my + 1e-8) -> softmax(/10) -> my edge rows
    curt = pa.tile([ISH, N], F32, name="curt", tag="atf0")
    nc.scalar.activation(curt, curm, AF.Tanh,
                         bias=veb2_t[0:ISH, 0:1], scale=1.0)
    ne = pa.tile([ISH, N], F32, name="ne", tag="atf1")
    nc.vector.scalar_tensor_tensor(out=ne, in0=vedge_my, scalar=1e-8,
                                   in1=curt, op0=OP.add, op1=OP.mult)
    rmx = pt.tile([ISH, 1], F32, name="vermx", tag="vermx")
    nc.vector.reduce_max(rmx, ne, axis=AX.X)
    bia = pt.tile([ISH, 1], F32, name="vebia", tag="vebia")
    nc.vector.tensor_scalar_mul(bia, rmx, -0.1)
    ex = pt1.tile([ISH, N], F32, name="veex", tag="veex")
    nc.scalar.activation(ex, ne, AF.Exp, bias=bia[0:ISH, 0:1], scale=0.1)
    sm = pt.tile([ISH, 1], F32, name="vesm", tag="vesm")
    nc.vector.reduce_sum(sm, ex, axis=AX.X)
    rr = pt.tile([ISH, 1], F32, name="verr", tag="verr")
    nc.vector.reciprocal(out=rr, in_=sm)
    vemine = pt1.tile([ISH, N], F16, name="vemine", tag="vemine")
    nc.vector.tensor_scalar(out=vemine, in0=ex, scalar1=rr[0:ISH, 0:1],
                            scalar2=None, op0=OP.mult)
    dma(out=ag_ve_in[:, :], in_=vemine)
    nc.gpsimd.collective_compute("AllGather", OP.bypass, replica_groups=GRP,
                                 ins=[ag_ve_in[:].opt()],
                                 outs=[ag_ve_out[:].opt()])

    # ---- AG(ve) shadow: fusion (VP2 half) + img @ VP2^T partials ----
    vu_sb = pt1.tile([1, 512], F32R, name="vu_sb", tag="vu_sb")
    nc.vector.memset(vu_sb[:].bitcast(F32), 0.0)
    pvu0 = ppt_s([1, 256], "ps_vu0")
    for m2 in range(2):
        ps = ppt([128, 256], "ps_fus")
        for kt in range(NT):
            nc.tensor.matmul(ps[:, 0:N],
                             LATE["fusWs_sl"](kt, m2 * 128, (m2 + 1) * 128),
                             VP2[kt][:, 0:N], start=(kt == 0),
                             stop=(kt == NT - 1))
        th = pt.tile([128, 256], F16, name="fth", tag="fth")
        nc.scalar.activation(th[:, 0:N], ps[:, 0:N], AF.Tanh)
        nc.tensor.matmul(pvu0[0:1, 0:N], fusUs_t[:, m2:m2 + 1], th[:, 0:N],
                         start=(m2 == 0), stop=(m2 == 1))
    nc.vector.tensor_copy(out=vu_sb[0:1, 0:N], in_=pvu0[0:1, 0:N])
    probv = []
    for bt in range(2):
        ps = ppt([128, N], "ps_prob")
        for kt in range(NT):
            nc.tensor.matmul(ps, imgT_sl(kt, bt), VP2[kt][:, 0:N],
                             start=(kt == 0), stop=(kt == NT - 1))
        pv_t = pa.tile([128, N], F32, name=f"probv{bt}")
        nc.scalar.copy(out=pv_t, in_=ps)
        probv.append(pv_t)
    if "ve2" in dbg:
        nc.sync.dma_start(out=dbg["ve2"].ap()[:, :], in_=ag_ve_out[:, :])
    ve2 = [pt1.tile([128, N], F16, name="ve2_0", tag="ve2_0"),
           pt1.tile([128, N], F16, name="ve2_1", tag="ve2_1")]
    for mt, (i0, iw) in enumerate(N_MT):
        dma(out=ve2[mt][0:iw, :], in_=ag_ve_out[i0:i0 + iw, :])
    ve2T = [pa.tile([128, 256], F16, name="ve2T0", tag="veT0"),
            pa.tile([128, 256], F16, name="ve2T1", tag="veT1")]
    for mt, (i0, iw) in enumerate(N_MT):
        for jb, (j0, jw) in enumerate(N_MT):
            pst = ppt_h([128, 128], "ps_tr6", F16)
            nc.tensor.transpose(pst[0:jw, 0:iw],
                                ve2[mt][0:iw, j0:j0 + jw],
                                ident_h[0:iw, 0:iw])
            nc.vector.tensor_copy(out=ve2T[jb][0:jw, i0:i0 + iw],
                                  in_=pst[0:jw, 0:iw])

    # =================================================================
    # P5: UpdateSemanticNode -> AllGather(z_s) -> SP2 (in-place set "za")
    #   (sps/zs_a were precomputed in the AR(h2) shadow)
    # =================================================================
    SP_n2 = transpose_nmajor(SP, "SP_n2", "nmj", F16)
    y2_r = []
    for m in range(NT):
        pes = pp.tile([128, 256], F32, name="ps_es", tag="mm")
        for jb, (j0, jw) in enumerate(N_MT):
            nc.tensor.matmul(pes[:, 0:N],
                             SP_n2[jb][0:jw, m * 128:(m + 1) * 128],
                             ve2T[jb][0:jw, 0:N], start=(jb == 0),
                             stop=(jb == 1))
        yb = sh1k(f"y2_{m}", m)
        nc.scalar.copy(out=yb[:, 0:N], in_=pes[:, 0:N])
        y2_r.append(yb)

    for m2 in range(2):
        ps = ppt([128, 256], "ps_zs")
        for kt in range(NT):
            nc.tensor.matmul(ps[:, 0:N], snWs_sl(kt, m2 * 128, (m2 + 1) * 128),
                             y2_r[kt][:, 0:N], start=(kt == 0),
                             stop=(kt == NT - 1))
        zc = pt.tile([128, N], F16, name="zsc", tag="zouts")
        nc.vector.tensor_tensor(zc, ps[:, 0:N], zsa[m2], OP.add)
        dma(out=ag_zs_in[m2 * 128:(m2 + 1) * 128, :], in_=zc)
    nc.gpsimd.collective_compute("AllGather", OP.bypass, replica_groups=GRP,
                                 ins=[ag_zs_in[:].opt()],
                                 outs=[ag_zs_out[:].opt()])

    def fin_sp2(m, z, t1):
        nc.vector.scalar_tensor_tensor(out=z[m][:, 0:N], in0=t1, scalar=0.0,
                                       in1=sp_f[m], op0=OP.max, op1=OP.add)
    SP2 = znorm(ag_zs_out, fin_sp2, "zs", "za")
    if "SP2" in dbg:
        for m in range(NT):
            nc.sync.dma_start(out=dbg["SP2"].ap()[m * 128:(m + 1) * 128, :],
                              in_=SP2[m][:, 0:N])

    # =================================================================
    # P7: fusion (SP2 half) -> AG(vu) || img @ SP2^T -> alpha -> prob
    # =================================================================
    pvu1 = ppt_s([1, 256], "ps_vu1")
    for m2 in range(2):
        ps = ppt([128, 256], "ps_fus")
        for kt in range(NT):
            nc.tensor.matmul(ps[:, 0:N],
                             LATE["fusWs_sl"](kt, m2 * 128, (m2 + 1) * 128),
                             SP2[kt][:, 0:N], start=(kt == 0),
                             stop=(kt == NT - 1))
        th = pt.tile([128, 256], F16, name="fth", tag="fth")
        nc.scalar.activation(th[:, 0:N], ps[:, 0:N], AF.Tanh)
        nc.tensor.matmul(pvu1[0:1, 0:N], fusUs_t[:, m2:m2 + 1], th[:, 0:N],
                         start=(m2 == 0), stop=(m2 == 1))
    nc.vector.tensor_copy(out=vu_sb[0:1, 256:256 + N], in_=pvu1[0:1, 0:N])
    dma(out=ag_vu_in[:, :], in_=vu_sb)
    nc.gpsimd.collective_compute("AllGather", OP.bypass, replica_groups=GRP,
                                 ins=[ag_vu_in[:].opt()],
                                 outs=[ag_vu_out[:].opt()])
    # AG(vu) shadow: img @ SP2^T partials
    probs = []
    for bt in range(2):
        ps = ppt([128, N], "ps_prob")
        for kt in range(NT):
            nc.tensor.matmul(ps, imgT_sl(kt, bt), SP2[kt][:, 0:N],
                             start=(kt == 0), stop=(kt == NT - 1))
        ps_t = pa.tile([128, N], F32, name=f"probs{bt}")
        nc.scalar.copy(out=ps_t, in_=ps)
        probs.append(ps_t)
    vus = pt1.tile([NCORES, 512], F32R, name="vus", tag="vu_sb")
    dma(out=vus, in_=ag_vu_out[:, :])
    pvk = ppt_s([1, 512], "ps_vuk")
    for k in range(2):
        nc.tensor.matmul(pvk[0:1, 256 * k:256 * k + 256],
                         ones8_r, vus[:, 256 * k:256 * k + 256],
                         start=True, stop=True)
    vuf = pt1.tile([1, 512], F32, name="vuf", tag="zstt")
    nc.vector.tensor_copy(out=vuf, in_=pvk)
    dtap("vuf", vuf[:, :])
    mx = pt.tile([1, N], F32, name="amx", tag="amx")
    nc.vector.tensor_tensor(mx, vuf[0:1, 0:N], vuf[0:1, 256:256 + N], OP.max)
    dv = pt1.tile([1, 512], F32R, name="adv", tag="adv")
    nc.vector.memset(dv[:].bitcast(F32), 0.0)
    for k in range(2):
        nc.vector.tensor_tensor(dv[0:1, 256 * k:256 * k + N],
                                vuf[0:1, 256 * k:256 * k + N], mx, OP.subtract)
    nc.scalar.activation(dv, dv, AF.Exp, scale=100.0)
    ssum = pt.tile([1, N], F32, name="assum", tag="assum")
    nc.vector.tensor_tensor(ssum, dv[0:1, 0:N], dv[0:1, 256:256 + N], OP.add)
    rsu = pt.tile([1, N], F32, name="arsu", tag="arsu")
    nc.vector.reciprocal(out=rsu, in_=ssum)
    for k in range(2):
        nc.vector.tensor_tensor(dv[0:1, 256 * k:256 * k + N],
                                dv[0:1, 256 * k:256 * k + N], rsu, OP.mult)
    alro = dv
    if "alpha" in dbg:
        al_f = pt1.tile([1, 512], F32, name="al_f", tag="zstt")
        nc.vector.tensor_copy(out=al_f, in_=alro)
        dtap("alpha", al_f[:, :])
    pal = ppt_h([128, 512], "ab_al")
    nc.tensor.matmul(pal, ones1_r, alro, start=True, stop=True)
    # prob = alpha_v * (img@VP2^T) + alpha_s * (img@SP2^T)
    for bt in range(2):
        t1 = pt.tile([128, N], F32, name="pr1", tag="zouts")
        nc.vector.tensor_tensor(t1, probv[bt], pal[:, 0:N], OP.mult)
        t2 = pt.tile([128, N], F32, name="pr2", tag="zt1")
        nc.vector.tensor_tensor(t2, probs[bt], pal[:, 256:256 + N], OP.mult)
        t3 = pt.tile([128, N], F32, name="probf", tag="zouts")
        nc.vector.tensor_tensor(t3, t1, t2, OP.add)
        dma(out=prob_out.ap()[bt * 128:(bt + 1) * 128, :], in_=t3)


# =====================================================================
# Host side
# =====================================================================
def _prep_inputs(inputs):
    bf = ml_dtypes.bfloat16
    f16 = np.float16
    f32 = np.float32
    att = np.asarray(inputs["attribute"], f32)
    cen = np.asarray(inputs["centers"], f32)
    expW = np.asarray(inputs["expert_W"], f32)
    expB = np.asarray(inputs["expert_b"], f32)
    w1 = np.asarray(inputs["s2v_W1"], f32)
    w2 = np.asarray(inputs["s2v_W2"], f32)
    in_maps = []
    for c in range(NCORES):
        cs = slice(c * DSH, (c + 1) * DSH)
        isl = slice(c * ISH, (c + 1) * ISH)
        bs = slice(c * BSH, (c + 1) * BSH)
        selv = np.zeros((N, ISH), f32)
        selv[np.arange(c * ISH, (c + 1) * ISH), np.arange(ISH)] = 1.0
        m = {
            "attrT": np.ascontiguousarray(att.T).astype(f16),
            "attrTb": np.ascontiguousarray(att.T).astype(bf),
            "centT": np.ascontiguousarray(cen.T),
            "expW": expW.astype(f16),
            "expBT": np.ascontiguousarray(expB.T),
            "w1s": np.ascontiguousarray(w1[:, cs]).astype(f16),
            "bnG": np.ascontiguousarray(np.asarray(inputs["bn_g"], f32)[cs].reshape(2, 128).T),
            "bnB": np.ascontiguousarray(np.asarray(inputs["bn_b"], f32)[cs].reshape(2, 128).T),
            "w2s": np.ascontiguousarray(w2[cs, :]).astype(f16),
            "b2o8": np.ascontiguousarray((np.asarray(inputs["s2v_b2"], f32) / NCORES).reshape(NT, 128).T),
            "vnWs": np.ascontiguousarray(np.asarray(inputs["vn_W"], f32)[:, cs]).astype(f16),
            "vnbs": np.ascontiguousarray(np.asarray(inputs["vn_b"], f32)[cs].reshape(2, 128).T),
            "snWs": np.ascontiguousarray(np.asarray(inputs["sn_W"], f32)[:, cs]).astype(f16),
            "snbs": np.ascontiguousarray(np.asarray(inputs["sn_b"], f32)[cs].reshape(2, 128).T),
            "veW1": np.asarray(inputs["ve_W1"], f32).astype(bf),
            "veb1": np.asarray(inputs["ve_b1"], f32)[:, None],
            "veW2": np.asarray(inputs["ve_W2"], f32).astype(bf),
            "veb2": np.full((ISH, 1), float(np.asarray(inputs["ve_b2"])[0]),
                            f32),
            "fusWs": np.ascontiguousarray(np.asarray(inputs["fus_W"], f32)[:, cs]).astype(f16),
            "fusUs": np.ascontiguousarray(np.asarray(inputs["fus_u"], f32)[cs, 0].reshape(2, 128).T).astype(f16),
            "imgT": np.ascontiguousarray(
                np.asarray(inputs["img_feat"], f32)[bs, :].T).astype(f16),
            "selv": selv.astype(bf),
        }
        in_maps.append(m)
    return in_maps


def kernel(**inputs):
    global _BUILT
    if _BUILT is None:
        _BUILT = build()
    nc = _BUILT
    in_maps = _prep_inputs(inputs)
    res = run_bass_kernel_spmd(nc, in_maps, core_ids=list(range(NCORES)))
    out = np.concatenate([res.results[c]["prob"] for c in range(NCORES)],
                         axis=0)
    return out.astype(np.float32)


def kernel_debug(**inputs):
    nc = build(debug=True)
    in_maps = _prep_inputs(inputs)
    res = run_bass_kernel_spmd(nc, in_maps, core_ids=list(range(NCORES)))
    out = np.concatenate([res.results[c]["prob"] for c in range(NCORES)],
                         axis=0)
    return out.astype(np.float32), res.results


if __name__ == "__main__":
    import reference
    inp = {k: np.asarray(v) for k, v in reference.setup_inputs().items()}
    got = kernel(**inp)
    exp = np.asarray(reference.reference(**reference.setup_inputs()))
    err = np.abs(got - exp).max() / (np.abs(exp).max() + 1e-9)
    print("Relative error:", err)

